# revision 1
# baseline (speedup 1.0000x reference)
"""Multi-head attention (B=2, S=2048, E=1024, H=16) on 8 Trainium2 NeuronCores.

Sharding: core c -> batch c//4, heads 4*(c%4)..4*(c%4)+3  (data + head parallel).
Each core computes a partial output projection [S, E] over its 256 head-dims;
the host sums the 4 partials per batch and adds the output bias (the
"all-reduce" happens in the unshard step).

On-chip layouts (contraction always on the partition dim, no on-chip
transposes; host pre-transposes query/key/value):
  QT, KT  [dim, S]   = Wx^T @ X^T      (rhs = X^T chunks streamed from HBM)
  V       [S, dim+ones]                 (natural; a ones column per head makes
                                         the PV matmul also emit softmax sums)
  scoresT [keys, q]  = KT_tile^T-block @ QT       per (head, q-group, key-tile)
  probsT  = exp(scoresT - 32)           (constant-shift softmax: exact up to
                                         fp32 rounding; masked lanes underflow
                                         to exactly 0 like the reference)
  attnoutT [d, q]    = (V|1)^T @ probsT (row 64 = Z = sum of probs)
  Zinv    = exp(-ln(Z))  on ACT;  broadcast across partitions via a K=1 matmul
  partial [S, E]     = attnoutT^T-chunks @ Wo-rows, accumulated over 4 heads

Matmul operands are bf16 (full PE rate); all accumulation is fp32 in PSUM.

Schedule (driven by TimelineSim cost-model profiling; this container has no
NTFF/neuron-profile path):
  phase A is DMA-bound: projections accumulate e-outer so the PE starts on
  the first arriving chunk; weight DMAs are interleaved with the X^T streams
  in consumption order.  The attention phase is ACT(exp)-bound: exp covers
  key-tile PAIRS (one ACT op per 2 tiles), masking is a post-exp 0/1
  multiply off the scores->exp chain, PV matmuls are software-pipelined one
  pair-group behind scores, and the four heads' Zinv chains are batched so
  the in-order ACT queue never stalls mid-stream.  Attention runs q-group
  outer (largest groups first) with the output projection emitted per
  q-group, sharing one 8-bank PSUM window (pv/b/o pools allocated before
  the scores pool so scores land on banks phase A never touched).
"""

import sys

for _p in ("/opt/trn_rl_repo", "/root/.axon_site/_ro/trn_rl_repo"):
    if _p not in sys.path:
        sys.path.insert(0, _p)

import numpy as np


# ---------------------------------------------------------------------------
# Patch: the walrus build in this container rejects >1 sem wait on one CTRL
# instruction ("Too many sync wait commands") and the TileContext exit drain
# aggregates every outstanding proc's wait onto a single Drain. Spill the
# excess waits onto SP nops (1 wait each) emitted right after the drain.
# ---------------------------------------------------------------------------
def _install_tile_drain_patch():
    import concourse.tile as tile
    import concourse.mybir as mybir
    from concourse.vector_clock import ScopedClock

    if getattr(tile.TileContext, "_drain_patch_installed", False):
        return

    def _patched_drain_and_barrier(self, tick_clock, wait_clock):
        drain_inst = self.nc.sync.drain()
        wait_clock.add_sem_waits(
            drain_inst.ins, ScopedClock({None: tick_clock.global_clock})
        )
        si = drain_inst.ins.sync_info
        waits = list(si.on_wait) if si and si.on_wait else []
        if len(waits) > 1:
            si.on_wait = waits[:1]
            for w in waits[1:]:
                nop = self.nc.sync.nop(nofuse=True, hint="drain_wait_spill")
                nop.ins.sync_info = mybir.SyncInfo(on_wait=[w], on_update=[])
        self.nc.all_engine_barrier()
        assert self.sems is not None
        popped = self.nc._tile_sem_poison_stack.pop()
        assert popped is self._sem_poison
        self.nc.clear_and_free_semaphores(list(self.sems.allocated().values()))
        self.nc.all_engine_barrier()

    tile.TileContext._drain_and_barrier = _patched_drain_and_barrier
    tile.TileContext._drain_patch_installed = True


def _split_multi_waits(nc, maxw=1):
    """Walrus here allows only `maxw` sem-wait commands per instruction.
    Hoist excess waits onto engine-queue NoOps inserted just before the
    instruction (the sequencer executes them in order, so semantics are
    identical)."""
    import concourse.mybir as mybir

    ctr = 0
    for bb in nc.main_func.blocks:
        new = []
        for inst in bb.instructions:
            si = inst.sync_info
            waits = list(si.on_wait) if si and si.on_wait else []
            if len(waits) > maxw:
                extras = waits[:-maxw]
                si.on_wait = waits[-maxw:]
                for i in range(0, len(extras), maxw):
                    nop = mybir.InstNoOp(
                        name=f"I-waitspill-{ctr}", engine=inst.engine,
                        ins=[], outs=[])
                    ctr += 1
                    nop.sync_info = mybir.SyncInfo(
                        on_wait=extras[i:i + maxw], on_update=[])
                    try:
                        nc.register_instruction(nop, overwrite=True)
                    except Exception:
                        pass
                    new.append(nop)
            new.append(inst)
        bb.instructions = new


# ---------------------------------------------------------------------------
# Mask classification (host side, from the actual mask array).
# Blocks are 128x128 in the *transposed* score layout: block (kt, qb) covers
# keys kt*128.. x queries qb*128... Returns per-block bias indices into a
# stack of unique additive-bias blocks (0 where attended, -1e9 where masked).
# ---------------------------------------------------------------------------
def classify_mask(mask2d, S, KB=128):
    nb = S // KB
    assert mask2d.shape == (S, S)
    assert mask2d.any(axis=1).all(), "a query row with no attended key"
    maskT = mask2d.T  # [keys, q]
    uniq = {}
    biases = []
    bias_idx = {}  # (kt, qb) -> None (all attended) or index
    block_live = np.zeros((nb, nb), dtype=bool)  # any attended key in block
    for kt in range(nb):
        for qb in range(nb):
            blk = maskT[kt * KB:(kt + 1) * KB, qb * KB:(qb + 1) * KB]
            if blk.all():
                bias_idx[(kt, qb)] = None
                block_live[kt, qb] = True
            else:
                b = np.where(blk, np.float32(1.0), np.float32(0.0))
                key = b.tobytes()
                if key not in uniq:
                    uniq[key] = len(biases)
                    biases.append(b)
                bias_idx[(kt, qb)] = uniq[key]
                block_live[kt, qb] = blk.any()
    return bias_idx, biases, block_live


# ---------------------------------------------------------------------------
# Bass program builder (one SPMD program, same for all cores).
# ---------------------------------------------------------------------------
def build_nc(S, E, D, HL, bias_idx, block_live, nuniq, shift=32.0, repeat=1):
    import concourse.bass as bass
    import concourse.mybir as mybir
    import concourse.tile as tile

    f32 = mybir.dt.float32
    bf16 = mybir.dt.bfloat16
    Act = mybir.ActivationFunctionType

    P = 128
    EC = E // P              # E chunks (contraction tiles for projections)
    DIM = HL * D             # this core's head dims (256)
    MT = DIM // P            # m-tiles of QT/KT (2)
    QG = 512                 # q-group width
    NQG = S // QG
    NKT = S // P             # key tiles
    NST = S // P             # s tiles
    VW = HL * (D + 1)        # V width incl. ones columns (260)
    EGW = min(QG, E)         # output E slice width
    NEG = E // EGW           # output E slices (2)

    # key tiles needed per q-group
    def kts_for_group(g):
        out = []
        for kt in range(NKT):
            if any(block_live[kt, g * (QG // P) + j] for j in range(QG // P)):
                out.append(kt)
        return out

    nc = bass.Bass()
    dp = nc.declare_dram_parameter
    d_xq = dp("xqT", [E, S], bf16, isOutput=False)
    d_xk = dp("xkT", [E, S], bf16, isOutput=False)
    d_xv = dp("xvT", [E, S], bf16, isOutput=False)
    d_wq = dp("wq", [E, DIM], bf16, isOutput=False)
    d_wk = dp("wk", [E, DIM], bf16, isOutput=False)
    d_wv = dp("wv", [E, VW], bf16, isOutput=False)
    d_wo = dp("wo", [DIM, E], bf16, isOutput=False)
    d_bias = dp("biasT", [P, max(nuniq, 1) * P], bf16, isOutput=False)
    d_out = dp("out_p", [S, E], f32, isOutput=True)

    import contextlib
    with tile.TileContext(nc) as tc, contextlib.ExitStack() as _stk:
        consts = _stk.enter_context(tc.tile_pool(name="consts", bufs=1))

        # weight tiles: [E, n] rearranged so one DMA loads all chunks
        # (chunk e lives at w_sb[:, e, :]).  DMA emission happens inside
        # emit_once, ordered so the first projection's weights land first.
        w_sb = {}
        for nm, width in (("wq", DIM), ("wk", DIM), ("wv", VW)):
            w_sb[nm] = consts.tile([P, EC, width], bf16, name=f"sb_{nm}",
                                   tag=f"sb_{nm}")
        w_dram = {"wq": d_wq, "wk": d_wk, "wv": d_wv}
        wo_sb = [consts.tile([2 * D, E], bf16, name=f"sb_wo{p}",
                             tag=f"sb_wo{p}") for p in range(HL // 2)]
        bias_sb = consts.tile([P, max(nuniq, 1) * P], bf16, name="sb_bias")
        ones128 = consts.tile([P, D], bf16, name="ones128")
        nc.vector.memset(ones128, 1.0)
        negshift = consts.tile([P, 1], f32, name="negshift")
        nc.vector.memset(negshift, -shift)

        def load_w(nm):
            nc.sync.dma_start(
                out=w_sb[nm],
                in_=w_dram[nm][:, :].rearrange("(e p) n -> p e n", p=P))

        def emit_once():
            # persistent projection outputs
            QT = [consts.tile([P, S], bf16, name=f"QT{m}", tag=f"QT{m}")
                  for m in range(MT)]
            KT = [consts.tile([P, S], bf16, name=f"KT{m}", tag=f"KT{m}")
                  for m in range(MT)]
            V = [consts.tile([P, VW], bf16, name=f"V{s}", tag=f"V{s}")
                 for s in range(NST)]

            # ---------------- phase A: projections ----------------
            # Order: V first, then QT/KT m-tile by m-tile, so attention on
            # heads 0/1 (m=0) can start while m=1 projections still run.
            with tc.tile_pool(name="xt", bufs=2 * EC + 4) as xt_pool, \
                 tc.tile_pool(name="psA", bufs=1, space="PSUM") as psA:

                def stream_chunks(dram):
                    chunks = []
                    for e in range(EC):
                        ch = xt_pool.tile([P, S], bf16, tag="xt", name=f"xch{e}")
                        nc.sync.dma_start(out=ch, in_=dram[e * P:(e + 1) * P, :])
                        chunks.append(ch)
                    return chunks

                # V natural [S, VW] = X chunks @ Wv.  e-outer with
                # st-blocks of 2: the first matmul only needs chunk 0, so PE
                # work starts as soon as the first DMA lands.
                load_w("wv")
                chunks = stream_chunks(d_xv)
                for stb in range(0, NST, 2):
                    pss = [psA.tile([P, VW], f32, tag=f"psA{i}",
                                    name=f"psv{i}") for i in range(2)]
                    for e in range(EC):
                        for i in range(2):
                            nc.tensor.matmul(
                                pss[i],
                                lhsT=chunks[e][:, (stb + i) * P:
                                               (stb + i + 1) * P],
                                rhs=w_sb["wv"][:, e, :],
                                start=(e == 0), stop=(e == EC - 1))
                    for i in range(2):
                        st = stb + i
                        nc.vector.tensor_copy(V[st], pss[i])
                        onescols = V[st].rearrange(
                            "p (h c) -> p h c", c=D + 1)[:, :, D]
                        nc.gpsimd.memset(onescols, 1.0)

                # QT / KT:  out^T [dim, S] = sum_e W[e-chunk, m-tile]^T-block
                load_w("wq")
                qchunks = stream_chunks(d_xq)
                load_w("wk")
                kchunks = stream_chunks(d_xk)
                for p in range(HL // 2):
                    nc.sync.dma_start(
                        out=wo_sb[p], in_=d_wo[p * 2 * D:(p + 1) * 2 * D, :])
                nc.sync.dma_start(out=bias_sb, in_=d_bias[:, :])
                for m in range(MT):
                    for wname, dst, chunks in (("wq", QT, qchunks),
                                               ("wk", KT, kchunks)):
                        pss = [psA.tile([P, QG], f32, tag=f"psA{g}",
                                        name=f"psqk{g}") for g in range(NQG)]
                        for e in range(EC):
                            for g in range(NQG):
                                nc.tensor.matmul(
                                    pss[g],
                                    lhsT=w_sb[wname][:, e, m * P:(m + 1) * P],
                                    rhs=chunks[e][:, g * QG:(g + 1) * QG],
                                    start=(e == 0), stop=(e == EC - 1))
                        for g in range(NQG):
                            nc.vector.tensor_copy(
                                dst[m][:, g * QG:(g + 1) * QG], pss[g])


            # ---------------- phase B: attention ----------------
            # attnT^T stored as head-PAIR tiles [128, S]: head 2p -> partitions
            # 0..63, head 2p+1 -> 64..127 (cross-partition DVE writes are
            # exact on HW), so the output projection contracts K=128.
            attnP = [[consts.tile([2 * D, QG], bf16, name=f"attnP{p}g{g}",
                                  tag=f"attnP{p}g{g}") for g in range(NQG)]
                     for p in range(HL // 2)]

            # key tiles processed in pairs: scores psum tile [128, 2*QG]
            # holds two key-tiles side by side in the free dim, so ONE exp
            # covers both (the +352-cycle ACT overhead halves).
            with tc.tile_pool(name="probs", bufs=6) as probs_pool, \
                 tc.tile_pool(name="evB", bufs=2) as evB_pool, \
                 tc.tile_pool(name="zrow", bufs=2) as z_pool, \
                 tc.tile_pool(name="outst", bufs=6) as out_pool, \
                 tc.tile_pool(name="psPV", bufs=2, space="PSUM") as psPV, \
                 tc.tile_pool(name="psB", bufs=1, space="PSUM") as psB, \
                 tc.tile_pool(name="psO", bufs=1, space="PSUM") as psO, \
                 tc.tile_pool(name="psS", bufs=2, space="PSUM") as psS:

                # g-outer / h-inner so each q-group's output projection can
                # be emitted (and thus queue-scheduled) right after the four
                # heads' attention for that group.
                # largest q-groups (most key tiles) first: the kernel tail
                # then ends on the smallest attention block
                g_order = sorted(range(NQG),
                                 key=lambda g: -len(kts_for_group(g)))
                pending_fin = []

                def flush_fin():
                    while pending_fin:
                        pending_fin.pop(0)()

                for g in g_order:
                    kts = kts_for_group(g)
                    pairs = [kts[i:i + 2] for i in range(0, len(kts), 2)]
                    evs = []
                    # two-head Z staging: head 2p -> row 0, head 2p+1 -> row 64
                    # (the only legal cross-partition offsets are 0/32/64)
                    zqs = [z_pool.tile([D + 1, QG], f32, tag=f"zq{p}",
                                       name=f"zq{p}") for p in range(HL // 2)]
                    for p in range(HL // 2):
                        nc.gpsimd.memset(zqs[p], 1.0)
                    for h in range(HL):
                        m, po = h // 2, (h % 2) * D
                        ev = evB_pool.tile([D, QG], f32, tag=f"ev{h}",
                                           name=f"ev{h}")
                        evs.append(ev)
                        pv = psPV.tile([D + 1, QG], f32, tag="pv")
                        npv = 0

                        def emit_pv(pair, pb):
                            nonlocal npv
                            for x, kt in enumerate(pair):
                                nc.tensor.matmul(
                                    pv,
                                    lhsT=V[kt][:, h * (D + 1):(h + 1) * (D + 1)],
                                    rhs=pb[:, x * QG:(x + 1) * QG],
                                    start=(npv == 0),
                                    stop=(npv == len(kts) - 1))
                                npv += 1

                        # software pipeline: PV for pair i is emitted after the
                        # scores+exp of pair i+1, so the in-order PE queue never
                        # blocks on ACT latency.
                        pend = []
                        for pair in pairs:
                            w = len(pair) * QG
                            sps = psS.tile([P, 2 * QG], f32, tag="s")
                            pb = probs_pool.tile([P, 2 * QG], bf16, tag="pb")
                            for x, kt in enumerate(pair):
                                nc.tensor.matmul(
                                    sps[:, x * QG:(x + 1) * QG],
                                    lhsT=KT[m][po:po + D, kt * P:(kt + 1) * P],
                                    rhs=QT[m][po:po + D, g * QG:(g + 1) * QG],
                                    start=True, stop=True)
                            nc.scalar.activation(pb[:, 0:w], sps[:, 0:w],
                                                 Act.Exp, bias=negshift[:, 0:1])
                            # masking applied AFTER exp (multiplicative 0/1,
                            # exact): partial blocks get a bf16 mask multiply,
                            # fully-masked blocks a gpsimd memset-zero.
                            for x, kt in enumerate(pair):
                                for j in range(QG // P):
                                    qb = g * (QG // P) + j
                                    bidx = bias_idx[(kt, qb)]
                                    if bidx is None:
                                        continue
                                    blkslice = pb[:, x * QG + j * P:
                                                  x * QG + (j + 1) * P]
                                    if not block_live[kt, qb]:
                                        nc.gpsimd.memset(blkslice, 0.0)
                                    else:
                                        nc.vector.tensor_mul(
                                            blkslice, blkslice,
                                            bias_sb[:, bidx * P:(bidx + 1) * P])
                            pend.append((pair, pb))
                            if len(pend) > 3:
                                emit_pv(*pend.pop(0))
                        while pend:
                            emit_pv(*pend.pop(0))
                        nc.vector.tensor_copy(ev, pv[0:D, :])
                        nc.vector.tensor_copy(
                            zqs[h // 2][(h % 2) * D:(h % 2) * D + 1, :],
                            pv[D:D + 1, :])
                        if h == 1:
                            flush_fin()

                    def _fin(g=g, evs=evs, zqs=zqs):
                        # Zinv = exp(-ln(Z)) for two heads per ACT op (rows 0 and
                        # 64 of the staging tile; ACT cost is free-dim driven so
                        # this halves the lane-starved Zinv chain count).
                        zbs = []
                        for p in range(HL // 2):
                            nc.scalar.activation(zqs[p], zqs[p], Act.Ln)
                            zb = z_pool.tile([D + 1, QG], bf16, tag=f"zb{p}",
                                             name=f"zb{p}")
                            nc.scalar.activation(zb, zqs[p], Act.Exp, scale=-1.0)
                            zbs.append(zb)
                        for h in range(HL):
                            po = (h % 2) * D
                            bps = psB.tile([D, QG], f32, tag="b")
                            nc.tensor.matmul(
                                bps,
                                lhsT=ones128[po:po + 1, :],
                                rhs=zbs[h // 2][po:po + 1, :],
                                start=True, stop=True)
                            nc.vector.tensor_mul(
                                attnP[h // 2][g][po:po + D, :], evs[h], bps)

                        # ---- output projection for this q-group ----
                        for j in range(QG // P):
                            st = g * (QG // P) + j
                            off = j * P
                            ot = out_pool.tile([P, NEG * EGW], f32, tag="ot")
                            for eg in range(NEG):
                                ops = psO.tile([P, EGW], f32, tag="o", name="opso")
                                for p in range(HL // 2):
                                    nc.tensor.matmul(
                                        ops,
                                        lhsT=attnP[p][g][:, off:off + P],
                                        rhs=wo_sb[p][:, eg * EGW:(eg + 1) * EGW],
                                        start=(p == 0), stop=(p == HL // 2 - 1))
                                nc.vector.tensor_copy(
                                    ot[:, eg * EGW:(eg + 1) * EGW], ops)
                            nc.sync.dma_start(
                                out=d_out[st * P:(st + 1) * P, :], in_=ot)
                    pending_fin.append(_fin)

                flush_fin()



        for _rep in range(repeat):
            emit_once()

    _split_multi_waits(nc)
    return nc


# ---------------------------------------------------------------------------
# Host entry point
# ---------------------------------------------------------------------------
LAST_EXEC_NS = None
LAST_RESULT = None


def kernel(query, key, value, mask, Wq, Wk, Wv, Wo, bo):
    global LAST_EXEC_NS, LAST_RESULT
    _install_tile_drain_patch()
    from concourse.bass_utils import run_bass_kernel_spmd

    B, S, E = 2, 2048, 1024
    H, D = 16, 64
    N_CORES = 8
    BG = 2                    # batch groups
    HG = N_CORES // BG        # head groups per batch
    HL = H // HG              # heads per core
    DIM = HL * D

    query = np.asarray(query, dtype=np.float32)
    key = np.asarray(key, dtype=np.float32)
    value = np.asarray(value, dtype=np.float32)
    mask2d = np.asarray(mask).reshape(S, S).astype(bool)
    Wq = np.asarray(Wq, dtype=np.float32)
    Wk = np.asarray(Wk, dtype=np.float32)
    Wv = np.asarray(Wv, dtype=np.float32)
    Wo = np.asarray(Wo, dtype=np.float32)
    bo = np.asarray(bo, dtype=np.float32)

    bias_idx, biases, block_live = classify_mask(mask2d, S)
    nuniq = len(biases)
    bias_stack = (np.concatenate(biases, axis=1) if nuniq
                  else np.zeros((128, 128), np.float32))

    nc = build_nc(S, E, D, HL, bias_idx, block_live, nuniq)

    scale = np.float32(1.0 / np.sqrt(D))
    in_maps = []
    for c in range(N_CORES):
        b, hg = c // HG, c % HG
        cols = slice(hg * DIM, (hg + 1) * DIM)
        wv_l = Wv[:, cols].reshape(E, HL, D)
        wv_aug = np.zeros((E, HL, D + 1), np.float32)
        wv_aug[:, :, :D] = wv_l
        in_maps.append({
            "xqT": _bf16(query[b].T),
            "xkT": _bf16(key[b].T),
            "xvT": _bf16(value[b].T),
            "wq": _bf16(Wq[:, cols] * scale),
            "wk": _bf16(Wk[:, cols]),
            "wv": _bf16(wv_aug.reshape(E, HL * (D + 1))),
            "wo": _bf16(Wo[cols, :]),
            "biasT": _bf16(bias_stack),
        })

    res = run_bass_kernel_spmd(nc, in_maps, list(range(N_CORES)))
    LAST_RESULT = res
    LAST_EXEC_NS = res.exec_time_ns or res.mean_exec_time_ns

    out = np.empty((B, S, E), np.float32)
    for b in range(BG):
        acc = res.results[b * HG]["out_p"].astype(np.float32)
        for j in range(1, HG):
            acc = acc + res.results[b * HG + j]["out_p"]
        out[b] = acc + bo[None, :]
    return out


def _bf16(a):
    import ml_dtypes
    return np.ascontiguousarray(np.asarray(a, np.float32)).astype(
        ml_dtypes.bfloat16)



# revision 5
# speedup vs baseline: 1.1423x; 1.1423x over previous
"""Multi-head attention (B=2, S=2048, E=1024, H=16) on 8 Trainium2 NeuronCores.

Sharding: core c -> batch c//4, heads 4*(c%4)..4*(c%4)+3  (data + head parallel).
Each core computes a partial output projection [S, E] over its 256 head-dims;
the host sums the 4 partials per batch and adds the output bias.

v2 redesign (driven by the TimelineSim cost model, where every engine op
costs free_size * cycle_t and matmuls cost out_free * 0.4167ns per
contraction chunk; ~149us vs the 179us v1):
  * scores/exp/PV are TRIMMED to the live (causal) columns of each key
    tile: per (group, kt) only q-columns >= the first live q-block are
    computed, masked blocks inside that range get the 0/1 multiply.
  * Zinv = DVE reciprocal of the PV ones-row (bf16), broadcast across
    partitions by a K=1 matmul into rows 64..127 of the SAME pv psum bank
    (replaces the Ln/Exp chains + staging copies; GPSIMD can't touch PSUM
    and partition_broadcast needs a gpsimd library, so neither is used).
  * Output partials leave in bf16 (halves the out-DMA; the host sums the
    4 partials per batch in fp32), psum->sbuf copies split ACT/DVE.
  * Phase order: V e-inner (PE runs continuously once xv lands, ramping
    to the full p-state), m0 e-outer, pass 1 (heads 0/1, groups
    ascending) with the 8 m1-projection pieces woven between later head
    visits, pass 2 (heads 2/3, descending) with each group's output
    projection deferred into the next group's emission.  Every head's PV
    tail + normalize chain is carried into the following head's first
    score matmuls so the in-order engine queues never stall on them; the
    last group batches its chains, runs Zinv on the idle ACT engine, and
    drains its output projection through the freed scores psum tiles.
"""

import sys

for _p in ("/opt/trn_rl_repo", "/root/.axon_site/_ro/trn_rl_repo"):
    if _p not in sys.path:
        sys.path.insert(0, _p)

import numpy as np


# ---------------------------------------------------------------------------
# Patch: the walrus build in this container rejects >1 sem wait on one CTRL
# instruction and the TileContext exit drain aggregates every outstanding
# proc's wait onto a single Drain. Spill the excess waits onto SP nops.
# ---------------------------------------------------------------------------
def _install_tile_drain_patch():
    import concourse.tile as tile
    import concourse.mybir as mybir
    from concourse.vector_clock import ScopedClock

    if getattr(tile.TileContext, "_drain_patch_installed", False):
        return

    def _patched_drain_and_barrier(self, tick_clock, wait_clock):
        drain_inst = self.nc.sync.drain()
        wait_clock.add_sem_waits(
            drain_inst.ins, ScopedClock({None: tick_clock.global_clock})
        )
        si = drain_inst.ins.sync_info
        waits = list(si.on_wait) if si and si.on_wait else []
        if len(waits) > 1:
            si.on_wait = waits[:1]
            for w in waits[1:]:
                nop = self.nc.sync.nop(nofuse=True, hint="drain_wait_spill")
                nop.ins.sync_info = mybir.SyncInfo(on_wait=[w], on_update=[])
        self.nc.all_engine_barrier()
        assert self.sems is not None
        popped = self.nc._tile_sem_poison_stack.pop()
        assert popped is self._sem_poison
        self.nc.clear_and_free_semaphores(list(self.sems.allocated().values()))
        self.nc.all_engine_barrier()

    tile.TileContext._drain_and_barrier = _patched_drain_and_barrier
    tile.TileContext._drain_patch_installed = True


def _split_multi_waits(nc, maxw=1):
    """Hoist excess sem waits onto engine-queue NoOps inserted just before
    the instruction (sequencer executes them in order; semantics identical)."""
    import concourse.mybir as mybir

    ctr = 0
    for bb in nc.main_func.blocks:
        new = []
        for inst in bb.instructions:
            si = inst.sync_info
            waits = list(si.on_wait) if si and si.on_wait else []
            if len(waits) > maxw:
                extras = waits[:-maxw]
                si.on_wait = waits[-maxw:]
                for i in range(0, len(extras), maxw):
                    nop = mybir.InstNoOp(
                        name=f"I-waitspill-{ctr}", engine=inst.engine,
                        ins=[], outs=[])
                    ctr += 1
                    nop.sync_info = mybir.SyncInfo(
                        on_wait=extras[i:i + maxw], on_update=[])
                    try:
                        nc.register_instruction(nop, overwrite=True)
                    except Exception:
                        pass
                    new.append(nop)
            new.append(inst)
        bb.instructions = new


# ---------------------------------------------------------------------------
# Mask classification (host side, from the actual mask array).
# Blocks are 128x128 in the *transposed* score layout: block (kt, qb) covers
# keys kt*128.. x queries qb*128...
# ---------------------------------------------------------------------------
def classify_mask(mask2d, S, KB=128):
    nb = S // KB
    assert mask2d.shape == (S, S)
    assert mask2d.any(axis=1).all(), "a query row with no attended key"
    maskT = mask2d.T  # [keys, q]
    uniq = {}
    biases = []
    bias_idx = {}  # (kt, qb) -> None (all attended) or index
    block_live = np.zeros((nb, nb), dtype=bool)  # any attended key in block
    for kt in range(nb):
        for qb in range(nb):
            blk = maskT[kt * KB:(kt + 1) * KB, qb * KB:(qb + 1) * KB]
            if blk.all():
                bias_idx[(kt, qb)] = None
                block_live[kt, qb] = True
            else:
                b = np.where(blk, np.float32(1.0), np.float32(0.0))
                key = b.tobytes()
                if key not in uniq:
                    uniq[key] = len(biases)
                    biases.append(b)
                bias_idx[(kt, qb)] = uniq[key]
                block_live[kt, qb] = blk.any()
    return bias_idx, biases, block_live


# ---------------------------------------------------------------------------
# Bass program builder (one SPMD program, same for all cores).
# ---------------------------------------------------------------------------
def build_nc(S, E, D, HL, bias_idx, block_live, nuniq, shift=32.0, repeat=1):
    import concourse.bass as bass
    import concourse.mybir as mybir
    import concourse.tile as tile

    f32 = mybir.dt.float32
    bf16 = mybir.dt.bfloat16
    Act = mybir.ActivationFunctionType

    P = 128
    EC = E // P              # E chunks (contraction tiles for projections)
    DIM = HL * D             # this core's head dims (256)
    MT = DIM // P            # m-tiles of QT/KT (2)
    QG = 512                 # q-group width
    NQG = S // QG
    NQB = QG // P            # q-blocks per group
    NKT = S // P             # key tiles
    NST = S // P             # s tiles
    VW = HL * (D + 1)        # V width incl. ones columns (260)
    EGW = min(QG, E)         # output E slice width
    NEG = E // EGW           # output E slices (2)

    # live key tiles of group g with the first live q-block column offset;
    # the first entry is widened to cover every column any later kt writes so
    # its start=True matmul initializes the whole accumulation region.
    def kt_offs(g):
        out = []
        for kt in range(NKT):
            lives = [j for j in range(NQB) if block_live[kt, g * NQB + j]]
            if lives:
                out.append((kt, lives[0] * P))
        if out:
            m0 = min(o for _, o in out)
            out[0] = (out[0][0], m0)
        return out

    nc = bass.Bass()
    dp = nc.declare_dram_parameter
    d_xq = dp("xqT", [E, S], bf16, isOutput=False)
    d_xk = dp("xkT", [E, S], bf16, isOutput=False)
    d_xv = dp("xvT", [E, S], bf16, isOutput=False)
    d_wq = dp("wq", [E, DIM], bf16, isOutput=False)
    d_wk = dp("wk", [E, DIM], bf16, isOutput=False)
    d_wv = dp("wv", [E, VW], bf16, isOutput=False)
    d_wo = dp("wo", [DIM, E], bf16, isOutput=False)
    d_bias = dp("biasT", [P, max(nuniq, 1) * P], bf16, isOutput=False)
    d_out = dp("out_p", [S, E], bf16, isOutput=True)

    import contextlib
    with tile.TileContext(nc) as tc, contextlib.ExitStack() as _stk:
        consts = _stk.enter_context(tc.tile_pool(name="consts", bufs=1))

        w_sb = {}
        for nm, width in (("wq", DIM), ("wk", DIM), ("wv", VW)):
            w_sb[nm] = consts.tile([P, EC, width], bf16, name=f"sb_{nm}",
                                   tag=f"sb_{nm}")
        w_dram = {"wq": d_wq, "wk": d_wk, "wv": d_wv}
        wo_sb = [consts.tile([2 * D, E], bf16, name=f"sb_wo{p}",
                             tag=f"sb_wo{p}") for p in range(HL // 2)]
        bias_sb = consts.tile([P, max(nuniq, 1) * P], bf16, name="sb_bias")
        negshift = consts.tile([P, 1], f32, name="negshift")
        nc.vector.memset(negshift, -shift)
        ones64 = consts.tile([1, D], bf16, name="ones64")
        nc.vector.memset(ones64, 1.0)

        def load_w(nm):
            nc.sync.dma_start(
                out=w_sb[nm],
                in_=w_dram[nm][:, :].rearrange("(e p) n -> p e n", p=P))

        def emit_once():
            # persistent projection outputs
            QT = [consts.tile([P, S], bf16, name=f"QT{m}", tag=f"QT{m}")
                  for m in range(MT)]
            KT = [consts.tile([P, S], bf16, name=f"KT{m}", tag=f"KT{m}")
                  for m in range(MT)]
            V = [consts.tile([P, VW], bf16, name=f"V{s}", tag=f"V{s}")
                 for s in range(NST)]
            attnP = [[consts.tile([2 * D, QG], bf16, name=f"attnP{p}g{g}",
                                  tag=f"attnP{p}g{g}") for g in range(NQG)]
                     for p in range(HL // 2)]

            with tc.tile_pool(name="xt", bufs=2 * EC + 4) as xt_pool:
                psA_ctx = contextlib.ExitStack()
                psV = psA_ctx.enter_context(
                    tc.tile_pool(name="psV", bufs=1, space="PSUM"))
                psM = psA_ctx.enter_context(
                    tc.tile_pool(name="psM", bufs=1, space="PSUM"))

                def stream_chunks(dram):
                    chunks = []
                    for e in range(EC):
                        ch = xt_pool.tile([P, S], bf16, tag="xt", name=f"xch{e}")
                        nc.sync.dma_start(out=ch, in_=dram[e * P:(e + 1) * P, :])
                        chunks.append(ch)
                    return chunks

                # ---------------- phase A: projections ----------------
                # wv chunk 0 first so the first V matmul only waits ~1.6us
                nc.sync.dma_start(out=w_sb["wv"][:, 0, :], in_=d_wv[0:P, :])
                vchunks = [xt_pool.tile([P, S], bf16, tag="xt", name="xch0")]
                nc.sync.dma_start(out=vchunks[0], in_=d_xv[0:P, :])
                nc.sync.dma_start(
                    out=w_sb["wv"][:, 1:EC, :],
                    in_=d_wv[P:, :].rearrange("(e p) n -> p e n", p=P))
                for e in range(1, EC):
                    ch = xt_pool.tile([P, S], bf16, tag="xt", name=f"xch{e}")
                    nc.sync.dma_start(out=ch, in_=d_xv[e * P:(e + 1) * P, :])
                    vchunks.append(ch)
                load_w("wq")
                qchunks = stream_chunks(d_xq)
                load_w("wk")
                kchunks = stream_chunks(d_xk)
                for p in range(HL // 2):
                    nc.sync.dma_start(
                        out=wo_sb[p], in_=d_wo[p * 2 * D:(p + 1) * 2 * D, :])
                nc.sync.dma_start(out=bias_sb, in_=d_bias[:, :])

                def finish_v(st, ps):
                    nc.vector.tensor_copy(V[st], ps)
                    onescols = V[st].rearrange(
                        "p (h c) -> p h c", c=D + 1)[:, :, D]
                    nc.gpsimd.memset(onescols, 1.0)

                # V tiles e-inner, starting once the whole xv stream has
                # landed (~11.4us): the PE then runs CONTINUOUSLY (ramping to
                # the full p-state) through V and the m0 projections while
                # xq/xk stream in, instead of chunk-paced stuttering.
                for st in range(0, NST):
                    ps = psV.tile([P, VW], f32, tag=f"v{st % 4}",
                                  name=f"psv{st}")
                    for e in range(EC):
                        nc.tensor.matmul(
                            ps,
                            lhsT=vchunks[e][:, st * P:(st + 1) * P],
                            rhs=w_sb["wv"][:, e, :],
                            start=(e == 0), stop=(e == EC - 1))
                    finish_v(st, ps)

                # QK projections for m-tile m into QT[m]/KT[m]; e-outer over
                # the 4 q-groups of each (weight, group) so PE work is
                # chunk-paced.  `pool`/`tag` chooses the psum bank set.
                def qk_proj(m, pool, tagf):
                    for wname, dst, chunks in (("wq", QT, qchunks),
                                               ("wk", KT, kchunks)):
                        pss = [pool.tile([P, QG], f32, tag=tagf(g),
                                         name=f"psqk{m}{wname}{g}")
                               for g in range(NQG)]
                        for e in range(EC):
                            for g in range(NQG):
                                nc.tensor.matmul(
                                    pss[g],
                                    lhsT=w_sb[wname][:, e, m * P:(m + 1) * P],
                                    rhs=chunks[e][:, g * QG:(g + 1) * QG],
                                    start=(e == 0), stop=(e == EC - 1))
                        for g in range(NQG):
                            nc.vector.tensor_copy(
                                dst[m][:, g * QG:(g + 1) * QG], pss[g])

                qk_proj(0, psM, lambda g: f"m{g}")
                psA_ctx.close()  # free the 8 phase-A PSUM banks for phase B

                # ---------------- phase B: attention ----------------
                # PSUM bank alignment: psPV/psO open first so they land on
                # the psV banks (free early); psS lands on the psM banks,
                # which free exactly when the m0 copies complete.
                with tc.tile_pool(name="probs", bufs=6) as probs_pool, \
                     tc.tile_pool(name="zrow", bufs=2) as z_pool, \
                     tc.tile_pool(name="evs", bufs=2) as ev_pool, \
                     tc.tile_pool(name="outst", bufs=4) as out_pool, \
                     tc.tile_pool(name="psS", bufs=2, space="PSUM") as psS, \
                     tc.tile_pool(name="psPV", bufs=2, space="PSUM") as psPV, \
                     tc.tile_pool(name="psO", bufs=2, space="PSUM") as psO:

                    def attn(g, h, carry=(), defer=False, act_recip=False):
                        """Emit attention for (g, h).  `carry` holds the
                        previous head's deferred PV-flush + normalize chain;
                        it is emitted right after this head's first score
                        matmuls so the ACT engine sees the next exp without
                        waiting for the previous head's PV tail.  With
                        defer=True the tail thunks are returned instead of
                        emitted."""
                        m, po = h // 2, (h % 2) * D
                        kts = kt_offs(g)
                        total = len(kts)
                        pairs = [kts[i:i + 2] for i in range(0, total, 2)]
                        pv = psPV.tile([D + 1, QG], f32, tag="pv")
                        npv = 0
                        pend = []
                        carried = list(carry)

                        def emit_pv(entry):
                            nonlocal npv
                            pb, regions = entry
                            for (kt, off), c, w in regions:
                                nc.tensor.matmul(
                                    pv[0:D + 1, off:QG],
                                    lhsT=V[kt][:, h * (D + 1):
                                               (h + 1) * (D + 1)],
                                    rhs=pb[:, c:c + w],
                                    start=(npv == 0),
                                    stop=(npv == total - 1),
                                    skip_group_check=True)
                                npv += 1

                        for pi, pair in enumerate(pairs):
                            regions = []
                            col = 0
                            for (kt, off) in pair:
                                w = QG - off
                                regions.append(((kt, off), col, w))
                                col += w
                            sps = psS.tile([P, 2 * QG], f32, tag="s")
                            pb = probs_pool.tile([P, 2 * QG], bf16, tag="pb")
                            for (kt, off), c, w in regions:
                                nc.tensor.matmul(
                                    sps[:, c:c + w],
                                    lhsT=KT[m][po:po + D, kt * P:(kt + 1) * P],
                                    rhs=QT[m][po:po + D,
                                              g * QG + off:(g + 1) * QG],
                                    start=True, stop=True)
                            if pi == 0:
                                while carried:
                                    carried.pop(0)()
                            nc.scalar.activation(pb[:, 0:col], sps[:, 0:col],
                                                 Act.Exp,
                                                 bias=negshift[:, 0:1])
                            # masking after exp: multiplicative 0/1, exact
                            for (kt, off), c, w in regions:
                                for j in range(off // P, NQB):
                                    qb = g * NQB + j
                                    bidx = bias_idx[(kt, qb)]
                                    if bidx is None:
                                        continue
                                    cc = c + j * P - off
                                    blk = pb[:, cc:cc + P]
                                    if not block_live[kt, qb]:
                                        nc.gpsimd.memset(blk, 0.0)
                                    else:
                                        nc.vector.tensor_mul(
                                            blk, blk,
                                            bias_sb[:, bidx * P:
                                                    (bidx + 1) * P])
                            pend.append((pb, regions))
                            if len(pend) > 2:
                                emit_pv(pend.pop(0))

                        def t_pv():
                            while pend:
                                emit_pv(pend.pop(0))

                        # normalize: Zinv = 1/Z (bf16, matching the
                        # reference-passing baseline's precision), broadcast
                        # across partitions by a K=1 matmul into a borrowed
                        # psO bank; ev leaves PSUM via the ACT engine so the
                        # final multiply has baseline-shaped operands
                        # (SBUF x PSUM).
                        def t_chain():
                            zi = z_pool.tile([1, QG], bf16, tag=f"zi{h % 2}",
                                             name=f"zi{h}")
                            if act_recip:
                                # tail path: ACT is idle there; Zinv =
                                # exp(-ln(Z)) is exact to bf16 rounding
                                zf = z_pool.tile([1, QG], f32,
                                                 tag=f"zf{h % 2}",
                                                 name=f"zf{h}")
                                nc.scalar.activation(zf, pv[D:D + 1, :],
                                                     Act.Ln)
                                nc.scalar.activation(zi, zf, Act.Exp,
                                                     scale=-1.0)
                            else:
                                with nc.allow_low_precision(
                                        reason="bf16 Zinv, like the Wo "
                                               "operands downstream"):
                                    nc.vector.reciprocal(zi, pv[D:D + 1, :])
                            ev = ev_pool.tile([D, QG], f32,
                                              tag=f"ev{h % 2}", name=f"ev{h}")
                            nc.scalar.copy(ev, pv[0:D, :])
                            bps = psO.tile([D, QG], f32, tag="o",
                                           name=f"bps{h}")
                            nc.tensor.matmul(bps, lhsT=ones64, rhs=zi,
                                             start=True, stop=True)
                            nc.vector.tensor_mul(
                                attnP[h // 2][g][po:po + D, :], ev, bps)

                        if defer:
                            return [t_pv, t_chain]
                        t_pv()
                        t_chain()
                        return []

                    def wo_proj(g, last=False):
                        # For the final group the psS pool is free (no more
                        # scores), so use its 2-bank tiles to double the
                        # psum buffering and shorten the drain.
                        for j in range(NQB):
                            st = g * NQB + j
                            ot = out_pool.tile([P, NEG * EGW], bf16, tag="ot")
                            if last:
                                ops2 = psS.tile([P, 2 * QG], f32, tag="s",
                                                name="opss")
                            for eg in range(NEG):
                                if last:
                                    ops = ops2[:, eg * EGW:(eg + 1) * EGW]
                                else:
                                    ops = psO.tile([P, EGW], f32, tag="o",
                                                   name="opso")
                                for p in range(HL // 2):
                                    nc.tensor.matmul(
                                        ops,
                                        lhsT=attnP[p][g][:, j * P:(j + 1) * P],
                                        rhs=wo_sb[p][:, eg * EGW:
                                                     (eg + 1) * EGW],
                                        start=(p == 0),
                                        stop=(p == HL // 2 - 1),
                                        skip_group_check=last)
                                # copy PSUM->SBUF (GPSIMD cannot touch PSUM);
                                # at the tail the idle ACT engine takes half
                                dst = ot[:, eg * EGW:(eg + 1) * EGW]
                                if eg == 0:
                                    nc.scalar.copy(dst, ops)
                                else:
                                    nc.vector.tensor_copy(dst, ops)
                                if last:
                                    nc.sync.dma_start(
                                        out=d_out[st * P:(st + 1) * P,
                                                  eg * EGW:(eg + 1) * EGW],
                                        in_=dst)
                            if not last:
                                nc.sync.dma_start(
                                    out=d_out[st * P:(st + 1) * P, :], in_=ot)

                    # m1 projection piece for one (weight, group): psum
                    # borrowed from the (pass-1-unused) psO pool; e-inner so
                    # the two psO buffers ping-pong.
                    def m1_piece(wname, g):
                        dst = QT if wname == "wq" else KT
                        chunks = qchunks if wname == "wq" else kchunks
                        ps = psO.tile([P, QG], f32, tag="o",
                                      name=f"psqk1{wname}{g}")
                        for e in range(EC):
                            nc.tensor.matmul(
                                ps,
                                lhsT=w_sb[wname][:, e, P:2 * P],
                                rhs=chunks[e][:, g * QG:(g + 1) * QG],
                                start=(e == 0), stop=(e == EC - 1))
                        nc.vector.tensor_copy(
                            dst[1][:, g * QG:(g + 1) * QG], ps)

                    # pass 1: heads 0/1 over all groups (needs only m0 + V),
                    # ascending (ends on the biggest exp backlog).  The 8 m1
                    # projection pieces are woven between the later head
                    # visits, where the exp backlog hides their ACT-less PE
                    # time; PV tails and normalize chains are carried into
                    # the next head so ACT never waits on them.
                    g_up = sorted(range(NQG), key=lambda g: len(kt_offs(g)))
                    m1_sched = {2: ["wq0"], 3: ["wq1"],
                                4: ["wq2", "wq3"], 5: ["wk0", "wk1"],
                                6: ["wk2"], 7: ["wk3"]}
                    carry = []
                    for i, g in enumerate(g_up):
                        for h in (0, 1):
                            carry = attn(g, h, carry=carry, defer=True)
                            for pc in m1_sched.get(2 * i + h, []):
                                m1_piece("wq" if pc[:2] == "wq" else "wk",
                                         int(pc[2]))
                    for t in carry:
                        t()

                    # pass 2: heads 2/3 + output projection per group;
                    # descending so the big group lands right after m1 and
                    # the kernel tail ends on the smallest one.  Each group's
                    # h3 tail + output projection are carried into the next
                    # group's first score matmuls; the last group runs its
                    # reciprocals on the (by then idle) ACT engine and
                    # interleaves its two normalize chains to cut the drain.
                    carry = []
                    gs2 = list(reversed(g_up))
                    for i, g in enumerate(gs2):
                        last = (i == NQG - 1)
                        c2 = attn(g, 2, carry=carry, defer=True,
                                  act_recip=last)
                        c3 = attn(g, 3, carry=c2[:1], defer=True,
                                  act_recip=last)
                        if not last:
                            # [pv3, chain2, chain3, wo(g)] ride into the next
                            # group's emission
                            carry = [c3[0], c2[1], c3[1],
                                     lambda g=g: wo_proj(g)]
                        else:
                            c3[0]()
                            c2[1]()
                            c3[1]()
                            wo_proj(g, last=True)

        for _rep in range(repeat):
            emit_once()

    _split_multi_waits(nc)
    return nc


# ---------------------------------------------------------------------------
# Host entry point
# ---------------------------------------------------------------------------
LAST_EXEC_NS = None
LAST_RESULT = None


def kernel(query, key, value, mask, Wq, Wk, Wv, Wo, bo):
    global LAST_EXEC_NS, LAST_RESULT
    _install_tile_drain_patch()
    from concourse.bass_utils import run_bass_kernel_spmd

    B, S, E = 2, 2048, 1024
    H, D = 16, 64
    N_CORES = 8
    BG = 2                    # batch groups
    HG = N_CORES // BG        # head groups per batch
    HL = H // HG              # heads per core
    DIM = HL * D

    query = np.asarray(query, dtype=np.float32)
    key = np.asarray(key, dtype=np.float32)
    value = np.asarray(value, dtype=np.float32)
    mask2d = np.asarray(mask).reshape(S, S).astype(bool)
    Wq = np.asarray(Wq, dtype=np.float32)
    Wk = np.asarray(Wk, dtype=np.float32)
    Wv = np.asarray(Wv, dtype=np.float32)
    Wo = np.asarray(Wo, dtype=np.float32)
    bo = np.asarray(bo, dtype=np.float32)

    bias_idx, biases, block_live = classify_mask(mask2d, S)
    nuniq = len(biases)
    bias_stack = (np.concatenate(biases, axis=1) if nuniq
                  else np.zeros((128, 128), np.float32))

    nc = build_nc(S, E, D, HL, bias_idx, block_live, nuniq)

    scale = np.float32(1.0 / np.sqrt(D))
    in_maps = []
    for c in range(N_CORES):
        b, hg = c // HG, c % HG
        cols = slice(hg * DIM, (hg + 1) * DIM)
        wv_l = Wv[:, cols].reshape(E, HL, D)
        wv_aug = np.zeros((E, HL, D + 1), np.float32)
        wv_aug[:, :, :D] = wv_l
        in_maps.append({
            "xqT": _bf16(query[b].T),
            "xkT": _bf16(key[b].T),
            "xvT": _bf16(value[b].T),
            "wq": _bf16(Wq[:, cols] * scale),
            "wk": _bf16(Wk[:, cols]),
            "wv": _bf16(wv_aug.reshape(E, HL * (D + 1))),
            "wo": _bf16(Wo[cols, :]),
            "biasT": _bf16(bias_stack),
        })

    res = run_bass_kernel_spmd(nc, in_maps, list(range(N_CORES)))
    LAST_RESULT = res
    LAST_EXEC_NS = res.exec_time_ns or res.mean_exec_time_ns

    out = np.empty((B, S, E), np.float32)
    for b in range(BG):
        acc = res.results[b * HG]["out_p"].astype(np.float32)
        for j in range(1, HG):
            acc = acc + res.results[b * HG + j]["out_p"]
        out[b] = acc + bo[None, :]
    return out


def _bf16(a):
    import ml_dtypes
    return np.ascontiguousarray(np.asarray(a, np.float32)).astype(
        ml_dtypes.bfloat16)


# revision 6
# speedup vs baseline: 1.1530x; 1.0094x over previous
"""Multi-head attention (B=2, S=2048, E=1024, H=16) on 8 Trainium2 NeuronCores.

Sharding: core c -> batch c//4, heads 4*(c%4)..4*(c%4)+3  (data + head parallel).
Each core computes a partial output projection [S, E] over its 256 head-dims;
the host sums the 4 partials per batch and adds the output bias.

v2 redesign (driven by the TimelineSim cost model, where every engine op
costs free_size * cycle_t and matmuls cost out_free * 0.4167ns per
contraction chunk; ~155us vs the 179us v1):
  * scores/exp/PV are TRIMMED to the live (causal) columns of each key
    tile: per (group, kt) only q-columns >= the first live q-block are
    computed, masked blocks inside that range get the 0/1 multiply.
  * Zinv = DVE reciprocal of the PV ones-row (bf16), broadcast across
    partitions by a K=1 matmul into a borrowed psO bank; ev leaves PSUM
    on the ACT engine so the normalize multiply has SBUF x PSUM operands
    (GPSIMD can't touch PSUM; partition_broadcast needs a gpsimd library;
    the walrus verifier rejects two-PSUM-input TensorTensor).
  * Output partials leave in bf16 (halves the out-DMA; the host sums the
    4 partials per batch in fp32), psum->sbuf copies split ACT/DVE.
  * Phase order: V e-inner (PE runs continuously once xv lands, ramping
    to the full p-state), m0 e-outer, pass 1 (heads 0/1, groups
    ascending) with the 8 m1-projection pieces woven between later head
    visits, pass 2 (heads 2/3, descending) with each group's output
    projection deferred into the next group's emission.  Every head's PV
    tail + normalize chain is carried into the following head's first
    score matmuls so the in-order engine queues never stall on them; the
    last group batches its chains, runs Zinv on the idle ACT engine, and
    drains its output projection through the freed scores psum tiles.
"""

import sys

for _p in ("/opt/trn_rl_repo", "/root/.axon_site/_ro/trn_rl_repo"):
    if _p not in sys.path:
        sys.path.insert(0, _p)

import numpy as np


# ---------------------------------------------------------------------------
# Patch: the walrus build in this container rejects >1 sem wait on one CTRL
# instruction and the TileContext exit drain aggregates every outstanding
# proc's wait onto a single Drain. Spill the excess waits onto SP nops.
# ---------------------------------------------------------------------------
def _install_tile_drain_patch():
    import concourse.tile as tile
    import concourse.mybir as mybir
    from concourse.vector_clock import ScopedClock

    if getattr(tile.TileContext, "_drain_patch_installed", False):
        return

    def _patched_drain_and_barrier(self, tick_clock, wait_clock):
        drain_inst = self.nc.sync.drain()
        wait_clock.add_sem_waits(
            drain_inst.ins, ScopedClock({None: tick_clock.global_clock})
        )
        si = drain_inst.ins.sync_info
        waits = list(si.on_wait) if si and si.on_wait else []
        if len(waits) > 1:
            si.on_wait = waits[:1]
            for w in waits[1:]:
                nop = self.nc.sync.nop(nofuse=True, hint="drain_wait_spill")
                nop.ins.sync_info = mybir.SyncInfo(on_wait=[w], on_update=[])
        self.nc.all_engine_barrier()
        assert self.sems is not None
        popped = self.nc._tile_sem_poison_stack.pop()
        assert popped is self._sem_poison
        self.nc.clear_and_free_semaphores(list(self.sems.allocated().values()))
        self.nc.all_engine_barrier()

    tile.TileContext._drain_and_barrier = _patched_drain_and_barrier
    tile.TileContext._drain_patch_installed = True


def _split_multi_waits(nc, maxw=1):
    """Hoist excess sem waits onto engine-queue NoOps inserted just before
    the instruction (sequencer executes them in order; semantics identical)."""
    import concourse.mybir as mybir

    ctr = 0
    for bb in nc.main_func.blocks:
        new = []
        for inst in bb.instructions:
            si = inst.sync_info
            waits = list(si.on_wait) if si and si.on_wait else []
            if len(waits) > maxw:
                extras = waits[:-maxw]
                si.on_wait = waits[-maxw:]
                for i in range(0, len(extras), maxw):
                    nop = mybir.InstNoOp(
                        name=f"I-waitspill-{ctr}", engine=inst.engine,
                        ins=[], outs=[])
                    ctr += 1
                    nop.sync_info = mybir.SyncInfo(
                        on_wait=extras[i:i + maxw], on_update=[])
                    try:
                        nc.register_instruction(nop, overwrite=True)
                    except Exception:
                        pass
                    new.append(nop)
            new.append(inst)
        bb.instructions = new


# ---------------------------------------------------------------------------
# Mask classification (host side, from the actual mask array).
# Blocks are 128x128 in the *transposed* score layout: block (kt, qb) covers
# keys kt*128.. x queries qb*128...
# ---------------------------------------------------------------------------
def classify_mask(mask2d, S, KB=128):
    nb = S // KB
    assert mask2d.shape == (S, S)
    assert mask2d.any(axis=1).all(), "a query row with no attended key"
    maskT = mask2d.T  # [keys, q]
    uniq = {}
    biases = []
    bias_idx = {}  # (kt, qb) -> None (all attended) or index
    block_live = np.zeros((nb, nb), dtype=bool)  # any attended key in block
    for kt in range(nb):
        for qb in range(nb):
            blk = maskT[kt * KB:(kt + 1) * KB, qb * KB:(qb + 1) * KB]
            if blk.all():
                bias_idx[(kt, qb)] = None
                block_live[kt, qb] = True
            else:
                b = np.where(blk, np.float32(1.0), np.float32(0.0))
                key = b.tobytes()
                if key not in uniq:
                    uniq[key] = len(biases)
                    biases.append(b)
                bias_idx[(kt, qb)] = uniq[key]
                block_live[kt, qb] = blk.any()
    return bias_idx, biases, block_live


# ---------------------------------------------------------------------------
# Bass program builder (one SPMD program, same for all cores).
# ---------------------------------------------------------------------------
def build_nc(S, E, D, HL, bias_idx, block_live, nuniq, shift=32.0, repeat=1):
    import concourse.bass as bass
    import concourse.mybir as mybir
    import concourse.tile as tile

    f32 = mybir.dt.float32
    bf16 = mybir.dt.bfloat16
    Act = mybir.ActivationFunctionType

    P = 128
    EC = E // P              # E chunks (contraction tiles for projections)
    DIM = HL * D             # this core's head dims (256)
    MT = DIM // P            # m-tiles of QT/KT (2)
    QG = 512                 # q-group width
    NQG = S // QG
    NQB = QG // P            # q-blocks per group
    NKT = S // P             # key tiles
    NST = S // P             # s tiles
    VW = HL * (D + 1)        # V width incl. ones columns (260)
    EGW = min(QG, E)         # output E slice width
    NEG = E // EGW           # output E slices (2)

    # live key tiles of group g with the first live q-block column offset;
    # the first entry is widened to cover every column any later kt writes so
    # its start=True matmul initializes the whole accumulation region.
    def kt_offs(g):
        out = []
        for kt in range(NKT):
            lives = [j for j in range(NQB) if block_live[kt, g * NQB + j]]
            if lives:
                out.append((kt, lives[0] * P))
        if out:
            m0 = min(o for _, o in out)
            out[0] = (out[0][0], m0)
        return out

    nc = bass.Bass()
    dp = nc.declare_dram_parameter
    d_xq = dp("xqT", [E, S], bf16, isOutput=False)
    d_xk = dp("xkT", [E, S], bf16, isOutput=False)
    d_xv = dp("xvT", [E, S], bf16, isOutput=False)
    d_wq = dp("wq", [E, DIM], bf16, isOutput=False)
    d_wk = dp("wk", [E, DIM], bf16, isOutput=False)
    d_wv = dp("wv", [E, VW], bf16, isOutput=False)
    d_wo = dp("wo", [DIM, E], bf16, isOutput=False)
    d_bias = dp("biasT", [P, max(nuniq, 1) * P], bf16, isOutput=False)
    d_out = dp("out_p", [S, E], bf16, isOutput=True)

    import contextlib
    with tile.TileContext(nc) as tc, contextlib.ExitStack() as _stk:
        consts = _stk.enter_context(tc.tile_pool(name="consts", bufs=1))

        w_sb = {}
        for nm, width in (("wq", DIM), ("wk", DIM), ("wv", VW)):
            w_sb[nm] = consts.tile([P, EC, width], bf16, name=f"sb_{nm}",
                                   tag=f"sb_{nm}")
        w_dram = {"wq": d_wq, "wk": d_wk, "wv": d_wv}
        wo_sb = [consts.tile([2 * D, E], bf16, name=f"sb_wo{p}",
                             tag=f"sb_wo{p}") for p in range(HL // 2)]
        bias_sb = consts.tile([P, max(nuniq, 1) * P], bf16, name="sb_bias")
        negshift = consts.tile([P, 1], f32, name="negshift")
        nc.vector.memset(negshift, -shift)
        ones64 = consts.tile([1, D], bf16, name="ones64")
        nc.vector.memset(ones64, 1.0)

        def load_w(nm):
            nc.sync.dma_start(
                out=w_sb[nm],
                in_=w_dram[nm][:, :].rearrange("(e p) n -> p e n", p=P))

        def emit_once():
            # persistent projection outputs
            QT = [consts.tile([P, S], bf16, name=f"QT{m}", tag=f"QT{m}")
                  for m in range(MT)]
            KT = [consts.tile([P, S], bf16, name=f"KT{m}", tag=f"KT{m}")
                  for m in range(MT)]
            V = [consts.tile([P, VW], bf16, name=f"V{s}", tag=f"V{s}")
                 for s in range(NST)]
            attnP = [[consts.tile([2 * D, QG], bf16, name=f"attnP{p}g{g}",
                                  tag=f"attnP{p}g{g}") for g in range(NQG)]
                     for p in range(HL // 2)]

            with tc.tile_pool(name="xt", bufs=2 * EC + 4) as xt_pool:
                psA_ctx = contextlib.ExitStack()
                psV = psA_ctx.enter_context(
                    tc.tile_pool(name="psV", bufs=1, space="PSUM"))
                psM = psA_ctx.enter_context(
                    tc.tile_pool(name="psM", bufs=1, space="PSUM"))

                def stream_chunks(dram):
                    chunks = []
                    for e in range(EC):
                        ch = xt_pool.tile([P, S], bf16, tag="xt", name=f"xch{e}")
                        nc.sync.dma_start(out=ch, in_=dram[e * P:(e + 1) * P, :])
                        chunks.append(ch)
                    return chunks

                # ---------------- phase A: projections ----------------
                # wv chunk 0 first so the first V matmul only waits ~1.6us
                nc.sync.dma_start(out=w_sb["wv"][:, 0, :], in_=d_wv[0:P, :])
                vchunks = [xt_pool.tile([P, S], bf16, tag="xt", name="xch0")]
                nc.sync.dma_start(out=vchunks[0], in_=d_xv[0:P, :])
                nc.sync.dma_start(
                    out=w_sb["wv"][:, 1:EC, :],
                    in_=d_wv[P:, :].rearrange("(e p) n -> p e n", p=P))
                for e in range(1, EC):
                    ch = xt_pool.tile([P, S], bf16, tag="xt", name=f"xch{e}")
                    nc.sync.dma_start(out=ch, in_=d_xv[e * P:(e + 1) * P, :])
                    vchunks.append(ch)
                load_w("wq")
                qchunks = stream_chunks(d_xq)
                load_w("wk")
                kchunks = stream_chunks(d_xk)
                for p in range(HL // 2):
                    nc.sync.dma_start(
                        out=wo_sb[p], in_=d_wo[p * 2 * D:(p + 1) * 2 * D, :])
                nc.sync.dma_start(out=bias_sb, in_=d_bias[:, :])

                def finish_v(st, ps):
                    nc.vector.tensor_copy(V[st], ps)
                    onescols = V[st].rearrange(
                        "p (h c) -> p h c", c=D + 1)[:, :, D]
                    nc.gpsimd.memset(onescols, 1.0)

                # V tiles e-inner, starting once the whole xv stream has
                # landed (~11.4us): the PE then runs CONTINUOUSLY (ramping to
                # the full p-state) through V and the m0 projections while
                # xq/xk stream in, instead of chunk-paced stuttering.
                for st in range(0, NST):
                    ps = psV.tile([P, VW], f32, tag=f"v{st % 4}",
                                  name=f"psv{st}")
                    for e in range(EC):
                        nc.tensor.matmul(
                            ps,
                            lhsT=vchunks[e][:, st * P:(st + 1) * P],
                            rhs=w_sb["wv"][:, e, :],
                            start=(e == 0), stop=(e == EC - 1))
                    finish_v(st, ps)

                # QK projections for m-tile m into QT[m]/KT[m]; e-outer over
                # the 4 q-groups of each (weight, group) so PE work is
                # chunk-paced.  `pool`/`tag` chooses the psum bank set.
                def qk_proj(m, pool, tagf):
                    for wname, dst, chunks in (("wq", QT, qchunks),
                                               ("wk", KT, kchunks)):
                        pss = [pool.tile([P, QG], f32, tag=tagf(g),
                                         name=f"psqk{m}{wname}{g}")
                               for g in range(NQG)]
                        for e in range(EC):
                            for g in range(NQG):
                                nc.tensor.matmul(
                                    pss[g],
                                    lhsT=w_sb[wname][:, e, m * P:(m + 1) * P],
                                    rhs=chunks[e][:, g * QG:(g + 1) * QG],
                                    start=(e == 0), stop=(e == EC - 1))
                        for g in range(NQG):
                            nc.vector.tensor_copy(
                                dst[m][:, g * QG:(g + 1) * QG], pss[g])

                qk_proj(0, psM, lambda g: f"m{g}")
                psA_ctx.close()  # free the 8 phase-A PSUM banks for phase B

                # ---------------- phase B: attention ----------------
                # PSUM bank alignment: psPV/psO open first so they land on
                # the psV banks (free early); psS lands on the psM banks,
                # which free exactly when the m0 copies complete.
                with tc.tile_pool(name="probs", bufs=6) as probs_pool, \
                     tc.tile_pool(name="zrow", bufs=2) as z_pool, \
                     tc.tile_pool(name="evs", bufs=2) as ev_pool, \
                     tc.tile_pool(name="outst", bufs=4) as out_pool, \
                     tc.tile_pool(name="psS", bufs=2, space="PSUM") as psS, \
                     tc.tile_pool(name="psPV", bufs=2, space="PSUM") as psPV, \
                     tc.tile_pool(name="psO", bufs=2, space="PSUM") as psO:

                    def attn(g, h, carry=(), defer=False, act_recip=False):
                        """Emit attention for (g, h).  `carry` holds the
                        previous head's deferred PV-flush + normalize chain;
                        it is emitted right after this head's first score
                        matmuls so the ACT engine sees the next exp without
                        waiting for the previous head's PV tail.  With
                        defer=True the tail thunks are returned instead of
                        emitted."""
                        m, po = h // 2, (h % 2) * D
                        kts = kt_offs(g)
                        total = len(kts)
                        pairs = [kts[i:i + 2] for i in range(0, total, 2)]
                        pv = psPV.tile([D + 1, QG], f32, tag="pv")
                        npv = 0
                        pend = []
                        carried = list(carry)

                        def emit_pv(entry):
                            nonlocal npv
                            pb, regions = entry
                            for (kt, off), c, w in regions:
                                nc.tensor.matmul(
                                    pv[0:D + 1, off:QG],
                                    lhsT=V[kt][:, h * (D + 1):
                                               (h + 1) * (D + 1)],
                                    rhs=pb[:, c:c + w],
                                    start=(npv == 0),
                                    stop=(npv == total - 1),
                                    skip_group_check=True)
                                npv += 1

                        for pi, pair in enumerate(pairs):
                            regions = []
                            col = 0
                            for (kt, off) in pair:
                                w = QG - off
                                regions.append(((kt, off), col, w))
                                col += w
                            sps = psS.tile([P, 2 * QG], f32, tag="s")
                            pb = probs_pool.tile([P, 2 * QG], bf16, tag="pb")
                            for (kt, off), c, w in regions:
                                nc.tensor.matmul(
                                    sps[:, c:c + w],
                                    lhsT=KT[m][po:po + D, kt * P:(kt + 1) * P],
                                    rhs=QT[m][po:po + D,
                                              g * QG + off:(g + 1) * QG],
                                    start=True, stop=True)
                            if pi == 0:
                                while carried:
                                    carried.pop(0)()
                            nc.scalar.activation(pb[:, 0:col], sps[:, 0:col],
                                                 Act.Exp,
                                                 bias=negshift[:, 0:1])
                            # masking after exp: multiplicative 0/1, exact
                            for (kt, off), c, w in regions:
                                for j in range(off // P, NQB):
                                    qb = g * NQB + j
                                    bidx = bias_idx[(kt, qb)]
                                    if bidx is None:
                                        continue
                                    cc = c + j * P - off
                                    blk = pb[:, cc:cc + P]
                                    if not block_live[kt, qb]:
                                        nc.gpsimd.memset(blk, 0.0)
                                    else:
                                        nc.vector.tensor_mul(
                                            blk, blk,
                                            bias_sb[:, bidx * P:
                                                    (bidx + 1) * P])
                            pend.append((pb, regions))
                            if len(pend) > 3:
                                emit_pv(pend.pop(0))

                        def t_pv():
                            while pend:
                                emit_pv(pend.pop(0))

                        # normalize: Zinv = 1/Z (bf16, matching the
                        # reference-passing baseline's precision), broadcast
                        # across partitions by a K=1 matmul into a borrowed
                        # psO bank; ev leaves PSUM via the ACT engine so the
                        # final multiply has baseline-shaped operands
                        # (SBUF x PSUM).
                        def t_chain():
                            zi = z_pool.tile([1, QG], bf16, tag=f"zi{h % 2}",
                                             name=f"zi{h}")
                            if act_recip:
                                # tail path: ACT is idle there; Zinv =
                                # exp(-ln(Z)) is exact to bf16 rounding
                                zf = z_pool.tile([1, QG], f32,
                                                 tag=f"zf{h % 2}",
                                                 name=f"zf{h}")
                                nc.scalar.activation(zf, pv[D:D + 1, :],
                                                     Act.Ln)
                                nc.scalar.activation(zi, zf, Act.Exp,
                                                     scale=-1.0)
                            else:
                                with nc.allow_low_precision(
                                        reason="bf16 Zinv, like the Wo "
                                               "operands downstream"):
                                    nc.vector.reciprocal(zi, pv[D:D + 1, :])
                            ev = ev_pool.tile([D, QG], f32,
                                              tag=f"ev{h % 2}", name=f"ev{h}")
                            nc.scalar.copy(ev, pv[0:D, :])
                            bps = psO.tile([D, QG], f32, tag="o",
                                           name=f"bps{h}")
                            nc.tensor.matmul(bps, lhsT=ones64, rhs=zi,
                                             start=True, stop=True)
                            nc.vector.tensor_mul(
                                attnP[h // 2][g][po:po + D, :], ev, bps)

                        if defer:
                            return [t_pv, t_chain]
                        t_pv()
                        t_chain()
                        return []

                    def wo_proj(g, last=False):
                        # For the final group the psS pool is free (no more
                        # scores), so use its 2-bank tiles to double the
                        # psum buffering and shorten the drain.
                        for j in range(NQB):
                            st = g * NQB + j
                            ot = out_pool.tile([P, NEG * EGW], bf16, tag="ot")
                            if last:
                                ops2 = psS.tile([P, 2 * QG], f32, tag="s",
                                                name="opss")
                            for eg in range(NEG):
                                if last:
                                    ops = ops2[:, eg * EGW:(eg + 1) * EGW]
                                else:
                                    ops = psO.tile([P, EGW], f32, tag="o",
                                                   name="opso")
                                for p in range(HL // 2):
                                    nc.tensor.matmul(
                                        ops,
                                        lhsT=attnP[p][g][:, j * P:(j + 1) * P],
                                        rhs=wo_sb[p][:, eg * EGW:
                                                     (eg + 1) * EGW],
                                        start=(p == 0),
                                        stop=(p == HL // 2 - 1),
                                        skip_group_check=last)
                                # copy PSUM->SBUF (GPSIMD cannot touch PSUM);
                                # at the tail the idle ACT engine takes half
                                dst = ot[:, eg * EGW:(eg + 1) * EGW]
                                if eg == 0:
                                    nc.scalar.copy(dst, ops)
                                else:
                                    nc.vector.tensor_copy(dst, ops)
                                if last:
                                    nc.sync.dma_start(
                                        out=d_out[st * P:(st + 1) * P,
                                                  eg * EGW:(eg + 1) * EGW],
                                        in_=dst)
                            if not last:
                                nc.sync.dma_start(
                                    out=d_out[st * P:(st + 1) * P, :], in_=ot)

                    # m1 projection piece for one (weight, group): psum
                    # borrowed from the (pass-1-unused) psO pool; e-inner so
                    # the two psO buffers ping-pong.
                    def m1_piece(wname, g):
                        dst = QT if wname == "wq" else KT
                        chunks = qchunks if wname == "wq" else kchunks
                        ps = psO.tile([P, QG], f32, tag="o",
                                      name=f"psqk1{wname}{g}")
                        for e in range(EC):
                            nc.tensor.matmul(
                                ps,
                                lhsT=w_sb[wname][:, e, P:2 * P],
                                rhs=chunks[e][:, g * QG:(g + 1) * QG],
                                start=(e == 0), stop=(e == EC - 1))
                        nc.vector.tensor_copy(
                            dst[1][:, g * QG:(g + 1) * QG], ps)

                    # pass 1: heads 0/1 over all groups (needs only m0 + V),
                    # ascending (ends on the biggest exp backlog).  The 8 m1
                    # projection pieces are woven between the later head
                    # visits, where the exp backlog hides their ACT-less PE
                    # time; PV tails and normalize chains are carried into
                    # the next head so ACT never waits on them.
                    g_up = sorted(range(NQG), key=lambda g: len(kt_offs(g)))
                    m1_sched = {1: ["wq0"], 2: ["wq1"], 3: ["wq2"],
                                4: ["wq3"], 5: ["wk0", "wk1"],
                                6: ["wk2"], 7: ["wk3"]}
                    carry = []
                    for i, g in enumerate(g_up):
                        for h in (0, 1):
                            carry = attn(g, h, carry=carry, defer=True)
                            for pc in m1_sched.get(2 * i + h, []):
                                m1_piece("wq" if pc[:2] == "wq" else "wk",
                                         int(pc[2]))
                    for t in carry:
                        t()

                    # pass 2: heads 2/3 + output projection per group;
                    # descending so the big group lands right after m1 and
                    # the kernel tail ends on the smallest one.  Each group's
                    # h3 tail + output projection are carried into the next
                    # group's first score matmuls; the last group runs its
                    # reciprocals on the (by then idle) ACT engine and
                    # interleaves its two normalize chains to cut the drain.
                    carry = []
                    gs2 = list(reversed(g_up))
                    for i, g in enumerate(gs2):
                        last = (i == NQG - 1)
                        c2 = attn(g, 2, carry=carry, defer=True,
                                  act_recip=last)
                        c3 = attn(g, 3, carry=c2[:1], defer=True,
                                  act_recip=last)
                        if not last:
                            # [pv3, chain2, chain3, wo(g)] ride into the next
                            # group's emission
                            carry = [c3[0], c2[1], c3[1],
                                     lambda g=g: wo_proj(g)]
                        else:
                            c3[0]()
                            c2[1]()
                            c3[1]()
                            wo_proj(g, last=True)

        for _rep in range(repeat):
            emit_once()

    _split_multi_waits(nc)
    return nc


# ---------------------------------------------------------------------------
# Host entry point
# ---------------------------------------------------------------------------
LAST_EXEC_NS = None
LAST_RESULT = None


def kernel(query, key, value, mask, Wq, Wk, Wv, Wo, bo):
    global LAST_EXEC_NS, LAST_RESULT
    _install_tile_drain_patch()
    from concourse.bass_utils import run_bass_kernel_spmd

    B, S, E = 2, 2048, 1024
    H, D = 16, 64
    N_CORES = 8
    BG = 2                    # batch groups
    HG = N_CORES // BG        # head groups per batch
    HL = H // HG              # heads per core
    DIM = HL * D

    query = np.asarray(query, dtype=np.float32)
    key = np.asarray(key, dtype=np.float32)
    value = np.asarray(value, dtype=np.float32)
    mask2d = np.asarray(mask).reshape(S, S).astype(bool)
    Wq = np.asarray(Wq, dtype=np.float32)
    Wk = np.asarray(Wk, dtype=np.float32)
    Wv = np.asarray(Wv, dtype=np.float32)
    Wo = np.asarray(Wo, dtype=np.float32)
    bo = np.asarray(bo, dtype=np.float32)

    bias_idx, biases, block_live = classify_mask(mask2d, S)
    nuniq = len(biases)
    bias_stack = (np.concatenate(biases, axis=1) if nuniq
                  else np.zeros((128, 128), np.float32))

    nc = build_nc(S, E, D, HL, bias_idx, block_live, nuniq)

    scale = np.float32(1.0 / np.sqrt(D))
    in_maps = []
    for c in range(N_CORES):
        b, hg = c // HG, c % HG
        cols = slice(hg * DIM, (hg + 1) * DIM)
        wv_l = Wv[:, cols].reshape(E, HL, D)
        wv_aug = np.zeros((E, HL, D + 1), np.float32)
        wv_aug[:, :, :D] = wv_l
        in_maps.append({
            "xqT": _bf16(query[b].T),
            "xkT": _bf16(key[b].T),
            "xvT": _bf16(value[b].T),
            "wq": _bf16(Wq[:, cols] * scale),
            "wk": _bf16(Wk[:, cols]),
            "wv": _bf16(wv_aug.reshape(E, HL * (D + 1))),
            "wo": _bf16(Wo[cols, :]),
            "biasT": _bf16(bias_stack),
        })

    res = run_bass_kernel_spmd(nc, in_maps, list(range(N_CORES)))
    LAST_RESULT = res
    LAST_EXEC_NS = res.exec_time_ns or res.mean_exec_time_ns

    out = np.empty((B, S, E), np.float32)
    for b in range(BG):
        acc = res.results[b * HG]["out_p"].astype(np.float32)
        for j in range(1, HG):
            acc = acc + res.results[b * HG + j]["out_p"]
        out[b] = acc + bo[None, :]
    return out


def _bf16(a):
    import ml_dtypes
    return np.ascontiguousarray(np.asarray(a, np.float32)).astype(
        ml_dtypes.bfloat16)


# revision 8
# speedup vs baseline: 1.1765x; 1.0204x over previous
"""Multi-head attention (B=2, S=2048, E=1024, H=16) on 8 Trainium2 NeuronCores.

Sharding: core c -> batch c//4, heads 4*(c%4)..4*(c%4)+3  (data + head parallel).
Each core computes a partial output projection [S, E] over its 256 head-dims;
the host sums the 4 partials per batch and adds the output bias.

v2 redesign (driven by the TimelineSim cost model, where every engine op
costs free_size * cycle_t and matmuls cost out_free * 0.4167ns per
contraction chunk; ~152us vs the 179us v1):
  * scores/exp/PV are TRIMMED to the live (causal) columns of each key
    tile: per (group, kt) only q-columns >= the first live q-block are
    computed, masked blocks inside that range get the 0/1 multiply.
  * Zinv = DVE reciprocal of the PV ones-row (bf16), broadcast across
    partitions by a K=1 matmul into a borrowed psO bank; ev leaves PSUM
    on the ACT engine so the normalize multiply has SBUF x PSUM operands
    (GPSIMD can't touch PSUM; partition_broadcast needs a gpsimd library;
    the walrus verifier rejects two-PSUM-input TensorTensor).
  * Output partials leave in bf16 (halves the out-DMA; the host sums the
    4 partials per batch in fp32); output psum->sbuf copies split ACT/DVE,
    the ev copies ride DVE so the in-order ACT queue carries only exps.
  * Phase order: V e-inner (PE runs continuously once xv lands, ramping
    to the full p-state), m0 e-outer, pass 1 (heads 0/1, groups
    ascending) with the 8 m1-projection pieces woven between later head
    visits, pass 2 (heads 2/3, descending) with each group's output
    projection deferred into the next group's emission.  Every head's PV
    tail + normalize chain is carried into the following head's first
    score matmuls so the in-order engine queues never stall on them; the
    last group batches its chains, runs Zinv on the idle ACT engine, and
    drains its output projection through the freed scores psum tiles.
"""

import sys

for _p in ("/opt/trn_rl_repo", "/root/.axon_site/_ro/trn_rl_repo"):
    if _p not in sys.path:
        sys.path.insert(0, _p)

import numpy as np


# ---------------------------------------------------------------------------
# Patch: the walrus build in this container rejects >1 sem wait on one CTRL
# instruction and the TileContext exit drain aggregates every outstanding
# proc's wait onto a single Drain. Spill the excess waits onto SP nops.
# ---------------------------------------------------------------------------
def _install_tile_drain_patch():
    import concourse.tile as tile
    import concourse.mybir as mybir
    from concourse.vector_clock import ScopedClock

    if getattr(tile.TileContext, "_drain_patch_installed", False):
        return

    def _patched_drain_and_barrier(self, tick_clock, wait_clock):
        drain_inst = self.nc.sync.drain()
        wait_clock.add_sem_waits(
            drain_inst.ins, ScopedClock({None: tick_clock.global_clock})
        )
        si = drain_inst.ins.sync_info
        waits = list(si.on_wait) if si and si.on_wait else []
        if len(waits) > 1:
            si.on_wait = waits[:1]
            for w in waits[1:]:
                nop = self.nc.sync.nop(nofuse=True, hint="drain_wait_spill")
                nop.ins.sync_info = mybir.SyncInfo(on_wait=[w], on_update=[])
        self.nc.all_engine_barrier()
        assert self.sems is not None
        popped = self.nc._tile_sem_poison_stack.pop()
        assert popped is self._sem_poison
        self.nc.clear_and_free_semaphores(list(self.sems.allocated().values()))
        self.nc.all_engine_barrier()

    tile.TileContext._drain_and_barrier = _patched_drain_and_barrier
    tile.TileContext._drain_patch_installed = True


def _split_multi_waits(nc, maxw=1):
    """Hoist excess sem waits onto engine-queue NoOps inserted just before
    the instruction (sequencer executes them in order; semantics identical)."""
    import concourse.mybir as mybir

    ctr = 0
    for bb in nc.main_func.blocks:
        new = []
        for inst in bb.instructions:
            si = inst.sync_info
            waits = list(si.on_wait) if si and si.on_wait else []
            if len(waits) > maxw:
                extras = waits[:-maxw]
                si.on_wait = waits[-maxw:]
                for i in range(0, len(extras), maxw):
                    nop = mybir.InstNoOp(
                        name=f"I-waitspill-{ctr}", engine=inst.engine,
                        ins=[], outs=[])
                    ctr += 1
                    nop.sync_info = mybir.SyncInfo(
                        on_wait=extras[i:i + maxw], on_update=[])
                    try:
                        nc.register_instruction(nop, overwrite=True)
                    except Exception:
                        pass
                    new.append(nop)
            new.append(inst)
        bb.instructions = new


# ---------------------------------------------------------------------------
# Mask classification (host side, from the actual mask array).
# Blocks are 128x128 in the *transposed* score layout: block (kt, qb) covers
# keys kt*128.. x queries qb*128...
# ---------------------------------------------------------------------------
def classify_mask(mask2d, S, KB=128):
    nb = S // KB
    assert mask2d.shape == (S, S)
    assert mask2d.any(axis=1).all(), "a query row with no attended key"
    maskT = mask2d.T  # [keys, q]
    uniq = {}
    biases = []
    bias_idx = {}  # (kt, qb) -> None (all attended) or index
    block_live = np.zeros((nb, nb), dtype=bool)  # any attended key in block
    for kt in range(nb):
        for qb in range(nb):
            blk = maskT[kt * KB:(kt + 1) * KB, qb * KB:(qb + 1) * KB]
            if blk.all():
                bias_idx[(kt, qb)] = None
                block_live[kt, qb] = True
            else:
                b = np.where(blk, np.float32(1.0), np.float32(0.0))
                key = b.tobytes()
                if key not in uniq:
                    uniq[key] = len(biases)
                    biases.append(b)
                bias_idx[(kt, qb)] = uniq[key]
                block_live[kt, qb] = blk.any()
    return bias_idx, biases, block_live


# ---------------------------------------------------------------------------
# Bass program builder (one SPMD program, same for all cores).
# ---------------------------------------------------------------------------
def build_nc(S, E, D, HL, bias_idx, block_live, nuniq, shift=32.0, repeat=1):
    import concourse.bass as bass
    import concourse.mybir as mybir
    import concourse.tile as tile

    f32 = mybir.dt.float32
    bf16 = mybir.dt.bfloat16
    Act = mybir.ActivationFunctionType

    P = 128
    EC = E // P              # E chunks (contraction tiles for projections)
    DIM = HL * D             # this core's head dims (256)
    MT = DIM // P            # m-tiles of QT/KT (2)
    QG = 512                 # q-group width
    NQG = S // QG
    NQB = QG // P            # q-blocks per group
    NKT = S // P             # key tiles
    NST = S // P             # s tiles
    VW = HL * (D + 1)        # V width incl. ones columns (260)
    EGW = min(QG, E)         # output E slice width
    NEG = E // EGW           # output E slices (2)

    # live key tiles of group g with the first live q-block column offset;
    # the first entry is widened to cover every column any later kt writes so
    # its start=True matmul initializes the whole accumulation region.
    def kt_offs(g):
        out = []
        for kt in range(NKT):
            lives = [j for j in range(NQB) if block_live[kt, g * NQB + j]]
            if lives:
                out.append((kt, lives[0] * P))
        if out:
            m0 = min(o for _, o in out)
            out[0] = (out[0][0], m0)
        return out

    nc = bass.Bass()
    dp = nc.declare_dram_parameter
    d_xq = dp("xqT", [E, S], bf16, isOutput=False)
    d_xk = dp("xkT", [E, S], bf16, isOutput=False)
    d_xv = dp("xvT", [E, S], bf16, isOutput=False)
    d_wq = dp("wq", [E, DIM], bf16, isOutput=False)
    d_wk = dp("wk", [E, DIM], bf16, isOutput=False)
    d_wv = dp("wv", [E, VW], bf16, isOutput=False)
    d_wo = dp("wo", [DIM, E], bf16, isOutput=False)
    d_bias = dp("biasT", [P, max(nuniq, 1) * P], bf16, isOutput=False)
    d_out = dp("out_p", [S, E], bf16, isOutput=True)

    import contextlib
    with tile.TileContext(nc) as tc, contextlib.ExitStack() as _stk:
        consts = _stk.enter_context(tc.tile_pool(name="consts", bufs=1))

        w_sb = {}
        for nm, width in (("wq", DIM), ("wk", DIM), ("wv", VW)):
            w_sb[nm] = consts.tile([P, EC, width], bf16, name=f"sb_{nm}",
                                   tag=f"sb_{nm}")
        w_dram = {"wq": d_wq, "wk": d_wk, "wv": d_wv}
        wo_sb = [consts.tile([2 * D, E], bf16, name=f"sb_wo{p}",
                             tag=f"sb_wo{p}") for p in range(HL // 2)]
        bias_sb = consts.tile([P, max(nuniq, 1) * P], bf16, name="sb_bias")
        negshift = consts.tile([P, 1], f32, name="negshift")
        nc.vector.memset(negshift, -shift)
        ones64 = consts.tile([1, D], bf16, name="ones64")
        nc.vector.memset(ones64, 1.0)

        def load_w(nm):
            nc.sync.dma_start(
                out=w_sb[nm],
                in_=w_dram[nm][:, :].rearrange("(e p) n -> p e n", p=P))

        def emit_once():
            # persistent projection outputs
            QT = [consts.tile([P, S], bf16, name=f"QT{m}", tag=f"QT{m}")
                  for m in range(MT)]
            KT = [consts.tile([P, S], bf16, name=f"KT{m}", tag=f"KT{m}")
                  for m in range(MT)]
            V = [consts.tile([P, VW], bf16, name=f"V{s}", tag=f"V{s}")
                 for s in range(NST)]
            attnP = [[consts.tile([2 * D, QG], bf16, name=f"attnP{p}g{g}",
                                  tag=f"attnP{p}g{g}") for g in range(NQG)]
                     for p in range(HL // 2)]

            with tc.tile_pool(name="xt", bufs=2 * EC + 4) as xt_pool:
                psA_ctx = contextlib.ExitStack()
                psV = psA_ctx.enter_context(
                    tc.tile_pool(name="psV", bufs=1, space="PSUM"))
                psM = psA_ctx.enter_context(
                    tc.tile_pool(name="psM", bufs=1, space="PSUM"))

                def stream_chunks(dram):
                    chunks = []
                    for e in range(EC):
                        ch = xt_pool.tile([P, S], bf16, tag="xt", name=f"xch{e}")
                        nc.sync.dma_start(out=ch, in_=dram[e * P:(e + 1) * P, :])
                        chunks.append(ch)
                    return chunks

                # ---------------- phase A: projections ----------------
                # wv chunk 0 first so the first V matmul only waits ~1.6us
                nc.sync.dma_start(out=w_sb["wv"][:, 0, :], in_=d_wv[0:P, :])
                vchunks = [xt_pool.tile([P, S], bf16, tag="xt", name="xch0")]
                nc.sync.dma_start(out=vchunks[0], in_=d_xv[0:P, :])
                nc.sync.dma_start(
                    out=w_sb["wv"][:, 1:EC, :],
                    in_=d_wv[P:, :].rearrange("(e p) n -> p e n", p=P))
                for e in range(1, EC):
                    ch = xt_pool.tile([P, S], bf16, tag="xt", name=f"xch{e}")
                    nc.sync.dma_start(out=ch, in_=d_xv[e * P:(e + 1) * P, :])
                    vchunks.append(ch)
                load_w("wq")
                qchunks = stream_chunks(d_xq)
                load_w("wk")
                kchunks = stream_chunks(d_xk)
                for p in range(HL // 2):
                    nc.sync.dma_start(
                        out=wo_sb[p], in_=d_wo[p * 2 * D:(p + 1) * 2 * D, :])
                nc.sync.dma_start(out=bias_sb, in_=d_bias[:, :])

                def finish_v(st, ps):
                    nc.vector.tensor_copy(V[st], ps)
                    onescols = V[st].rearrange(
                        "p (h c) -> p h c", c=D + 1)[:, :, D]
                    nc.gpsimd.memset(onescols, 1.0)

                # V tiles e-inner, starting once the whole xv stream has
                # landed (~11.4us): the PE then runs CONTINUOUSLY (ramping to
                # the full p-state) through V and the m0 projections while
                # xq/xk stream in, instead of chunk-paced stuttering.
                for st in range(0, NST):
                    ps = psV.tile([P, VW], f32, tag=f"v{st % 4}",
                                  name=f"psv{st}")
                    for e in range(EC):
                        nc.tensor.matmul(
                            ps,
                            lhsT=vchunks[e][:, st * P:(st + 1) * P],
                            rhs=w_sb["wv"][:, e, :],
                            start=(e == 0), stop=(e == EC - 1))
                    finish_v(st, ps)

                # QK projections for m-tile m into QT[m]/KT[m]; e-outer over
                # the 4 q-groups of each (weight, group) so PE work is
                # chunk-paced.  `pool`/`tag` chooses the psum bank set.
                def qk_proj(m, pool, tagf):
                    for wname, dst, chunks in (("wq", QT, qchunks),
                                               ("wk", KT, kchunks)):
                        pss = [pool.tile([P, QG], f32, tag=tagf(g),
                                         name=f"psqk{m}{wname}{g}")
                               for g in range(NQG)]
                        for e in range(EC):
                            for g in range(NQG):
                                nc.tensor.matmul(
                                    pss[g],
                                    lhsT=w_sb[wname][:, e, m * P:(m + 1) * P],
                                    rhs=chunks[e][:, g * QG:(g + 1) * QG],
                                    start=(e == 0), stop=(e == EC - 1))
                        for g in range(NQG):
                            nc.vector.tensor_copy(
                                dst[m][:, g * QG:(g + 1) * QG], pss[g])

                qk_proj(0, psM, lambda g: f"m{g}")
                psA_ctx.close()  # free the 8 phase-A PSUM banks for phase B

                # ---------------- phase B: attention ----------------
                # PSUM bank alignment: psPV/psO open first so they land on
                # the psV banks (free early); psS lands on the psM banks,
                # which free exactly when the m0 copies complete.
                with tc.tile_pool(name="probs", bufs=6) as probs_pool, \
                     tc.tile_pool(name="zrow", bufs=2) as z_pool, \
                     tc.tile_pool(name="evs", bufs=2) as ev_pool, \
                     tc.tile_pool(name="outst", bufs=4) as out_pool, \
                     tc.tile_pool(name="psS", bufs=2, space="PSUM") as psS, \
                     tc.tile_pool(name="psPV", bufs=2, space="PSUM") as psPV, \
                     tc.tile_pool(name="psO", bufs=2, space="PSUM") as psO:

                    def attn(g, h, carry=(), defer=False, act_recip=False):
                        """Emit attention for (g, h).  `carry` holds the
                        previous head's deferred PV-flush + normalize chain;
                        it is emitted right after this head's first score
                        matmuls so the ACT engine sees the next exp without
                        waiting for the previous head's PV tail.  With
                        defer=True the tail thunks are returned instead of
                        emitted."""
                        m, po = h // 2, (h % 2) * D
                        kts = kt_offs(g)
                        total = len(kts)
                        pairs = [kts[i:i + 2] for i in range(0, total, 2)]
                        pv = psPV.tile([D + 1, QG], f32, tag="pv")
                        npv = 0
                        pend = []
                        carried = list(carry)

                        def emit_pv(entry):
                            nonlocal npv
                            pb, regions = entry
                            for (kt, off), c, w in regions:
                                nc.tensor.matmul(
                                    pv[0:D + 1, off:QG],
                                    lhsT=V[kt][:, h * (D + 1):
                                               (h + 1) * (D + 1)],
                                    rhs=pb[:, c:c + w],
                                    start=(npv == 0),
                                    stop=(npv == total - 1),
                                    skip_group_check=True)
                                npv += 1

                        for pi, pair in enumerate(pairs):
                            regions = []
                            col = 0
                            for (kt, off) in pair:
                                w = QG - off
                                regions.append(((kt, off), col, w))
                                col += w
                            sps = psS.tile([P, 2 * QG], f32, tag="s")
                            pb = probs_pool.tile([P, 2 * QG], bf16, tag="pb")
                            for (kt, off), c, w in regions:
                                nc.tensor.matmul(
                                    sps[:, c:c + w],
                                    lhsT=KT[m][po:po + D, kt * P:(kt + 1) * P],
                                    rhs=QT[m][po:po + D,
                                              g * QG + off:(g + 1) * QG],
                                    start=True, stop=True)
                            if pi == 0:
                                while carried:
                                    carried.pop(0)()
                            nc.scalar.activation(pb[:, 0:col], sps[:, 0:col],
                                                 Act.Exp,
                                                 bias=negshift[:, 0:1])
                            # masking after exp: multiplicative 0/1, exact
                            for (kt, off), c, w in regions:
                                for j in range(off // P, NQB):
                                    qb = g * NQB + j
                                    bidx = bias_idx[(kt, qb)]
                                    if bidx is None:
                                        continue
                                    cc = c + j * P - off
                                    blk = pb[:, cc:cc + P]
                                    if not block_live[kt, qb]:
                                        nc.gpsimd.memset(blk, 0.0)
                                    else:
                                        nc.vector.tensor_mul(
                                            blk, blk,
                                            bias_sb[:, bidx * P:
                                                    (bidx + 1) * P])
                            pend.append((pb, regions))
                            if len(pend) > 3:
                                emit_pv(pend.pop(0))

                        def t_pv():
                            while pend:
                                emit_pv(pend.pop(0))

                        # normalize: Zinv = 1/Z (bf16, matching the
                        # reference-passing baseline's precision), broadcast
                        # across partitions by a K=1 matmul into a borrowed
                        # psO bank; ev leaves PSUM via the ACT engine so the
                        # final multiply has baseline-shaped operands
                        # (SBUF x PSUM).
                        def t_chain():
                            zi = z_pool.tile([1, QG], bf16, tag=f"zi{h % 2}",
                                             name=f"zi{h}")
                            if act_recip:
                                # tail path: ACT is idle there; Zinv =
                                # exp(-ln(Z)) is exact to bf16 rounding
                                zf = z_pool.tile([1, QG], f32,
                                                 tag=f"zf{h % 2}",
                                                 name=f"zf{h}")
                                nc.scalar.activation(zf, pv[D:D + 1, :],
                                                     Act.Ln)
                                nc.scalar.activation(zi, zf, Act.Exp,
                                                     scale=-1.0)
                            else:
                                with nc.allow_low_precision(
                                        reason="bf16 Zinv, like the Wo "
                                               "operands downstream"):
                                    nc.vector.reciprocal(zi, pv[D:D + 1, :])
                            ev = ev_pool.tile([D, QG], f32,
                                              tag=f"ev{h % 2}", name=f"ev{h}")
                            if act_recip:
                                nc.scalar.copy(ev, pv[0:D, :])
                            else:
                                nc.vector.tensor_copy(ev, pv[0:D, :])
                            bps = psO.tile([D, QG], f32, tag="o",
                                           name=f"bps{h}")
                            nc.tensor.matmul(bps, lhsT=ones64, rhs=zi,
                                             start=True, stop=True)
                            nc.vector.tensor_mul(
                                attnP[h // 2][g][po:po + D, :], ev, bps)

                        if defer:
                            return [t_pv, t_chain]
                        t_pv()
                        t_chain()
                        return []

                    def wo_proj(g, last=False):
                        # For the final group the psS pool is free (no more
                        # scores), so use its 2-bank tiles to double the
                        # psum buffering and shorten the drain.
                        for j in range(NQB):
                            st = g * NQB + j
                            ot = out_pool.tile([P, NEG * EGW], bf16, tag="ot")
                            if last:
                                ops2 = psS.tile([P, 2 * QG], f32, tag="s",
                                                name="opss")
                            for eg in range(NEG):
                                if last:
                                    ops = ops2[:, eg * EGW:(eg + 1) * EGW]
                                else:
                                    ops = psO.tile([P, EGW], f32, tag="o",
                                                   name="opso")
                                for p in range(HL // 2):
                                    nc.tensor.matmul(
                                        ops,
                                        lhsT=attnP[p][g][:, j * P:(j + 1) * P],
                                        rhs=wo_sb[p][:, eg * EGW:
                                                     (eg + 1) * EGW],
                                        start=(p == 0),
                                        stop=(p == HL // 2 - 1),
                                        skip_group_check=last)
                                # copy PSUM->SBUF (GPSIMD cannot touch PSUM);
                                # at the tail the idle ACT engine takes half
                                dst = ot[:, eg * EGW:(eg + 1) * EGW]
                                if eg == 0:
                                    nc.scalar.copy(dst, ops)
                                else:
                                    nc.vector.tensor_copy(dst, ops)
                                if last:
                                    nc.sync.dma_start(
                                        out=d_out[st * P:(st + 1) * P,
                                                  eg * EGW:(eg + 1) * EGW],
                                        in_=dst)
                            if not last:
                                nc.sync.dma_start(
                                    out=d_out[st * P:(st + 1) * P, :], in_=ot)

                    # m1 projection piece for one (weight, group): psum
                    # borrowed from the (pass-1-unused) psO pool; e-inner so
                    # the two psO buffers ping-pong.
                    def m1_piece(wname, g):
                        dst = QT if wname == "wq" else KT
                        chunks = qchunks if wname == "wq" else kchunks
                        ps = psO.tile([P, QG], f32, tag="o",
                                      name=f"psqk1{wname}{g}")
                        for e in range(EC):
                            nc.tensor.matmul(
                                ps,
                                lhsT=w_sb[wname][:, e, P:2 * P],
                                rhs=chunks[e][:, g * QG:(g + 1) * QG],
                                start=(e == 0), stop=(e == EC - 1))
                        nc.vector.tensor_copy(
                            dst[1][:, g * QG:(g + 1) * QG], ps)

                    # pass 1: heads 0/1 over all groups (needs only m0 + V),
                    # ascending (ends on the biggest exp backlog).  The 8 m1
                    # projection pieces are woven between the later head
                    # visits, where the exp backlog hides their ACT-less PE
                    # time; PV tails and normalize chains are carried into
                    # the next head so ACT never waits on them.
                    g_up = sorted(range(NQG), key=lambda g: len(kt_offs(g)))
                    m1_sched = {1: ["wq0"], 2: ["wq1"], 3: ["wq2"],
                                4: ["wq3"], 5: ["wk0", "wk1"],
                                6: ["wk2"], 7: ["wk3"]}
                    carry = []
                    for i, g in enumerate(g_up):
                        for h in (0, 1):
                            carry = attn(g, h, carry=carry, defer=True)
                            for pc in m1_sched.get(2 * i + h, []):
                                m1_piece("wq" if pc[:2] == "wq" else "wk",
                                         int(pc[2]))
                    for t in carry:
                        t()

                    # pass 2: heads 2/3 + output projection per group;
                    # descending so the big group lands right after m1 and
                    # the kernel tail ends on the smallest one.  Each group's
                    # h3 tail + output projection are carried into the next
                    # group's first score matmuls; the last group runs its
                    # reciprocals on the (by then idle) ACT engine and
                    # interleaves its two normalize chains to cut the drain.
                    carry = []
                    gs2 = list(reversed(g_up))
                    for i, g in enumerate(gs2):
                        last = (i == NQG - 1)
                        c2 = attn(g, 2, carry=carry, defer=True,
                                  act_recip=last)
                        c3 = attn(g, 3, carry=c2[:1], defer=True,
                                  act_recip=last)
                        if not last:
                            # [pv3, chain2, chain3, wo(g)] ride into the next
                            # group's emission
                            carry = [c3[0], c2[1], c3[1],
                                     lambda g=g: wo_proj(g)]
                        else:
                            c3[0]()
                            c2[1]()
                            c3[1]()
                            wo_proj(g, last=True)

        for _rep in range(repeat):
            emit_once()

    _split_multi_waits(nc)
    return nc


# ---------------------------------------------------------------------------
# Host entry point
# ---------------------------------------------------------------------------
LAST_EXEC_NS = None
LAST_RESULT = None


def kernel(query, key, value, mask, Wq, Wk, Wv, Wo, bo):
    global LAST_EXEC_NS, LAST_RESULT
    _install_tile_drain_patch()
    from concourse.bass_utils import run_bass_kernel_spmd

    B, S, E = 2, 2048, 1024
    H, D = 16, 64
    N_CORES = 8
    BG = 2                    # batch groups
    HG = N_CORES // BG        # head groups per batch
    HL = H // HG              # heads per core
    DIM = HL * D

    query = np.asarray(query, dtype=np.float32)
    key = np.asarray(key, dtype=np.float32)
    value = np.asarray(value, dtype=np.float32)
    mask2d = np.asarray(mask).reshape(S, S).astype(bool)
    Wq = np.asarray(Wq, dtype=np.float32)
    Wk = np.asarray(Wk, dtype=np.float32)
    Wv = np.asarray(Wv, dtype=np.float32)
    Wo = np.asarray(Wo, dtype=np.float32)
    bo = np.asarray(bo, dtype=np.float32)

    bias_idx, biases, block_live = classify_mask(mask2d, S)
    nuniq = len(biases)
    bias_stack = (np.concatenate(biases, axis=1) if nuniq
                  else np.zeros((128, 128), np.float32))

    nc = build_nc(S, E, D, HL, bias_idx, block_live, nuniq)

    scale = np.float32(1.0 / np.sqrt(D))
    in_maps = []
    for c in range(N_CORES):
        b, hg = c // HG, c % HG
        cols = slice(hg * DIM, (hg + 1) * DIM)
        wv_l = Wv[:, cols].reshape(E, HL, D)
        wv_aug = np.zeros((E, HL, D + 1), np.float32)
        wv_aug[:, :, :D] = wv_l
        in_maps.append({
            "xqT": _bf16(query[b].T),
            "xkT": _bf16(key[b].T),
            "xvT": _bf16(value[b].T),
            "wq": _bf16(Wq[:, cols] * scale),
            "wk": _bf16(Wk[:, cols]),
            "wv": _bf16(wv_aug.reshape(E, HL * (D + 1))),
            "wo": _bf16(Wo[cols, :]),
            "biasT": _bf16(bias_stack),
        })

    res = run_bass_kernel_spmd(nc, in_maps, list(range(N_CORES)))
    LAST_RESULT = res
    LAST_EXEC_NS = res.exec_time_ns or res.mean_exec_time_ns

    out = np.empty((B, S, E), np.float32)
    for b in range(BG):
        acc = res.results[b * HG]["out_p"].astype(np.float32)
        for j in range(1, HG):
            acc = acc + res.results[b * HG + j]["out_p"]
        out[b] = acc + bo[None, :]
    return out


def _bf16(a):
    import ml_dtypes
    return np.ascontiguousarray(np.asarray(a, np.float32)).astype(
        ml_dtypes.bfloat16)


# revision 12
# speedup vs baseline: 1.1835x; 1.0060x over previous
"""Multi-head attention (B=2, S=2048, E=1024, H=16) on 8 Trainium2 NeuronCores.

Sharding: core c -> batch c//4, heads 4*(c%4)..4*(c%4)+3  (data + head parallel).
Each core computes a partial output projection [S, E] over its 256 head-dims;
the host sums the 4 partials per batch and adds the output bias.

v2 redesign (driven by the TimelineSim cost model, where every engine op
costs free_size * cycle_t and matmuls cost out_free * 0.4167ns per
contraction chunk; ~151us vs the 179us v1):
  * scores/exp/PV are TRIMMED to the live (causal) columns of each key
    tile: per (group, kt) only q-columns >= the first live q-block are
    computed, masked blocks inside that range get the 0/1 multiply.
  * Zinv = DVE reciprocal of the PV ones-row (bf16); the two heads of a
    pair stack their Zinv rows at partitions 0/64 of a persistent zeroed
    tile and ONE K=65 selector matmul broadcasts both into a borrowed psO
    bank; ev leaves PSUM on DVE so the normalize multiply has SBUF x PSUM
    operands.  (GPSIMD can't touch PSUM; partition_broadcast needs a
    gpsimd library; the verifier rejects two-PSUM-input TensorTensor and
    partition bases other than 0/32/64; matmul over uninitialized SBUF
    rows NaNs even against zero weights - hence the persistent memset.)
  * Output partials leave in bf16 (halves the out-DMA; the host sums the
    4 partials per batch in fp32); output psum->sbuf copies split ACT/DVE,
    the ev copies ride DVE so the in-order ACT queue carries only exps.
  * Phase order: V e-inner (PE runs continuously once xv lands, ramping
    to the full p-state), m0 e-outer, pass 1 (heads 0/1, groups
    ascending) with the 8 m1-projection pieces woven between later head
    visits, pass 2 (heads 2/3, descending) with each group's output
    projection deferred into the next group's emission.  Every head's PV
    tail + normalize chain is carried into the following head's first
    score matmuls so the in-order engine queues never stall on them; the
    last group batches its chains, runs Zinv on the idle ACT engine, and
    drains its output projection through the freed scores psum tiles.
"""

import sys

for _p in ("/opt/trn_rl_repo", "/root/.axon_site/_ro/trn_rl_repo"):
    if _p not in sys.path:
        sys.path.insert(0, _p)

import numpy as np


# ---------------------------------------------------------------------------
# Patch: the walrus build in this container rejects >1 sem wait on one CTRL
# instruction and the TileContext exit drain aggregates every outstanding
# proc's wait onto a single Drain. Spill the excess waits onto SP nops.
# ---------------------------------------------------------------------------
def _install_tile_drain_patch():
    import concourse.tile as tile
    import concourse.mybir as mybir
    from concourse.vector_clock import ScopedClock

    if getattr(tile.TileContext, "_drain_patch_installed", False):
        return

    def _patched_drain_and_barrier(self, tick_clock, wait_clock):
        drain_inst = self.nc.sync.drain()
        wait_clock.add_sem_waits(
            drain_inst.ins, ScopedClock({None: tick_clock.global_clock})
        )
        si = drain_inst.ins.sync_info
        waits = list(si.on_wait) if si and si.on_wait else []
        if len(waits) > 1:
            si.on_wait = waits[:1]
            for w in waits[1:]:
                nop = self.nc.sync.nop(nofuse=True, hint="drain_wait_spill")
                nop.ins.sync_info = mybir.SyncInfo(on_wait=[w], on_update=[])
        self.nc.all_engine_barrier()
        assert self.sems is not None
        popped = self.nc._tile_sem_poison_stack.pop()
        assert popped is self._sem_poison
        self.nc.clear_and_free_semaphores(list(self.sems.allocated().values()))
        self.nc.all_engine_barrier()

    tile.TileContext._drain_and_barrier = _patched_drain_and_barrier
    tile.TileContext._drain_patch_installed = True


def _split_multi_waits(nc, maxw=1):
    """Hoist excess sem waits onto engine-queue NoOps inserted just before
    the instruction (sequencer executes them in order; semantics identical)."""
    import concourse.mybir as mybir

    ctr = 0
    for bb in nc.main_func.blocks:
        new = []
        for inst in bb.instructions:
            si = inst.sync_info
            waits = list(si.on_wait) if si and si.on_wait else []
            if len(waits) > maxw:
                extras = waits[:-maxw]
                si.on_wait = waits[-maxw:]
                for i in range(0, len(extras), maxw):
                    nop = mybir.InstNoOp(
                        name=f"I-waitspill-{ctr}", engine=inst.engine,
                        ins=[], outs=[])
                    ctr += 1
                    nop.sync_info = mybir.SyncInfo(
                        on_wait=extras[i:i + maxw], on_update=[])
                    try:
                        nc.register_instruction(nop, overwrite=True)
                    except Exception:
                        pass
                    new.append(nop)
            new.append(inst)
        bb.instructions = new


# ---------------------------------------------------------------------------
# Mask classification (host side, from the actual mask array).
# Blocks are 128x128 in the *transposed* score layout: block (kt, qb) covers
# keys kt*128.. x queries qb*128...
# ---------------------------------------------------------------------------
def classify_mask(mask2d, S, KB=128):
    nb = S // KB
    assert mask2d.shape == (S, S)
    assert mask2d.any(axis=1).all(), "a query row with no attended key"
    maskT = mask2d.T  # [keys, q]
    uniq = {}
    biases = []
    bias_idx = {}  # (kt, qb) -> None (all attended) or index
    block_live = np.zeros((nb, nb), dtype=bool)  # any attended key in block
    for kt in range(nb):
        for qb in range(nb):
            blk = maskT[kt * KB:(kt + 1) * KB, qb * KB:(qb + 1) * KB]
            if blk.all():
                bias_idx[(kt, qb)] = None
                block_live[kt, qb] = True
            else:
                b = np.where(blk, np.float32(1.0), np.float32(0.0))
                key = b.tobytes()
                if key not in uniq:
                    uniq[key] = len(biases)
                    biases.append(b)
                bias_idx[(kt, qb)] = uniq[key]
                block_live[kt, qb] = blk.any()
    return bias_idx, biases, block_live


# ---------------------------------------------------------------------------
# Bass program builder (one SPMD program, same for all cores).
# ---------------------------------------------------------------------------
def build_nc(S, E, D, HL, bias_idx, block_live, nuniq, shift=32.0, repeat=1):
    import concourse.bass as bass
    import concourse.mybir as mybir
    import concourse.tile as tile

    f32 = mybir.dt.float32
    bf16 = mybir.dt.bfloat16
    Act = mybir.ActivationFunctionType

    P = 128
    EC = E // P              # E chunks (contraction tiles for projections)
    DIM = HL * D             # this core's head dims (256)
    MT = DIM // P            # m-tiles of QT/KT (2)
    QG = 512                 # q-group width
    NQG = S // QG
    NQB = QG // P            # q-blocks per group
    NKT = S // P             # key tiles
    NST = S // P             # s tiles
    VW = HL * (D + 1)        # V width incl. ones columns (260)
    EGW = min(QG, E)         # output E slice width
    NEG = E // EGW           # output E slices (2)

    # live key tiles of group g with the first live q-block column offset;
    # the first entry is widened to cover every column any later kt writes so
    # its start=True matmul initializes the whole accumulation region.
    def kt_offs(g):
        out = []
        for kt in range(NKT):
            lives = [j for j in range(NQB) if block_live[kt, g * NQB + j]]
            if lives:
                out.append((kt, lives[0] * P))
        if out:
            m0 = min(o for _, o in out)
            out[0] = (out[0][0], m0)
        return out

    nc = bass.Bass()
    dp = nc.declare_dram_parameter
    d_xq = dp("xqT", [E, S], bf16, isOutput=False)
    d_xk = dp("xkT", [E, S], bf16, isOutput=False)
    d_xv = dp("xvT", [E, S], bf16, isOutput=False)
    d_wq = dp("wq", [E, DIM], bf16, isOutput=False)
    d_wk = dp("wk", [E, DIM], bf16, isOutput=False)
    d_wv = dp("wv", [E, VW], bf16, isOutput=False)
    d_wo = dp("wo", [DIM, E], bf16, isOutput=False)
    d_bias = dp("biasT", [P, max(nuniq, 1) * P], bf16, isOutput=False)
    d_out = dp("out_p", [S, E], bf16, isOutput=True)

    import contextlib
    with tile.TileContext(nc) as tc, contextlib.ExitStack() as _stk:
        consts = _stk.enter_context(tc.tile_pool(name="consts", bufs=1))

        w_sb = {}
        for nm, width in (("wq", DIM), ("wk", DIM), ("wv", VW)):
            w_sb[nm] = consts.tile([P, EC, width], bf16, name=f"sb_{nm}",
                                   tag=f"sb_{nm}")
        w_dram = {"wq": d_wq, "wk": d_wk, "wv": d_wv}
        wo_sb = [consts.tile([2 * D, E], bf16, name=f"sb_wo{p}",
                             tag=f"sb_wo{p}") for p in range(HL // 2)]
        bias_sb = consts.tile([P, max(nuniq, 1) * P], bf16, name="sb_bias")
        negshift = consts.tile([P, 1], f32, name="negshift")
        nc.vector.memset(negshift, -shift)
        # selector for the pair Zinv broadcast: the two heads' Zinv rows
        # live at partitions 0 and 64 (the only legal cross-partition write
        # offsets); out rows 0..63 copy row 0, rows 64..127 copy row 64
        sel2 = consts.tile([D + 1, P], bf16, name="sel2")
        nc.vector.memset(sel2, 0.0)
        nc.vector.memset(sel2[0:1, 0:D], 1.0)
        nc.vector.memset(sel2[D:D + 1, D:P], 1.0)

        def load_w(nm):
            nc.sync.dma_start(
                out=w_sb[nm],
                in_=w_dram[nm][:, :].rearrange("(e p) n -> p e n", p=P))

        def emit_once():
            # persistent projection outputs
            QT = [consts.tile([P, S], bf16, name=f"QT{m}", tag=f"QT{m}")
                  for m in range(MT)]
            KT = [consts.tile([P, S], bf16, name=f"KT{m}", tag=f"KT{m}")
                  for m in range(MT)]
            V = [consts.tile([P, VW], bf16, name=f"V{s}", tag=f"V{s}")
                 for s in range(NST)]
            attnP = [[consts.tile([2 * D, QG], bf16, name=f"attnP{p}g{g}",
                                  tag=f"attnP{p}g{g}") for g in range(NQG)]
                     for p in range(HL // 2)]
            # persistent, fully-zeroed pair-Zinv tiles (rows 1..63 must be
            # finite zeros: the K=65 selector matmul reads every partition)
            z2 = [consts.tile([D + 1, QG], bf16, name=f"zi2_{i}",
                              tag=f"zi2_{i}") for i in range(2)]
            for t2 in z2:
                nc.vector.memset(t2, 0.0)

            with tc.tile_pool(name="xt", bufs=2 * EC + 4) as xt_pool:
                psA_ctx = contextlib.ExitStack()
                psV = psA_ctx.enter_context(
                    tc.tile_pool(name="psV", bufs=1, space="PSUM"))
                psM = psA_ctx.enter_context(
                    tc.tile_pool(name="psM", bufs=1, space="PSUM"))

                def stream_chunks(dram):
                    chunks = []
                    for e in range(EC):
                        ch = xt_pool.tile([P, S], bf16, tag="xt", name=f"xch{e}")
                        nc.sync.dma_start(out=ch, in_=dram[e * P:(e + 1) * P, :])
                        chunks.append(ch)
                    return chunks

                # ---------------- phase A: projections ----------------
                # wv chunk 0 first so the first V matmul only waits ~1.6us
                nc.sync.dma_start(out=w_sb["wv"][:, 0, :], in_=d_wv[0:P, :])
                vchunks = [xt_pool.tile([P, S], bf16, tag="xt", name="xch0")]
                nc.sync.dma_start(out=vchunks[0], in_=d_xv[0:P, :])
                nc.sync.dma_start(
                    out=w_sb["wv"][:, 1:EC, :],
                    in_=d_wv[P:, :].rearrange("(e p) n -> p e n", p=P))
                for e in range(1, EC):
                    ch = xt_pool.tile([P, S], bf16, tag="xt", name=f"xch{e}")
                    nc.sync.dma_start(out=ch, in_=d_xv[e * P:(e + 1) * P, :])
                    vchunks.append(ch)
                load_w("wq")
                qchunks = stream_chunks(d_xq)
                load_w("wk")
                kchunks = stream_chunks(d_xk)
                for p in range(HL // 2):
                    nc.sync.dma_start(
                        out=wo_sb[p], in_=d_wo[p * 2 * D:(p + 1) * 2 * D, :])
                nc.sync.dma_start(out=bias_sb, in_=d_bias[:, :])

                def finish_v(st, ps):
                    nc.vector.tensor_copy(V[st], ps)
                    onescols = V[st].rearrange(
                        "p (h c) -> p h c", c=D + 1)[:, :, D]
                    nc.gpsimd.memset(onescols, 1.0)

                # V tiles e-inner, starting once the whole xv stream has
                # landed (~11.4us): the PE then runs CONTINUOUSLY (ramping to
                # the full p-state) through V and the m0 projections while
                # xq/xk stream in, instead of chunk-paced stuttering.
                for st in range(0, NST):
                    ps = psV.tile([P, VW], f32, tag=f"v{st % 4}",
                                  name=f"psv{st}")
                    for e in range(EC):
                        nc.tensor.matmul(
                            ps,
                            lhsT=vchunks[e][:, st * P:(st + 1) * P],
                            rhs=w_sb["wv"][:, e, :],
                            start=(e == 0), stop=(e == EC - 1))
                    finish_v(st, ps)

                # QK projections for m-tile m into QT[m]/KT[m]; e-outer over
                # the 4 q-groups of each (weight, group) so PE work is
                # chunk-paced.  `pool`/`tag` chooses the psum bank set.
                def qk_proj(m, pool, tagf):
                    for wname, dst, chunks in (("wq", QT, qchunks),
                                               ("wk", KT, kchunks)):
                        pss = [pool.tile([P, QG], f32, tag=tagf(g),
                                         name=f"psqk{m}{wname}{g}")
                               for g in range(NQG)]
                        for e in range(EC):
                            for g in range(NQG):
                                nc.tensor.matmul(
                                    pss[g],
                                    lhsT=w_sb[wname][:, e, m * P:(m + 1) * P],
                                    rhs=chunks[e][:, g * QG:(g + 1) * QG],
                                    start=(e == 0), stop=(e == EC - 1))
                        for g in range(NQG):
                            nc.vector.tensor_copy(
                                dst[m][:, g * QG:(g + 1) * QG], pss[g])

                qk_proj(0, psM, lambda g: f"m{g}")
                psA_ctx.close()  # free the 8 phase-A PSUM banks for phase B

                # ---------------- phase B: attention ----------------
                # PSUM bank alignment: psPV/psO open first so they land on
                # the psV banks (free early); psS lands on the psM banks,
                # which free exactly when the m0 copies complete.
                with tc.tile_pool(name="probs", bufs=6) as probs_pool, \
                     tc.tile_pool(name="zrow", bufs=2) as z_pool, \
                     tc.tile_pool(name="evs", bufs=2) as ev_pool, \
                     tc.tile_pool(name="outst", bufs=4) as out_pool, \
                     tc.tile_pool(name="psS", bufs=2, space="PSUM") as psS, \
                     tc.tile_pool(name="psPV", bufs=2, space="PSUM") as psPV, \
                     tc.tile_pool(name="psO", bufs=2, space="PSUM") as psO:

                    def attn(g, h, carry=(), defer=False, act_recip=False,
                             pair_ctx=None):
                        """Emit attention for (g, h).  `carry` holds the
                        previous head's deferred PV-flush + normalize chain;
                        it is emitted right after this head's first score
                        matmuls so the ACT engine sees the next exp without
                        waiting for the previous head's PV tail.  With
                        defer=True the tail thunks are returned instead of
                        emitted."""
                        m, po = h // 2, (h % 2) * D
                        kts = kt_offs(g)
                        total = len(kts)
                        pairs = [kts[i:i + 2] for i in range(0, total, 2)]
                        pv = psPV.tile([D + 1, QG], f32, tag="pv")
                        npv = 0
                        pend = []
                        carried = list(carry)

                        def emit_pv(entry):
                            nonlocal npv
                            pb, regions = entry
                            for (kt, off), c, w in regions:
                                nc.tensor.matmul(
                                    pv[0:D + 1, off:QG],
                                    lhsT=V[kt][:, h * (D + 1):
                                               (h + 1) * (D + 1)],
                                    rhs=pb[:, c:c + w],
                                    start=(npv == 0),
                                    stop=(npv == total - 1),
                                    skip_group_check=True)
                                npv += 1

                        for pi, pair in enumerate(pairs):
                            regions = []
                            col = 0
                            for (kt, off) in pair:
                                w = QG - off
                                regions.append(((kt, off), col, w))
                                col += w
                            sps = psS.tile([P, 2 * QG], f32, tag="s")
                            pb = probs_pool.tile([P, 2 * QG], bf16, tag="pb")
                            for (kt, off), c, w in regions:
                                nc.tensor.matmul(
                                    sps[:, c:c + w],
                                    lhsT=KT[m][po:po + D, kt * P:(kt + 1) * P],
                                    rhs=QT[m][po:po + D,
                                              g * QG + off:(g + 1) * QG],
                                    start=True, stop=True)
                            if pi == 0:
                                while carried:
                                    carried.pop(0)()
                            nc.scalar.activation(pb[:, 0:col], sps[:, 0:col],
                                                 Act.Exp,
                                                 bias=negshift[:, 0:1])
                            # masking after exp: multiplicative 0/1, exact
                            for (kt, off), c, w in regions:
                                for j in range(off // P, NQB):
                                    qb = g * NQB + j
                                    bidx = bias_idx[(kt, qb)]
                                    if bidx is None:
                                        continue
                                    cc = c + j * P - off
                                    blk = pb[:, cc:cc + P]
                                    if not block_live[kt, qb]:
                                        nc.gpsimd.memset(blk, 0.0)
                                    else:
                                        nc.vector.tensor_mul(
                                            blk, blk,
                                            bias_sb[:, bidx * P:
                                                    (bidx + 1) * P])
                            pend.append((pb, regions))
                            if len(pend) > 3:
                                emit_pv(pend.pop(0))

                        def t_pv():
                            while pend:
                                emit_pv(pend.pop(0))

                        # normalize: Zinv = 1/Z (bf16, matching the
                        # reference-passing baseline's precision).  The two
                        # heads of a pair stack their Zinv rows in one
                        # [2, QG] tile; the ODD head's chain broadcasts both
                        # with a single K=2 selector matmul into a borrowed
                        # psO bank, then multiplies both ev tiles
                        # (SBUF x PSUM, verifier-friendly).
                        def t_chain():
                            zi2 = z2[pair_ctx["idx"]]
                            zo = (h % 2) * D
                            zrow = zi2[zo:zo + 1, :]
                            if act_recip:
                                # tail path: ACT is idle there; Zinv =
                                # exp(-ln(Z)) is exact to bf16 rounding
                                zf = z_pool.tile([1, QG], f32,
                                                 tag=f"zf{h % 2}",
                                                 name=f"zf{h}")
                                nc.scalar.activation(zf, pv[D:D + 1, :],
                                                     Act.Ln)
                                nc.scalar.activation(zrow, zf, Act.Exp,
                                                     scale=-1.0)
                            else:
                                with nc.allow_low_precision(
                                        reason="bf16 Zinv, like the Wo "
                                               "operands downstream"):
                                    nc.vector.reciprocal(zrow, pv[D:D + 1, :])
                            ev = ev_pool.tile([D, QG], f32,
                                              tag=f"ev{h % 2}", name=f"ev{h}")
                            if act_recip:
                                nc.scalar.copy(ev, pv[0:D, :])
                            else:
                                nc.vector.tensor_copy(ev, pv[0:D, :])
                            pair_ctx[f"ev{h % 2}"] = ev
                            if h % 2 == 1:
                                bps2 = psO.tile([P, QG], f32, tag="o",
                                                name=f"bps{h}")
                                nc.tensor.matmul(bps2, lhsT=sel2, rhs=zi2,
                                                 start=True, stop=True)
                                nc.vector.tensor_mul(
                                    attnP[h // 2][g][0:D, :],
                                    pair_ctx["ev0"], bps2[0:D, :])
                                nc.vector.tensor_mul(
                                    attnP[h // 2][g][D:2 * D, :],
                                    pair_ctx["ev1"], bps2[D:2 * D, :])

                        if defer:
                            return [t_pv, t_chain]
                        t_pv()
                        t_chain()
                        return []

                    def wo_proj(g, last=False):
                        # For the final group the psS pool is free (no more
                        # scores), so use its 2-bank tiles to double the
                        # psum buffering and shorten the drain.
                        for j in range(NQB):
                            st = g * NQB + j
                            ot = out_pool.tile([P, NEG * EGW], bf16, tag="ot")
                            if last:
                                ops2 = psS.tile([P, 2 * QG], f32, tag="s",
                                                name="opss")
                            for eg in range(NEG):
                                if last:
                                    ops = ops2[:, eg * EGW:(eg + 1) * EGW]
                                else:
                                    ops = psO.tile([P, EGW], f32, tag="o",
                                                   name="opso")
                                for p in range(HL // 2):
                                    nc.tensor.matmul(
                                        ops,
                                        lhsT=attnP[p][g][:, j * P:(j + 1) * P],
                                        rhs=wo_sb[p][:, eg * EGW:
                                                     (eg + 1) * EGW],
                                        start=(p == 0),
                                        stop=(p == HL // 2 - 1),
                                        skip_group_check=last)
                                # copy PSUM->SBUF (GPSIMD cannot touch PSUM);
                                # at the tail the idle ACT engine takes half
                                dst = ot[:, eg * EGW:(eg + 1) * EGW]
                                if eg == 0:
                                    nc.scalar.copy(dst, ops)
                                else:
                                    nc.vector.tensor_copy(dst, ops)
                                if last:
                                    nc.sync.dma_start(
                                        out=d_out[st * P:(st + 1) * P,
                                                  eg * EGW:(eg + 1) * EGW],
                                        in_=dst)
                            if not last:
                                nc.sync.dma_start(
                                    out=d_out[st * P:(st + 1) * P, :], in_=ot)

                    # m1 projection piece for one (weight, group): psum
                    # borrowed from the (pass-1-unused) psO pool; e-inner so
                    # the two psO buffers ping-pong.
                    def m1_piece(wname, g):
                        dst = QT if wname == "wq" else KT
                        chunks = qchunks if wname == "wq" else kchunks
                        ps = psO.tile([P, QG], f32, tag="o",
                                      name=f"psqk1{wname}{g}")
                        for e in range(EC):
                            nc.tensor.matmul(
                                ps,
                                lhsT=w_sb[wname][:, e, P:2 * P],
                                rhs=chunks[e][:, g * QG:(g + 1) * QG],
                                start=(e == 0), stop=(e == EC - 1))
                        nc.vector.tensor_copy(
                            dst[1][:, g * QG:(g + 1) * QG], ps)

                    # pass 1: heads 0/1 over all groups (needs only m0 + V),
                    # ascending (ends on the biggest exp backlog).  The 8 m1
                    # projection pieces are woven between the later head
                    # visits, where the exp backlog hides their ACT-less PE
                    # time; PV tails and normalize chains are carried into
                    # the next head so ACT never waits on them.
                    g_up = sorted(range(NQG), key=lambda g: len(kt_offs(g)))
                    m1_sched = {1: ["wq0"], 2: ["wq1"], 3: ["wq2"],
                                4: ["wq3"], 5: ["wk0", "wk1"],
                                6: ["wk2"], 7: ["wk3"]}
                    carry = []
                    for i, g in enumerate(g_up):
                        pctx = {"idx": i % 2}
                        for h in (0, 1):
                            carry = attn(g, h, carry=carry, defer=True,
                                         pair_ctx=pctx)
                            for pc in m1_sched.get(2 * i + h, []):
                                m1_piece("wq" if pc[:2] == "wq" else "wk",
                                         int(pc[2]))
                    for t in carry:
                        t()

                    # pass 2: heads 2/3 + output projection per group;
                    # descending so the big group lands right after m1 and
                    # the kernel tail ends on the smallest one.  Each group's
                    # h3 tail + output projection are carried into the next
                    # group's first score matmuls; the last group runs its
                    # reciprocals on the (by then idle) ACT engine and
                    # interleaves its two normalize chains to cut the drain.
                    carry = []
                    gs2 = list(reversed(g_up))
                    for i, g in enumerate(gs2):
                        last = (i == NQG - 1)
                        pctx = {"idx": i % 2}
                        c2 = attn(g, 2, carry=carry, defer=True,
                                  pair_ctx=pctx)
                        c3 = attn(g, 3, carry=c2[:1], defer=True,
                                  act_recip=last, pair_ctx=pctx)
                        if not last:
                            # [pv3, chain2, chain3, wo(g)] ride into the next
                            # group's emission
                            carry = [c3[0], c2[1], c3[1],
                                     lambda g=g: wo_proj(g)]
                        else:
                            c3[0]()
                            c2[1]()
                            c3[1]()
                            wo_proj(g, last=True)

        for _rep in range(repeat):
            emit_once()

    _split_multi_waits(nc)
    return nc


# ---------------------------------------------------------------------------
# Host entry point
# ---------------------------------------------------------------------------
LAST_EXEC_NS = None
LAST_RESULT = None


def kernel(query, key, value, mask, Wq, Wk, Wv, Wo, bo):
    global LAST_EXEC_NS, LAST_RESULT
    _install_tile_drain_patch()
    from concourse.bass_utils import run_bass_kernel_spmd

    B, S, E = 2, 2048, 1024
    H, D = 16, 64
    N_CORES = 8
    BG = 2                    # batch groups
    HG = N_CORES // BG        # head groups per batch
    HL = H // HG              # heads per core
    DIM = HL * D

    query = np.asarray(query, dtype=np.float32)
    key = np.asarray(key, dtype=np.float32)
    value = np.asarray(value, dtype=np.float32)
    mask2d = np.asarray(mask).reshape(S, S).astype(bool)
    Wq = np.asarray(Wq, dtype=np.float32)
    Wk = np.asarray(Wk, dtype=np.float32)
    Wv = np.asarray(Wv, dtype=np.float32)
    Wo = np.asarray(Wo, dtype=np.float32)
    bo = np.asarray(bo, dtype=np.float32)

    bias_idx, biases, block_live = classify_mask(mask2d, S)
    nuniq = len(biases)
    bias_stack = (np.concatenate(biases, axis=1) if nuniq
                  else np.zeros((128, 128), np.float32))

    nc = build_nc(S, E, D, HL, bias_idx, block_live, nuniq)

    scale = np.float32(1.0 / np.sqrt(D))
    in_maps = []
    for c in range(N_CORES):
        b, hg = c // HG, c % HG
        cols = slice(hg * DIM, (hg + 1) * DIM)
        wv_l = Wv[:, cols].reshape(E, HL, D)
        wv_aug = np.zeros((E, HL, D + 1), np.float32)
        wv_aug[:, :, :D] = wv_l
        in_maps.append({
            "xqT": _bf16(query[b].T),
            "xkT": _bf16(key[b].T),
            "xvT": _bf16(value[b].T),
            "wq": _bf16(Wq[:, cols] * scale),
            "wk": _bf16(Wk[:, cols]),
            "wv": _bf16(wv_aug.reshape(E, HL * (D + 1))),
            "wo": _bf16(Wo[cols, :]),
            "biasT": _bf16(bias_stack),
        })

    res = run_bass_kernel_spmd(nc, in_maps, list(range(N_CORES)))
    LAST_RESULT = res
    LAST_EXEC_NS = res.exec_time_ns or res.mean_exec_time_ns

    out = np.empty((B, S, E), np.float32)
    for b in range(BG):
        acc = res.results[b * HG]["out_p"].astype(np.float32)
        for j in range(1, HG):
            acc = acc + res.results[b * HG + j]["out_p"]
        out[b] = acc + bo[None, :]
    return out


def _bf16(a):
    import ml_dtypes
    return np.ascontiguousarray(np.asarray(a, np.float32)).astype(
        ml_dtypes.bfloat16)


# revision 14
# speedup vs baseline: 1.2023x; 1.0159x over previous
"""Multi-head attention (B=2, S=2048, E=1024, H=16) on 8 Trainium2 NeuronCores.

Sharding: core c -> batch c//4, heads 4*(c%4)..4*(c%4)+3  (data + head parallel).
Each core computes a partial output projection [S, E] over its 256 head-dims;
the host sums the 4 partials per batch and adds the output bias.

v2 redesign (driven by the TimelineSim cost model, where every engine op
costs free_size * cycle_t and matmuls cost out_free * 0.4167ns per
contraction chunk; ~149us vs the 179us v1):
  * scores/exp/PV are TRIMMED to the live (causal) columns of each key
    tile: per (group, kt) only q-columns >= the first live q-block are
    computed, masked blocks inside that range get the 0/1 multiply.
  * Zinv = DVE reciprocal of the PV ones-row (bf16); the two heads of a
    pair stack their Zinv rows at partitions 0/64 of a persistent zeroed
    tile and ONE K=65 selector matmul broadcasts both into a borrowed psO
    bank; both heads' ev also stack (bases 0/64) so the pair normalizes
    with a single full-width SBUF x PSUM multiply.  (GPSIMD can't touch PSUM; partition_broadcast needs a
    gpsimd library; the verifier rejects two-PSUM-input TensorTensor and
    partition bases other than 0/32/64; matmul over uninitialized SBUF
    rows NaNs even against zero weights - hence the persistent memset.)
  * Output partials leave in bf16 (halves the out-DMA; the host sums the
    4 partials per batch in fp32); output psum->sbuf copies split ACT/DVE,
    the ev copies ride DVE so the in-order ACT queue carries only exps.
  * Phase order: V e-inner (PE runs continuously once xv lands, ramping
    to the full p-state), m0 e-outer, pass 1 (heads 0/1, groups
    ascending) with the 8 m1-projection pieces woven between later head
    visits, pass 2 (heads 2/3, descending) with each group's output
    projection deferred into the next group's emission.  Every head's PV
    tail + normalize chain is carried into the following head's first
    score matmuls so the in-order engine queues never stall on them; the
    last group batches its chains, runs Zinv on the idle ACT engine, and
    drains its output projection through the freed scores psum tiles.
"""

import sys

for _p in ("/opt/trn_rl_repo", "/root/.axon_site/_ro/trn_rl_repo"):
    if _p not in sys.path:
        sys.path.insert(0, _p)

import numpy as np


# ---------------------------------------------------------------------------
# Patch: the walrus build in this container rejects >1 sem wait on one CTRL
# instruction and the TileContext exit drain aggregates every outstanding
# proc's wait onto a single Drain. Spill the excess waits onto SP nops.
# ---------------------------------------------------------------------------
def _install_tile_drain_patch():
    import concourse.tile as tile
    import concourse.mybir as mybir
    from concourse.vector_clock import ScopedClock

    if getattr(tile.TileContext, "_drain_patch_installed", False):
        return

    def _patched_drain_and_barrier(self, tick_clock, wait_clock):
        drain_inst = self.nc.sync.drain()
        wait_clock.add_sem_waits(
            drain_inst.ins, ScopedClock({None: tick_clock.global_clock})
        )
        si = drain_inst.ins.sync_info
        waits = list(si.on_wait) if si and si.on_wait else []
        if len(waits) > 1:
            si.on_wait = waits[:1]
            for w in waits[1:]:
                nop = self.nc.sync.nop(nofuse=True, hint="drain_wait_spill")
                nop.ins.sync_info = mybir.SyncInfo(on_wait=[w], on_update=[])
        self.nc.all_engine_barrier()
        assert self.sems is not None
        popped = self.nc._tile_sem_poison_stack.pop()
        assert popped is self._sem_poison
        self.nc.clear_and_free_semaphores(list(self.sems.allocated().values()))
        self.nc.all_engine_barrier()

    tile.TileContext._drain_and_barrier = _patched_drain_and_barrier
    tile.TileContext._drain_patch_installed = True


def _split_multi_waits(nc, maxw=1):
    """Hoist excess sem waits onto engine-queue NoOps inserted just before
    the instruction (sequencer executes them in order; semantics identical)."""
    import concourse.mybir as mybir

    ctr = 0
    for bb in nc.main_func.blocks:
        new = []
        for inst in bb.instructions:
            si = inst.sync_info
            waits = list(si.on_wait) if si and si.on_wait else []
            if len(waits) > maxw:
                extras = waits[:-maxw]
                si.on_wait = waits[-maxw:]
                for i in range(0, len(extras), maxw):
                    nop = mybir.InstNoOp(
                        name=f"I-waitspill-{ctr}", engine=inst.engine,
                        ins=[], outs=[])
                    ctr += 1
                    nop.sync_info = mybir.SyncInfo(
                        on_wait=extras[i:i + maxw], on_update=[])
                    try:
                        nc.register_instruction(nop, overwrite=True)
                    except Exception:
                        pass
                    new.append(nop)
            new.append(inst)
        bb.instructions = new


# ---------------------------------------------------------------------------
# Mask classification (host side, from the actual mask array).
# Blocks are 128x128 in the *transposed* score layout: block (kt, qb) covers
# keys kt*128.. x queries qb*128...
# ---------------------------------------------------------------------------
def classify_mask(mask2d, S, KB=128):
    nb = S // KB
    assert mask2d.shape == (S, S)
    assert mask2d.any(axis=1).all(), "a query row with no attended key"
    maskT = mask2d.T  # [keys, q]
    uniq = {}
    biases = []
    bias_idx = {}  # (kt, qb) -> None (all attended) or index
    block_live = np.zeros((nb, nb), dtype=bool)  # any attended key in block
    for kt in range(nb):
        for qb in range(nb):
            blk = maskT[kt * KB:(kt + 1) * KB, qb * KB:(qb + 1) * KB]
            if blk.all():
                bias_idx[(kt, qb)] = None
                block_live[kt, qb] = True
            else:
                b = np.where(blk, np.float32(1.0), np.float32(0.0))
                key = b.tobytes()
                if key not in uniq:
                    uniq[key] = len(biases)
                    biases.append(b)
                bias_idx[(kt, qb)] = uniq[key]
                block_live[kt, qb] = blk.any()
    return bias_idx, biases, block_live


# ---------------------------------------------------------------------------
# Bass program builder (one SPMD program, same for all cores).
# ---------------------------------------------------------------------------
def build_nc(S, E, D, HL, bias_idx, block_live, nuniq, shift=32.0, repeat=1):
    import concourse.bass as bass
    import concourse.mybir as mybir
    import concourse.tile as tile

    f32 = mybir.dt.float32
    bf16 = mybir.dt.bfloat16
    Act = mybir.ActivationFunctionType

    P = 128
    EC = E // P              # E chunks (contraction tiles for projections)
    DIM = HL * D             # this core's head dims (256)
    MT = DIM // P            # m-tiles of QT/KT (2)
    QG = 512                 # q-group width
    NQG = S // QG
    NQB = QG // P            # q-blocks per group
    NKT = S // P             # key tiles
    NST = S // P             # s tiles
    VW = HL * (D + 1)        # V width incl. ones columns (260)
    EGW = min(QG, E)         # output E slice width
    NEG = E // EGW           # output E slices (2)

    # live key tiles of group g with the first live q-block column offset;
    # the first entry is widened to cover every column any later kt writes so
    # its start=True matmul initializes the whole accumulation region.
    def kt_offs(g):
        out = []
        for kt in range(NKT):
            lives = [j for j in range(NQB) if block_live[kt, g * NQB + j]]
            if lives:
                out.append((kt, lives[0] * P))
        if out:
            m0 = min(o for _, o in out)
            out[0] = (out[0][0], m0)
        return out

    nc = bass.Bass()
    dp = nc.declare_dram_parameter
    d_xq = dp("xqT", [E, S], bf16, isOutput=False)
    d_xk = dp("xkT", [E, S], bf16, isOutput=False)
    d_xv = dp("xvT", [E, S], bf16, isOutput=False)
    d_wq = dp("wq", [E, DIM], bf16, isOutput=False)
    d_wk = dp("wk", [E, DIM], bf16, isOutput=False)
    d_wv = dp("wv", [E, VW], bf16, isOutput=False)
    d_wo = dp("wo", [DIM, E], bf16, isOutput=False)
    d_bias = dp("biasT", [P, max(nuniq, 1) * P], bf16, isOutput=False)
    d_out = dp("out_p", [S, E], bf16, isOutput=True)

    import contextlib
    with tile.TileContext(nc) as tc, contextlib.ExitStack() as _stk:
        consts = _stk.enter_context(tc.tile_pool(name="consts", bufs=1))

        w_sb = {}
        for nm, width in (("wq", DIM), ("wk", DIM), ("wv", VW)):
            w_sb[nm] = consts.tile([P, EC, width], bf16, name=f"sb_{nm}",
                                   tag=f"sb_{nm}")
        w_dram = {"wq": d_wq, "wk": d_wk, "wv": d_wv}
        wo_sb = [consts.tile([2 * D, E], bf16, name=f"sb_wo{p}",
                             tag=f"sb_wo{p}") for p in range(HL // 2)]
        bias_sb = consts.tile([P, max(nuniq, 1) * P], bf16, name="sb_bias")
        negshift = consts.tile([P, 1], f32, name="negshift")
        nc.vector.memset(negshift, -shift)
        # selector for the pair Zinv broadcast: the two heads' Zinv rows
        # live at partitions 0 and 64 (the only legal cross-partition write
        # offsets); out rows 0..63 copy row 0, rows 64..127 copy row 64
        sel2 = consts.tile([D + 1, P], bf16, name="sel2")
        nc.vector.memset(sel2, 0.0)
        nc.vector.memset(sel2[0:1, 0:D], 1.0)
        nc.vector.memset(sel2[D:D + 1, D:P], 1.0)

        def load_w(nm):
            nc.sync.dma_start(
                out=w_sb[nm],
                in_=w_dram[nm][:, :].rearrange("(e p) n -> p e n", p=P))

        def emit_once():
            # persistent projection outputs
            QT = [consts.tile([P, S], bf16, name=f"QT{m}", tag=f"QT{m}")
                  for m in range(MT)]
            KT = [consts.tile([P, S], bf16, name=f"KT{m}", tag=f"KT{m}")
                  for m in range(MT)]
            V = [consts.tile([P, VW], bf16, name=f"V{s}", tag=f"V{s}")
                 for s in range(NST)]
            attnP = [[consts.tile([2 * D, QG], bf16, name=f"attnP{p}g{g}",
                                  tag=f"attnP{p}g{g}") for g in range(NQG)]
                     for p in range(HL // 2)]
            # persistent, fully-zeroed pair-Zinv tiles (rows 1..63 must be
            # finite zeros: the K=65 selector matmul reads every partition)
            z2 = [consts.tile([D + 1, QG], bf16, name=f"zi2_{i}",
                              tag=f"zi2_{i}") for i in range(2)]
            for t2 in z2:
                nc.vector.memset(t2, 0.0)

            with tc.tile_pool(name="xt", bufs=2 * EC + 4) as xt_pool:
                psA_ctx = contextlib.ExitStack()
                psV = psA_ctx.enter_context(
                    tc.tile_pool(name="psV", bufs=1, space="PSUM"))
                psM = psA_ctx.enter_context(
                    tc.tile_pool(name="psM", bufs=1, space="PSUM"))

                def stream_chunks(dram):
                    chunks = []
                    for e in range(EC):
                        ch = xt_pool.tile([P, S], bf16, tag="xt", name=f"xch{e}")
                        nc.sync.dma_start(out=ch, in_=dram[e * P:(e + 1) * P, :])
                        chunks.append(ch)
                    return chunks

                # ---------------- phase A: projections ----------------
                # wv chunk 0 first so the first V matmul only waits ~1.6us
                nc.sync.dma_start(out=w_sb["wv"][:, 0, :], in_=d_wv[0:P, :])
                vchunks = [xt_pool.tile([P, S], bf16, tag="xt", name="xch0")]
                nc.sync.dma_start(out=vchunks[0], in_=d_xv[0:P, :])
                nc.sync.dma_start(
                    out=w_sb["wv"][:, 1:EC, :],
                    in_=d_wv[P:, :].rearrange("(e p) n -> p e n", p=P))
                for e in range(1, EC):
                    ch = xt_pool.tile([P, S], bf16, tag="xt", name=f"xch{e}")
                    nc.sync.dma_start(out=ch, in_=d_xv[e * P:(e + 1) * P, :])
                    vchunks.append(ch)
                load_w("wq")
                qchunks = stream_chunks(d_xq)
                load_w("wk")
                kchunks = stream_chunks(d_xk)
                for p in range(HL // 2):
                    nc.sync.dma_start(
                        out=wo_sb[p], in_=d_wo[p * 2 * D:(p + 1) * 2 * D, :])
                nc.sync.dma_start(out=bias_sb, in_=d_bias[:, :])

                def finish_v(st, ps):
                    nc.vector.tensor_copy(V[st], ps)
                    onescols = V[st].rearrange(
                        "p (h c) -> p h c", c=D + 1)[:, :, D]
                    nc.gpsimd.memset(onescols, 1.0)

                # V tiles e-inner, starting once the whole xv stream has
                # landed (~11.4us): the PE then runs CONTINUOUSLY (ramping to
                # the full p-state) through V and the m0 projections while
                # xq/xk stream in, instead of chunk-paced stuttering.
                for st in range(0, NST):
                    ps = psV.tile([P, VW], f32, tag=f"v{st % 4}",
                                  name=f"psv{st}")
                    for e in range(EC):
                        nc.tensor.matmul(
                            ps,
                            lhsT=vchunks[e][:, st * P:(st + 1) * P],
                            rhs=w_sb["wv"][:, e, :],
                            start=(e == 0), stop=(e == EC - 1))
                    finish_v(st, ps)

                # QK projections for m-tile m into QT[m]/KT[m]; e-outer over
                # the 4 q-groups of each (weight, group) so PE work is
                # chunk-paced.  `pool`/`tag` chooses the psum bank set.
                def qk_proj(m, pool, tagf):
                    for wname, dst, chunks in (("wq", QT, qchunks),
                                               ("wk", KT, kchunks)):
                        pss = [pool.tile([P, QG], f32, tag=tagf(g),
                                         name=f"psqk{m}{wname}{g}")
                               for g in range(NQG)]
                        for e in range(EC):
                            for g in range(NQG):
                                nc.tensor.matmul(
                                    pss[g],
                                    lhsT=w_sb[wname][:, e, m * P:(m + 1) * P],
                                    rhs=chunks[e][:, g * QG:(g + 1) * QG],
                                    start=(e == 0), stop=(e == EC - 1))
                        for g in range(NQG):
                            # the first two K copies gate the first attention
                            # scores (data + psS bank WAR); run them on the
                            # still-idle ACT engine, in parallel with DVE
                            if wname == "wk" and g < 2:
                                nc.scalar.copy(
                                    dst[m][:, g * QG:(g + 1) * QG], pss[g])
                            else:
                                nc.vector.tensor_copy(
                                    dst[m][:, g * QG:(g + 1) * QG], pss[g])

                qk_proj(0, psM, lambda g: f"m{g}")
                psA_ctx.close()  # free the 8 phase-A PSUM banks for phase B

                # ---------------- phase B: attention ----------------
                # PSUM bank alignment: psPV/psO open first so they land on
                # the psV banks (free early); psS lands on the psM banks,
                # which free exactly when the m0 copies complete.
                with tc.tile_pool(name="probs", bufs=6) as probs_pool, \
                     tc.tile_pool(name="zrow", bufs=2) as z_pool, \
                     tc.tile_pool(name="evs", bufs=2) as ev_pool, \
                     tc.tile_pool(name="outst", bufs=4) as out_pool, \
                     tc.tile_pool(name="psS", bufs=2, space="PSUM") as psS, \
                     tc.tile_pool(name="psPV", bufs=2, space="PSUM") as psPV, \
                     tc.tile_pool(name="psO", bufs=2, space="PSUM") as psO:

                    def attn(g, h, carry=(), defer=False, act_recip=False,
                             pair_ctx=None):
                        """Emit attention for (g, h).  `carry` holds the
                        previous head's deferred PV-flush + normalize chain;
                        it is emitted right after this head's first score
                        matmuls so the ACT engine sees the next exp without
                        waiting for the previous head's PV tail.  With
                        defer=True the tail thunks are returned instead of
                        emitted."""
                        m, po = h // 2, (h % 2) * D
                        kts = kt_offs(g)
                        total = len(kts)
                        pairs = [kts[i:i + 2] for i in range(0, total, 2)]
                        pv = psPV.tile([D + 1, QG], f32, tag="pv")
                        npv = 0
                        pend = []
                        carried = list(carry)

                        def emit_pv(entry):
                            nonlocal npv
                            pb, regions = entry
                            for (kt, off), c, w in regions:
                                nc.tensor.matmul(
                                    pv[0:D + 1, off:QG],
                                    lhsT=V[kt][:, h * (D + 1):
                                               (h + 1) * (D + 1)],
                                    rhs=pb[:, c:c + w],
                                    start=(npv == 0),
                                    stop=(npv == total - 1),
                                    skip_group_check=True)
                                npv += 1

                        for pi, pair in enumerate(pairs):
                            regions = []
                            col = 0
                            for (kt, off) in pair:
                                w = QG - off
                                regions.append(((kt, off), col, w))
                                col += w
                            sps = psS.tile([P, 2 * QG], f32, tag="s")
                            pb = probs_pool.tile([P, 2 * QG], bf16, tag="pb")
                            for (kt, off), c, w in regions:
                                nc.tensor.matmul(
                                    sps[:, c:c + w],
                                    lhsT=KT[m][po:po + D, kt * P:(kt + 1) * P],
                                    rhs=QT[m][po:po + D,
                                              g * QG + off:(g + 1) * QG],
                                    start=True, stop=True)
                            if pi == 0:
                                while carried:
                                    carried.pop(0)()
                            nc.scalar.activation(pb[:, 0:col], sps[:, 0:col],
                                                 Act.Exp,
                                                 bias=negshift[:, 0:1])
                            # masking after exp: multiplicative 0/1, exact
                            for (kt, off), c, w in regions:
                                for j in range(off // P, NQB):
                                    qb = g * NQB + j
                                    bidx = bias_idx[(kt, qb)]
                                    if bidx is None:
                                        continue
                                    cc = c + j * P - off
                                    blk = pb[:, cc:cc + P]
                                    if not block_live[kt, qb]:
                                        nc.gpsimd.memset(blk, 0.0)
                                    else:
                                        nc.vector.tensor_mul(
                                            blk, blk,
                                            bias_sb[:, bidx * P:
                                                    (bidx + 1) * P])
                            pend.append((pb, regions))
                            if len(pend) > 3:
                                emit_pv(pend.pop(0))

                        def t_pv():
                            while pend:
                                emit_pv(pend.pop(0))

                        # normalize: Zinv = 1/Z (bf16, matching the
                        # reference-passing baseline's precision).  The two
                        # heads of a pair stack their Zinv rows in one
                        # [2, QG] tile; the ODD head's chain broadcasts both
                        # with a single K=2 selector matmul into a borrowed
                        # psO bank, then multiplies both ev tiles
                        # (SBUF x PSUM, verifier-friendly).
                        def t_chain():
                            zi2 = z2[pair_ctx["idx"]]
                            zo = (h % 2) * D
                            zrow = zi2[zo:zo + 1, :]
                            if act_recip:
                                # tail path: ACT is idle there; Zinv =
                                # exp(-ln(Z)) is exact to bf16 rounding
                                zf = z_pool.tile([1, QG], f32,
                                                 tag=f"zf{h % 2}",
                                                 name=f"zf{h}")
                                nc.scalar.activation(zf, pv[D:D + 1, :],
                                                     Act.Ln)
                                nc.scalar.activation(zrow, zf, Act.Exp,
                                                     scale=-1.0)
                            else:
                                with nc.allow_low_precision(
                                        reason="bf16 Zinv, like the Wo "
                                               "operands downstream"):
                                    nc.vector.reciprocal(zrow, pv[D:D + 1, :])
                            # both heads' ev stack in one [128, QG] tile
                            # (bases 0/64) so the normalize is ONE multiply
                            if h % 2 == 0:
                                ev2 = ev_pool.tile([P, QG], f32, tag="ev2",
                                                   name=f"ev2g{g}")
                                pair_ctx["ev2"] = ev2
                            else:
                                ev2 = pair_ctx["ev2"]
                            evd = ev2[zo:zo + D, :]
                            if act_recip:
                                nc.scalar.copy(evd, pv[0:D, :])
                            else:
                                nc.vector.tensor_copy(evd, pv[0:D, :])
                            if h % 2 == 1:
                                bps2 = psO.tile([P, QG], f32, tag="o",
                                                name=f"bps{h}")
                                nc.tensor.matmul(bps2, lhsT=sel2, rhs=zi2,
                                                 start=True, stop=True)
                                nc.vector.tensor_mul(
                                    attnP[h // 2][g], ev2, bps2)

                        if defer:
                            return [t_pv, t_chain]
                        t_pv()
                        t_chain()
                        return []

                    def wo_proj(g, last=False):
                        # For the final group the psS pool is free (no more
                        # scores), so use its 2-bank tiles to double the
                        # psum buffering and shorten the drain.
                        for j in range(NQB):
                            st = g * NQB + j
                            ot = out_pool.tile([P, NEG * EGW], bf16, tag="ot")
                            if last:
                                ops2 = psS.tile([P, 2 * QG], f32, tag="s",
                                                name="opss")
                            for eg in range(NEG):
                                if last:
                                    ops = ops2[:, eg * EGW:(eg + 1) * EGW]
                                else:
                                    ops = psO.tile([P, EGW], f32, tag="o",
                                                   name="opso")
                                for p in range(HL // 2):
                                    nc.tensor.matmul(
                                        ops,
                                        lhsT=attnP[p][g][:, j * P:(j + 1) * P],
                                        rhs=wo_sb[p][:, eg * EGW:
                                                     (eg + 1) * EGW],
                                        start=(p == 0),
                                        stop=(p == HL // 2 - 1),
                                        skip_group_check=last)
                                # copy PSUM->SBUF (GPSIMD cannot touch PSUM);
                                # at the tail the idle ACT engine takes half
                                dst = ot[:, eg * EGW:(eg + 1) * EGW]
                                if eg == 0:
                                    nc.scalar.copy(dst, ops)
                                else:
                                    nc.vector.tensor_copy(dst, ops)
                                if last:
                                    nc.sync.dma_start(
                                        out=d_out[st * P:(st + 1) * P,
                                                  eg * EGW:(eg + 1) * EGW],
                                        in_=dst)
                            if not last:
                                nc.sync.dma_start(
                                    out=d_out[st * P:(st + 1) * P, :], in_=ot)

                    # m1 projection piece for one (weight, group): psum
                    # borrowed from the (pass-1-unused) psO pool; e-inner so
                    # the two psO buffers ping-pong.
                    def m1_piece(wname, g):
                        dst = QT if wname == "wq" else KT
                        chunks = qchunks if wname == "wq" else kchunks
                        ps = psO.tile([P, QG], f32, tag="o",
                                      name=f"psqk1{wname}{g}")
                        for e in range(EC):
                            nc.tensor.matmul(
                                ps,
                                lhsT=w_sb[wname][:, e, P:2 * P],
                                rhs=chunks[e][:, g * QG:(g + 1) * QG],
                                start=(e == 0), stop=(e == EC - 1))
                        nc.vector.tensor_copy(
                            dst[1][:, g * QG:(g + 1) * QG], ps)

                    # pass 1: heads 0/1 over all groups (needs only m0 + V),
                    # ascending (ends on the biggest exp backlog).  The 8 m1
                    # projection pieces are woven between the later head
                    # visits, where the exp backlog hides their ACT-less PE
                    # time; PV tails and normalize chains are carried into
                    # the next head so ACT never waits on them.
                    g_up = sorted(range(NQG), key=lambda g: len(kt_offs(g)))
                    m1_sched = {1: ["wq0"], 2: ["wq1"], 3: ["wq2"],
                                4: ["wq3"], 5: ["wk0", "wk1"],
                                6: ["wk2"], 7: ["wk3"]}
                    carry = []
                    for i, g in enumerate(g_up):
                        pctx = {"idx": i % 2}
                        for h in (0, 1):
                            carry = attn(g, h, carry=carry, defer=True,
                                         pair_ctx=pctx)
                            for pc in m1_sched.get(2 * i + h, []):
                                m1_piece("wq" if pc[:2] == "wq" else "wk",
                                         int(pc[2]))
                    for t in carry:
                        t()

                    # pass 2: heads 2/3 + output projection per group;
                    # descending so the big group lands right after m1 and
                    # the kernel tail ends on the smallest one.  Each group's
                    # h3 tail + output projection are carried into the next
                    # group's first score matmuls; the last group runs its
                    # reciprocals on the (by then idle) ACT engine and
                    # interleaves its two normalize chains to cut the drain.
                    carry = []
                    gs2 = list(reversed(g_up))
                    for i, g in enumerate(gs2):
                        last = (i == NQG - 1)
                        pctx = {"idx": i % 2}
                        c2 = attn(g, 2, carry=carry, defer=True,
                                  pair_ctx=pctx)
                        c3 = attn(g, 3, carry=c2 if last else c2[:1],
                                  defer=True, act_recip=last, pair_ctx=pctx)
                        if not last:
                            # [pv3, chain2, chain3, wo(g)] ride into the next
                            # group's emission
                            carry = [c3[0], c2[1], c3[1],
                                     lambda g=g: wo_proj(g)]
                        else:
                            c3[0]()
                            c3[1]()
                            wo_proj(g, last=True)

        for _rep in range(repeat):
            emit_once()

    _split_multi_waits(nc)
    return nc


# ---------------------------------------------------------------------------
# Host entry point
# ---------------------------------------------------------------------------
LAST_EXEC_NS = None
LAST_RESULT = None


def kernel(query, key, value, mask, Wq, Wk, Wv, Wo, bo):
    global LAST_EXEC_NS, LAST_RESULT
    _install_tile_drain_patch()
    from concourse.bass_utils import run_bass_kernel_spmd

    B, S, E = 2, 2048, 1024
    H, D = 16, 64
    N_CORES = 8
    BG = 2                    # batch groups
    HG = N_CORES // BG        # head groups per batch
    HL = H // HG              # heads per core
    DIM = HL * D

    query = np.asarray(query, dtype=np.float32)
    key = np.asarray(key, dtype=np.float32)
    value = np.asarray(value, dtype=np.float32)
    mask2d = np.asarray(mask).reshape(S, S).astype(bool)
    Wq = np.asarray(Wq, dtype=np.float32)
    Wk = np.asarray(Wk, dtype=np.float32)
    Wv = np.asarray(Wv, dtype=np.float32)
    Wo = np.asarray(Wo, dtype=np.float32)
    bo = np.asarray(bo, dtype=np.float32)

    bias_idx, biases, block_live = classify_mask(mask2d, S)
    nuniq = len(biases)
    bias_stack = (np.concatenate(biases, axis=1) if nuniq
                  else np.zeros((128, 128), np.float32))

    nc = build_nc(S, E, D, HL, bias_idx, block_live, nuniq)

    scale = np.float32(1.0 / np.sqrt(D))
    in_maps = []
    for c in range(N_CORES):
        b, hg = c // HG, c % HG
        cols = slice(hg * DIM, (hg + 1) * DIM)
        wv_l = Wv[:, cols].reshape(E, HL, D)
        wv_aug = np.zeros((E, HL, D + 1), np.float32)
        wv_aug[:, :, :D] = wv_l
        in_maps.append({
            "xqT": _bf16(query[b].T),
            "xkT": _bf16(key[b].T),
            "xvT": _bf16(value[b].T),
            "wq": _bf16(Wq[:, cols] * scale),
            "wk": _bf16(Wk[:, cols]),
            "wv": _bf16(wv_aug.reshape(E, HL * (D + 1))),
            "wo": _bf16(Wo[cols, :]),
            "biasT": _bf16(bias_stack),
        })

    res = run_bass_kernel_spmd(nc, in_maps, list(range(N_CORES)))
    LAST_RESULT = res
    LAST_EXEC_NS = res.exec_time_ns or res.mean_exec_time_ns

    out = np.empty((B, S, E), np.float32)
    for b in range(BG):
        acc = res.results[b * HG]["out_p"].astype(np.float32)
        for j in range(1, HG):
            acc = acc + res.results[b * HG + j]["out_p"]
        out[b] = acc + bo[None, :]
    return out


def _bf16(a):
    import ml_dtypes
    return np.ascontiguousarray(np.asarray(a, np.float32)).astype(
        ml_dtypes.bfloat16)


# revision 15
# speedup vs baseline: 1.2075x; 1.0043x over previous
"""Multi-head attention (B=2, S=2048, E=1024, H=16) on 8 Trainium2 NeuronCores.

Sharding: core c -> batch c//4, heads 4*(c%4)..4*(c%4)+3  (data + head parallel).
Each core computes a partial output projection [S, E] over its 256 head-dims;
the host sums the 4 partials per batch and adds the output bias.

v2 redesign (driven by the TimelineSim cost model, where every engine op
costs free_size * cycle_t and matmuls cost out_free * 0.4167ns per
contraction chunk; ~149us vs the 179us v1):
  * scores/exp/PV are TRIMMED to the live (causal) columns of each key
    tile: per (group, kt) only q-columns >= the first live q-block are
    computed, masked blocks inside that range get the 0/1 multiply.
  * Zinv = DVE reciprocal of the PV ones-row (bf16); the two heads of a
    pair stack their Zinv rows at partitions 0/64 of a persistent zeroed
    tile and ONE K=65 selector matmul broadcasts both into a borrowed psO
    bank; both heads' ev also stack (bases 0/64) so the pair normalizes
    with a single full-width SBUF x PSUM multiply.  (GPSIMD can't touch PSUM; partition_broadcast needs a
    gpsimd library; the verifier rejects two-PSUM-input TensorTensor and
    partition bases other than 0/32/64; matmul over uninitialized SBUF
    rows NaNs even against zero weights - hence the persistent memset.)
  * Output partials leave in bf16 (halves the out-DMA; the host sums the
    4 partials per batch in fp32); output psum->sbuf copies split ACT/DVE,
    the ev copies ride DVE so the in-order ACT queue carries only exps.
  * Phase order: V e-inner (PE runs continuously once xv lands, ramping
    to the full p-state), m0 e-outer, pass 1 (heads 0/1, groups
    ascending) with the 8 m1-projection pieces woven between later head
    visits, pass 2 (heads 2/3, descending) with each group's output
    projection deferred into the next group's emission.  Every head's PV
    tail + normalize chain is carried into the following head's first
    score matmuls so the in-order engine queues never stall on them; the
    last group batches its chains, runs Zinv on the idle ACT engine, and
    drains its output projection through the freed scores psum tiles.
"""

import sys

for _p in ("/opt/trn_rl_repo", "/root/.axon_site/_ro/trn_rl_repo"):
    if _p not in sys.path:
        sys.path.insert(0, _p)

import numpy as np


# ---------------------------------------------------------------------------
# Patch: the walrus build in this container rejects >1 sem wait on one CTRL
# instruction and the TileContext exit drain aggregates every outstanding
# proc's wait onto a single Drain. Spill the excess waits onto SP nops.
# ---------------------------------------------------------------------------
def _install_tile_drain_patch():
    import concourse.tile as tile
    import concourse.mybir as mybir
    from concourse.vector_clock import ScopedClock

    if getattr(tile.TileContext, "_drain_patch_installed", False):
        return

    def _patched_drain_and_barrier(self, tick_clock, wait_clock):
        drain_inst = self.nc.sync.drain()
        wait_clock.add_sem_waits(
            drain_inst.ins, ScopedClock({None: tick_clock.global_clock})
        )
        si = drain_inst.ins.sync_info
        waits = list(si.on_wait) if si and si.on_wait else []
        if len(waits) > 1:
            si.on_wait = waits[:1]
            for w in waits[1:]:
                nop = self.nc.sync.nop(nofuse=True, hint="drain_wait_spill")
                nop.ins.sync_info = mybir.SyncInfo(on_wait=[w], on_update=[])
        self.nc.all_engine_barrier()
        assert self.sems is not None
        popped = self.nc._tile_sem_poison_stack.pop()
        assert popped is self._sem_poison
        self.nc.clear_and_free_semaphores(list(self.sems.allocated().values()))
        self.nc.all_engine_barrier()

    tile.TileContext._drain_and_barrier = _patched_drain_and_barrier
    tile.TileContext._drain_patch_installed = True


def _split_multi_waits(nc, maxw=1):
    """Hoist excess sem waits onto engine-queue NoOps inserted just before
    the instruction (sequencer executes them in order; semantics identical)."""
    import concourse.mybir as mybir

    ctr = 0
    for bb in nc.main_func.blocks:
        new = []
        for inst in bb.instructions:
            si = inst.sync_info
            waits = list(si.on_wait) if si and si.on_wait else []
            if len(waits) > maxw:
                extras = waits[:-maxw]
                si.on_wait = waits[-maxw:]
                for i in range(0, len(extras), maxw):
                    nop = mybir.InstNoOp(
                        name=f"I-waitspill-{ctr}", engine=inst.engine,
                        ins=[], outs=[])
                    ctr += 1
                    nop.sync_info = mybir.SyncInfo(
                        on_wait=extras[i:i + maxw], on_update=[])
                    try:
                        nc.register_instruction(nop, overwrite=True)
                    except Exception:
                        pass
                    new.append(nop)
            new.append(inst)
        bb.instructions = new


# ---------------------------------------------------------------------------
# Mask classification (host side, from the actual mask array).
# Blocks are 128x128 in the *transposed* score layout: block (kt, qb) covers
# keys kt*128.. x queries qb*128...
# ---------------------------------------------------------------------------
def classify_mask(mask2d, S, KB=128):
    nb = S // KB
    assert mask2d.shape == (S, S)
    assert mask2d.any(axis=1).all(), "a query row with no attended key"
    maskT = mask2d.T  # [keys, q]
    uniq = {}
    biases = []
    bias_idx = {}  # (kt, qb) -> None (all attended) or index
    block_live = np.zeros((nb, nb), dtype=bool)  # any attended key in block
    for kt in range(nb):
        for qb in range(nb):
            blk = maskT[kt * KB:(kt + 1) * KB, qb * KB:(qb + 1) * KB]
            if blk.all():
                bias_idx[(kt, qb)] = None
                block_live[kt, qb] = True
            else:
                b = np.where(blk, np.float32(1.0), np.float32(0.0))
                key = b.tobytes()
                if key not in uniq:
                    uniq[key] = len(biases)
                    biases.append(b)
                bias_idx[(kt, qb)] = uniq[key]
                block_live[kt, qb] = blk.any()
    return bias_idx, biases, block_live


# ---------------------------------------------------------------------------
# Bass program builder (one SPMD program, same for all cores).
# ---------------------------------------------------------------------------
def build_nc(S, E, D, HL, bias_idx, block_live, nuniq, shift=32.0, repeat=1):
    import concourse.bass as bass
    import concourse.mybir as mybir
    import concourse.tile as tile

    f32 = mybir.dt.float32
    bf16 = mybir.dt.bfloat16
    Act = mybir.ActivationFunctionType

    P = 128
    EC = E // P              # E chunks (contraction tiles for projections)
    DIM = HL * D             # this core's head dims (256)
    MT = DIM // P            # m-tiles of QT/KT (2)
    QG = 512                 # q-group width
    NQG = S // QG
    NQB = QG // P            # q-blocks per group
    NKT = S // P             # key tiles
    NST = S // P             # s tiles
    VW = HL * (D + 1)        # V width incl. ones columns (260)
    EGW = min(QG, E)         # output E slice width
    NEG = E // EGW           # output E slices (2)

    # live key tiles of group g with the first live q-block column offset;
    # the first entry is widened to cover every column any later kt writes so
    # its start=True matmul initializes the whole accumulation region.
    def kt_offs(g):
        out = []
        for kt in range(NKT):
            lives = [j for j in range(NQB) if block_live[kt, g * NQB + j]]
            if lives:
                out.append((kt, lives[0] * P))
        if out:
            m0 = min(o for _, o in out)
            out[0] = (out[0][0], m0)
        return out

    nc = bass.Bass()
    dp = nc.declare_dram_parameter
    d_xq = dp("xqT", [E, S], bf16, isOutput=False)
    d_xk = dp("xkT", [E, S], bf16, isOutput=False)
    d_xv = dp("xvT", [E, S], bf16, isOutput=False)
    d_wq = dp("wq", [E, DIM], bf16, isOutput=False)
    d_wk = dp("wk", [E, DIM], bf16, isOutput=False)
    d_wv = dp("wv", [E, VW], bf16, isOutput=False)
    d_wo = dp("wo", [DIM, E], bf16, isOutput=False)
    d_bias = dp("biasT", [P, max(nuniq, 1) * P], bf16, isOutput=False)
    d_out = dp("out_p", [S, E], bf16, isOutput=True)

    import contextlib
    with tile.TileContext(nc) as tc, contextlib.ExitStack() as _stk:
        consts = _stk.enter_context(tc.tile_pool(name="consts", bufs=1))

        w_sb = {}
        for nm, width in (("wq", DIM), ("wk", DIM), ("wv", VW)):
            w_sb[nm] = consts.tile([P, EC, width], bf16, name=f"sb_{nm}",
                                   tag=f"sb_{nm}")
        w_dram = {"wq": d_wq, "wk": d_wk, "wv": d_wv}
        wo_sb = [consts.tile([2 * D, E], bf16, name=f"sb_wo{p}",
                             tag=f"sb_wo{p}") for p in range(HL // 2)]
        bias_sb = consts.tile([P, max(nuniq, 1) * P], bf16, name="sb_bias")
        negshift = consts.tile([P, 1], f32, name="negshift")
        nc.vector.memset(negshift, -shift)
        # selector for the pair Zinv broadcast: the two heads' Zinv rows
        # live at partitions 0 and 64 (the only legal cross-partition write
        # offsets); out rows 0..63 copy row 0, rows 64..127 copy row 64
        sel2 = consts.tile([D + 1, P], bf16, name="sel2")
        nc.vector.memset(sel2, 0.0)
        nc.vector.memset(sel2[0:1, 0:D], 1.0)
        nc.vector.memset(sel2[D:D + 1, D:P], 1.0)

        def load_w(nm):
            nc.sync.dma_start(
                out=w_sb[nm],
                in_=w_dram[nm][:, :].rearrange("(e p) n -> p e n", p=P))

        def emit_once():
            # persistent projection outputs
            QT = [consts.tile([P, S], bf16, name=f"QT{m}", tag=f"QT{m}")
                  for m in range(MT)]
            KT = [consts.tile([P, S], bf16, name=f"KT{m}", tag=f"KT{m}")
                  for m in range(MT)]
            V = [consts.tile([P, VW], bf16, name=f"V{s}", tag=f"V{s}")
                 for s in range(NST)]
            attnP = [[consts.tile([2 * D, QG], bf16, name=f"attnP{p}g{g}",
                                  tag=f"attnP{p}g{g}") for g in range(NQG)]
                     for p in range(HL // 2)]
            # persistent, fully-zeroed pair-Zinv tiles (rows 1..63 must be
            # finite zeros: the K=65 selector matmul reads every partition)
            z2 = [consts.tile([D + 1, QG], bf16, name=f"zi2_{i}",
                              tag=f"zi2_{i}") for i in range(2)]
            for t2 in z2:
                nc.vector.memset(t2, 0.0)

            with tc.tile_pool(name="xt", bufs=2 * EC + 4) as xt_pool:
                psA_ctx = contextlib.ExitStack()
                psV = psA_ctx.enter_context(
                    tc.tile_pool(name="psV", bufs=1, space="PSUM"))
                psM = psA_ctx.enter_context(
                    tc.tile_pool(name="psM", bufs=1, space="PSUM"))

                def stream_chunks(dram):
                    chunks = []
                    for e in range(EC):
                        ch = xt_pool.tile([P, S], bf16, tag="xt", name=f"xch{e}")
                        nc.sync.dma_start(out=ch, in_=dram[e * P:(e + 1) * P, :])
                        chunks.append(ch)
                    return chunks

                # ---------------- phase A: projections ----------------
                # wv chunk 0 first so the first V matmul only waits ~1.6us
                nc.sync.dma_start(out=w_sb["wv"][:, 0, :], in_=d_wv[0:P, :])
                vchunks = [xt_pool.tile([P, S], bf16, tag="xt", name="xch0")]
                nc.sync.dma_start(out=vchunks[0], in_=d_xv[0:P, :])
                nc.sync.dma_start(
                    out=w_sb["wv"][:, 1:EC, :],
                    in_=d_wv[P:, :].rearrange("(e p) n -> p e n", p=P))
                for e in range(1, EC):
                    ch = xt_pool.tile([P, S], bf16, tag="xt", name=f"xch{e}")
                    nc.sync.dma_start(out=ch, in_=d_xv[e * P:(e + 1) * P, :])
                    vchunks.append(ch)
                load_w("wq")
                qchunks = stream_chunks(d_xq)
                load_w("wk")
                kchunks = stream_chunks(d_xk)
                for p in range(HL // 2):
                    nc.sync.dma_start(
                        out=wo_sb[p], in_=d_wo[p * 2 * D:(p + 1) * 2 * D, :])
                nc.sync.dma_start(out=bias_sb, in_=d_bias[:, :])

                def finish_v(st, ps):
                    nc.vector.tensor_copy(V[st], ps)
                    onescols = V[st].rearrange(
                        "p (h c) -> p h c", c=D + 1)[:, :, D]
                    nc.gpsimd.memset(onescols, 1.0)

                # V tiles e-inner, starting once the whole xv stream has
                # landed (~11.4us): the PE then runs CONTINUOUSLY (ramping to
                # the full p-state) through V and the m0 projections while
                # xq/xk stream in, instead of chunk-paced stuttering.
                for st in range(0, NST):
                    ps = psV.tile([P, VW], f32, tag=f"v{st % 4}",
                                  name=f"psv{st}")
                    for e in range(EC):
                        nc.tensor.matmul(
                            ps,
                            lhsT=vchunks[e][:, st * P:(st + 1) * P],
                            rhs=w_sb["wv"][:, e, :],
                            start=(e == 0), stop=(e == EC - 1))
                    finish_v(st, ps)

                # QK projections for m-tile m into QT[m]/KT[m]; e-outer over
                # the 4 q-groups of each (weight, group) so PE work is
                # chunk-paced.  `pool`/`tag` chooses the psum bank set.
                def qk_proj(m, pool, tagf):
                    for wname, dst, chunks in (("wq", QT, qchunks),
                                               ("wk", KT, kchunks)):
                        pss = [pool.tile([P, QG], f32, tag=tagf(g),
                                         name=f"psqk{m}{wname}{g}")
                               for g in range(NQG)]
                        for e in range(EC):
                            for g in range(NQG):
                                nc.tensor.matmul(
                                    pss[g],
                                    lhsT=w_sb[wname][:, e, m * P:(m + 1) * P],
                                    rhs=chunks[e][:, g * QG:(g + 1) * QG],
                                    start=(e == 0), stop=(e == EC - 1))
                        for g in range(NQG):
                            # the first two K copies gate the first attention
                            # scores (data + psS bank WAR); run them on the
                            # still-idle ACT engine, in parallel with DVE
                            if wname == "wk" and g < 2:
                                nc.scalar.copy(
                                    dst[m][:, g * QG:(g + 1) * QG], pss[g])
                            else:
                                nc.vector.tensor_copy(
                                    dst[m][:, g * QG:(g + 1) * QG], pss[g])

                qk_proj(0, psM, lambda g: f"m{g}")
                psA_ctx.close()  # free the 8 phase-A PSUM banks for phase B

                # ---------------- phase B: attention ----------------
                # PSUM bank alignment: psPV/psO open first so they land on
                # the psV banks (free early); psS lands on the psM banks,
                # which free exactly when the m0 copies complete.
                with tc.tile_pool(name="probs", bufs=6) as probs_pool, \
                     tc.tile_pool(name="zrow", bufs=2) as z_pool, \
                     tc.tile_pool(name="evs", bufs=2) as ev_pool, \
                     tc.tile_pool(name="outst", bufs=16) as out_pool, \
                     tc.tile_pool(name="psS", bufs=2, space="PSUM") as psS, \
                     tc.tile_pool(name="psPV", bufs=2, space="PSUM") as psPV, \
                     tc.tile_pool(name="psO", bufs=2, space="PSUM") as psO:

                    def attn(g, h, carry=(), defer=False, act_recip=False,
                             pair_ctx=None):
                        """Emit attention for (g, h).  `carry` holds the
                        previous head's deferred PV-flush + normalize chain;
                        it is emitted right after this head's first score
                        matmuls so the ACT engine sees the next exp without
                        waiting for the previous head's PV tail.  With
                        defer=True the tail thunks are returned instead of
                        emitted."""
                        m, po = h // 2, (h % 2) * D
                        kts = kt_offs(g)
                        total = len(kts)
                        pairs = [kts[i:i + 2] for i in range(0, total, 2)]
                        pv = psPV.tile([D + 1, QG], f32, tag="pv")
                        npv = 0
                        pend = []
                        carried = list(carry)

                        def emit_pv(entry):
                            nonlocal npv
                            pb, regions = entry
                            for (kt, off), c, w in regions:
                                nc.tensor.matmul(
                                    pv[0:D + 1, off:QG],
                                    lhsT=V[kt][:, h * (D + 1):
                                               (h + 1) * (D + 1)],
                                    rhs=pb[:, c:c + w],
                                    start=(npv == 0),
                                    stop=(npv == total - 1),
                                    skip_group_check=True)
                                npv += 1

                        for pi, pair in enumerate(pairs):
                            regions = []
                            col = 0
                            for (kt, off) in pair:
                                w = QG - off
                                regions.append(((kt, off), col, w))
                                col += w
                            sps = psS.tile([P, 2 * QG], f32, tag="s")
                            pb = probs_pool.tile([P, 2 * QG], bf16, tag="pb")
                            for (kt, off), c, w in regions:
                                nc.tensor.matmul(
                                    sps[:, c:c + w],
                                    lhsT=KT[m][po:po + D, kt * P:(kt + 1) * P],
                                    rhs=QT[m][po:po + D,
                                              g * QG + off:(g + 1) * QG],
                                    start=True, stop=True)
                            if pi == 0:
                                while carried:
                                    carried.pop(0)()
                            nc.scalar.activation(pb[:, 0:col], sps[:, 0:col],
                                                 Act.Exp,
                                                 bias=negshift[:, 0:1])
                            # masking after exp: multiplicative 0/1, exact
                            for (kt, off), c, w in regions:
                                for j in range(off // P, NQB):
                                    qb = g * NQB + j
                                    bidx = bias_idx[(kt, qb)]
                                    if bidx is None:
                                        continue
                                    cc = c + j * P - off
                                    blk = pb[:, cc:cc + P]
                                    if not block_live[kt, qb]:
                                        nc.gpsimd.memset(blk, 0.0)
                                    else:
                                        nc.vector.tensor_mul(
                                            blk, blk,
                                            bias_sb[:, bidx * P:
                                                    (bidx + 1) * P])
                            pend.append((pb, regions))
                            if len(pend) > 3:
                                emit_pv(pend.pop(0))

                        def t_pv():
                            while pend:
                                emit_pv(pend.pop(0))

                        # normalize: Zinv = 1/Z (bf16, matching the
                        # reference-passing baseline's precision).  The two
                        # heads of a pair stack their Zinv rows in one
                        # [2, QG] tile; the ODD head's chain broadcasts both
                        # with a single K=2 selector matmul into a borrowed
                        # psO bank, then multiplies both ev tiles
                        # (SBUF x PSUM, verifier-friendly).
                        def t_chain():
                            zi2 = z2[pair_ctx["idx"]]
                            zo = (h % 2) * D
                            zrow = zi2[zo:zo + 1, :]
                            if act_recip:
                                # tail path: ACT is idle there; Zinv =
                                # exp(-ln(Z)) is exact to bf16 rounding
                                zf = z_pool.tile([1, QG], f32,
                                                 tag=f"zf{h % 2}",
                                                 name=f"zf{h}")
                                nc.scalar.activation(zf, pv[D:D + 1, :],
                                                     Act.Ln)
                                nc.scalar.activation(zrow, zf, Act.Exp,
                                                     scale=-1.0)
                            else:
                                with nc.allow_low_precision(
                                        reason="bf16 Zinv, like the Wo "
                                               "operands downstream"):
                                    nc.vector.reciprocal(zrow, pv[D:D + 1, :])
                            # both heads' ev stack in one [128, QG] tile
                            # (bases 0/64) so the normalize is ONE multiply
                            if h % 2 == 0:
                                ev2 = ev_pool.tile([P, QG], f32, tag="ev2",
                                                   name=f"ev2g{g}")
                                pair_ctx["ev2"] = ev2
                            else:
                                ev2 = pair_ctx["ev2"]
                            evd = ev2[zo:zo + D, :]
                            if act_recip:
                                nc.scalar.copy(evd, pv[0:D, :])
                            else:
                                nc.vector.tensor_copy(evd, pv[0:D, :])
                            if h % 2 == 1:
                                bps2 = psO.tile([P, QG], f32, tag="o",
                                                name=f"bps{h}")
                                nc.tensor.matmul(bps2, lhsT=sel2, rhs=zi2,
                                                 start=True, stop=True)
                                nc.vector.tensor_mul(
                                    attnP[h // 2][g], ev2, bps2)

                        if defer:
                            return [t_pv, t_chain]
                        t_pv()
                        t_chain()
                        return []

                    def wo_proj(g, last=False):
                        # For the final group the psS pool is free (no more
                        # scores), so use its 2-bank tiles to double the
                        # psum buffering and shorten the drain.
                        for j in range(NQB):
                            st = g * NQB + j
                            ot = out_pool.tile([P, NEG * EGW], bf16, tag="ot")
                            if last:
                                ops2 = psS.tile([P, 2 * QG], f32, tag="s",
                                                name="opss")
                            for eg in range(NEG):
                                if last:
                                    ops = ops2[:, eg * EGW:(eg + 1) * EGW]
                                else:
                                    ops = psO.tile([P, EGW], f32, tag="o",
                                                   name="opso")
                                for p in range(HL // 2):
                                    nc.tensor.matmul(
                                        ops,
                                        lhsT=attnP[p][g][:, j * P:(j + 1) * P],
                                        rhs=wo_sb[p][:, eg * EGW:
                                                     (eg + 1) * EGW],
                                        start=(p == 0),
                                        stop=(p == HL // 2 - 1),
                                        skip_group_check=last)
                                # copy PSUM->SBUF (GPSIMD cannot touch PSUM);
                                # at the tail the idle ACT engine takes half
                                dst = ot[:, eg * EGW:(eg + 1) * EGW]
                                if eg == 0:
                                    nc.scalar.copy(dst, ops)
                                else:
                                    nc.vector.tensor_copy(dst, ops)
                                if last:
                                    nc.sync.dma_start(
                                        out=d_out[st * P:(st + 1) * P,
                                                  eg * EGW:(eg + 1) * EGW],
                                        in_=dst)
                            if not last:
                                nc.sync.dma_start(
                                    out=d_out[st * P:(st + 1) * P, :], in_=ot)

                    # m1 projection piece for one (weight, group): psum
                    # borrowed from the (pass-1-unused) psO pool; e-inner so
                    # the two psO buffers ping-pong.
                    def m1_piece(wname, g):
                        dst = QT if wname == "wq" else KT
                        chunks = qchunks if wname == "wq" else kchunks
                        ps = psO.tile([P, QG], f32, tag="o",
                                      name=f"psqk1{wname}{g}")
                        for e in range(EC):
                            nc.tensor.matmul(
                                ps,
                                lhsT=w_sb[wname][:, e, P:2 * P],
                                rhs=chunks[e][:, g * QG:(g + 1) * QG],
                                start=(e == 0), stop=(e == EC - 1))
                        nc.vector.tensor_copy(
                            dst[1][:, g * QG:(g + 1) * QG], ps)

                    # pass 1: heads 0/1 over all groups (needs only m0 + V),
                    # ascending (ends on the biggest exp backlog).  The 8 m1
                    # projection pieces are woven between the later head
                    # visits, where the exp backlog hides their ACT-less PE
                    # time; PV tails and normalize chains are carried into
                    # the next head so ACT never waits on them.
                    g_up = sorted(range(NQG), key=lambda g: len(kt_offs(g)))
                    m1_sched = {1: ["wq0"], 2: ["wq1"], 3: ["wq2"],
                                4: ["wq3"], 5: ["wk0", "wk1"],
                                6: ["wk2"], 7: ["wk3"]}
                    carry = []
                    for i, g in enumerate(g_up):
                        pctx = {"idx": i % 2}
                        for h in (0, 1):
                            carry = attn(g, h, carry=carry, defer=True,
                                         pair_ctx=pctx)
                            for pc in m1_sched.get(2 * i + h, []):
                                m1_piece("wq" if pc[:2] == "wq" else "wk",
                                         int(pc[2]))
                    for t in carry:
                        t()

                    # pass 2: heads 2/3 + output projection per group;
                    # descending so the big group lands right after m1 and
                    # the kernel tail ends on the smallest one.  Each group's
                    # h3 tail + output projection are carried into the next
                    # group's first score matmuls; the last group runs its
                    # reciprocals on the (by then idle) ACT engine and
                    # interleaves its two normalize chains to cut the drain.
                    carry = []
                    gs2 = list(reversed(g_up))
                    for i, g in enumerate(gs2):
                        last = (i == NQG - 1)
                        pctx = {"idx": i % 2}
                        c2 = attn(g, 2, carry=carry, defer=True,
                                  pair_ctx=pctx)
                        c3 = attn(g, 3, carry=c2 if last else c2[:1],
                                  defer=True, act_recip=last, pair_ctx=pctx)
                        if not last:
                            # [pv3, chain2, chain3, wo(g)] ride into the next
                            # group's emission
                            carry = [c3[0], c2[1], c3[1],
                                     lambda g=g: wo_proj(g)]
                        else:
                            c3[0]()
                            c3[1]()
                            wo_proj(g, last=True)

        for _rep in range(repeat):
            emit_once()

    _split_multi_waits(nc)
    return nc


# ---------------------------------------------------------------------------
# Host entry point
# ---------------------------------------------------------------------------
LAST_EXEC_NS = None
LAST_RESULT = None


def kernel(query, key, value, mask, Wq, Wk, Wv, Wo, bo):
    global LAST_EXEC_NS, LAST_RESULT
    _install_tile_drain_patch()
    from concourse.bass_utils import run_bass_kernel_spmd

    B, S, E = 2, 2048, 1024
    H, D = 16, 64
    N_CORES = 8
    BG = 2                    # batch groups
    HG = N_CORES // BG        # head groups per batch
    HL = H // HG              # heads per core
    DIM = HL * D

    query = np.asarray(query, dtype=np.float32)
    key = np.asarray(key, dtype=np.float32)
    value = np.asarray(value, dtype=np.float32)
    mask2d = np.asarray(mask).reshape(S, S).astype(bool)
    Wq = np.asarray(Wq, dtype=np.float32)
    Wk = np.asarray(Wk, dtype=np.float32)
    Wv = np.asarray(Wv, dtype=np.float32)
    Wo = np.asarray(Wo, dtype=np.float32)
    bo = np.asarray(bo, dtype=np.float32)

    bias_idx, biases, block_live = classify_mask(mask2d, S)
    nuniq = len(biases)
    bias_stack = (np.concatenate(biases, axis=1) if nuniq
                  else np.zeros((128, 128), np.float32))

    nc = build_nc(S, E, D, HL, bias_idx, block_live, nuniq)

    scale = np.float32(1.0 / np.sqrt(D))
    in_maps = []
    for c in range(N_CORES):
        b, hg = c // HG, c % HG
        cols = slice(hg * DIM, (hg + 1) * DIM)
        wv_l = Wv[:, cols].reshape(E, HL, D)
        wv_aug = np.zeros((E, HL, D + 1), np.float32)
        wv_aug[:, :, :D] = wv_l
        in_maps.append({
            "xqT": _bf16(query[b].T),
            "xkT": _bf16(key[b].T),
            "xvT": _bf16(value[b].T),
            "wq": _bf16(Wq[:, cols] * scale),
            "wk": _bf16(Wk[:, cols]),
            "wv": _bf16(wv_aug.reshape(E, HL * (D + 1))),
            "wo": _bf16(Wo[cols, :]),
            "biasT": _bf16(bias_stack),
        })

    res = run_bass_kernel_spmd(nc, in_maps, list(range(N_CORES)))
    LAST_RESULT = res
    LAST_EXEC_NS = res.exec_time_ns or res.mean_exec_time_ns

    out = np.empty((B, S, E), np.float32)
    for b in range(BG):
        acc = res.results[b * HG]["out_p"].astype(np.float32)
        for j in range(1, HG):
            acc = acc + res.results[b * HG + j]["out_p"]
        out[b] = acc + bo[None, :]
    return out


def _bf16(a):
    import ml_dtypes
    return np.ascontiguousarray(np.asarray(a, np.float32)).astype(
        ml_dtypes.bfloat16)


# revision 16
# speedup vs baseline: 1.2109x; 1.0028x over previous
"""Multi-head attention (B=2, S=2048, E=1024, H=16) on 8 Trainium2 NeuronCores.

Sharding: core c -> batch c//4, heads 4*(c%4)..4*(c%4)+3  (data + head parallel).
Each core computes a partial output projection [S, E] over its 256 head-dims;
the host sums the 4 partials per batch and adds the output bias.

v2 redesign (driven by the TimelineSim cost model, where every engine op
costs free_size * cycle_t and matmuls cost out_free * 0.4167ns per
contraction chunk; ~149us vs the 179us v1):
  * scores/exp/PV are TRIMMED to the live (causal) columns of each key
    tile: per (group, kt) only q-columns >= the first live q-block are
    computed, masked blocks inside that range get the 0/1 multiply.
  * Zinv = DVE reciprocal of the PV ones-row (bf16); the two heads of a
    pair stack their Zinv rows at partitions 0/64 of a persistent zeroed
    tile and ONE K=65 selector matmul broadcasts both into a borrowed psO
    bank; both heads' ev also stack (bases 0/64) so the pair normalizes
    with a single full-width SBUF x PSUM multiply.  (GPSIMD can't touch PSUM; partition_broadcast needs a
    gpsimd library; the verifier rejects two-PSUM-input TensorTensor and
    partition bases other than 0/32/64; matmul over uninitialized SBUF
    rows NaNs even against zero weights - hence the persistent memset.)
  * Output partials leave in bf16 (halves the out-DMA; the host sums the
    4 partials per batch in fp32); output psum->sbuf copies split ACT/DVE,
    the ev copies ride DVE so the in-order ACT queue carries only exps.
  * Phase order: V e-inner (PE runs continuously once xv lands, ramping
    to the full p-state), m0 e-outer, pass 1 (heads 0/1, groups
    ascending) with the 8 m1-projection pieces woven between later head
    visits, pass 2 (heads 2/3, descending) with each group's output
    projection deferred into the next group's emission.  Every head's PV
    tail + normalize chain is carried into the following head's first
    score matmuls so the in-order engine queues never stall on them; the
    last group batches its chains, runs Zinv on the idle ACT engine, and
    drains its output projection through the freed scores psum tiles.
"""

import sys

for _p in ("/opt/trn_rl_repo", "/root/.axon_site/_ro/trn_rl_repo"):
    if _p not in sys.path:
        sys.path.insert(0, _p)

import numpy as np


# ---------------------------------------------------------------------------
# Patch: the walrus build in this container rejects >1 sem wait on one CTRL
# instruction and the TileContext exit drain aggregates every outstanding
# proc's wait onto a single Drain. Spill the excess waits onto SP nops.
# ---------------------------------------------------------------------------
def _install_tile_drain_patch():
    import concourse.tile as tile
    import concourse.mybir as mybir
    from concourse.vector_clock import ScopedClock

    if getattr(tile.TileContext, "_drain_patch_installed", False):
        return

    def _patched_drain_and_barrier(self, tick_clock, wait_clock):
        drain_inst = self.nc.sync.drain()
        wait_clock.add_sem_waits(
            drain_inst.ins, ScopedClock({None: tick_clock.global_clock})
        )
        si = drain_inst.ins.sync_info
        waits = list(si.on_wait) if si and si.on_wait else []
        if len(waits) > 1:
            si.on_wait = waits[:1]
            for w in waits[1:]:
                nop = self.nc.sync.nop(nofuse=True, hint="drain_wait_spill")
                nop.ins.sync_info = mybir.SyncInfo(on_wait=[w], on_update=[])
        self.nc.all_engine_barrier()
        assert self.sems is not None
        popped = self.nc._tile_sem_poison_stack.pop()
        assert popped is self._sem_poison
        self.nc.clear_and_free_semaphores(list(self.sems.allocated().values()))
        self.nc.all_engine_barrier()

    tile.TileContext._drain_and_barrier = _patched_drain_and_barrier
    tile.TileContext._drain_patch_installed = True


def _split_multi_waits(nc, maxw=1):
    """Hoist excess sem waits onto engine-queue NoOps inserted just before
    the instruction (sequencer executes them in order; semantics identical)."""
    import concourse.mybir as mybir

    ctr = 0
    for bb in nc.main_func.blocks:
        new = []
        for inst in bb.instructions:
            si = inst.sync_info
            waits = list(si.on_wait) if si and si.on_wait else []
            if len(waits) > maxw:
                extras = waits[:-maxw]
                si.on_wait = waits[-maxw:]
                for i in range(0, len(extras), maxw):
                    nop = mybir.InstNoOp(
                        name=f"I-waitspill-{ctr}", engine=inst.engine,
                        ins=[], outs=[])
                    ctr += 1
                    nop.sync_info = mybir.SyncInfo(
                        on_wait=extras[i:i + maxw], on_update=[])
                    try:
                        nc.register_instruction(nop, overwrite=True)
                    except Exception:
                        pass
                    new.append(nop)
            new.append(inst)
        bb.instructions = new


# ---------------------------------------------------------------------------
# Mask classification (host side, from the actual mask array).
# Blocks are 128x128 in the *transposed* score layout: block (kt, qb) covers
# keys kt*128.. x queries qb*128...
# ---------------------------------------------------------------------------
def classify_mask(mask2d, S, KB=128):
    nb = S // KB
    assert mask2d.shape == (S, S)
    assert mask2d.any(axis=1).all(), "a query row with no attended key"
    maskT = mask2d.T  # [keys, q]
    uniq = {}
    biases = []
    bias_idx = {}  # (kt, qb) -> None (all attended) or index
    block_live = np.zeros((nb, nb), dtype=bool)  # any attended key in block
    for kt in range(nb):
        for qb in range(nb):
            blk = maskT[kt * KB:(kt + 1) * KB, qb * KB:(qb + 1) * KB]
            if blk.all():
                bias_idx[(kt, qb)] = None
                block_live[kt, qb] = True
            else:
                b = np.where(blk, np.float32(1.0), np.float32(0.0))
                key = b.tobytes()
                if key not in uniq:
                    uniq[key] = len(biases)
                    biases.append(b)
                bias_idx[(kt, qb)] = uniq[key]
                block_live[kt, qb] = blk.any()
    return bias_idx, biases, block_live


# ---------------------------------------------------------------------------
# Bass program builder (one SPMD program, same for all cores).
# ---------------------------------------------------------------------------
def build_nc(S, E, D, HL, bias_idx, block_live, nuniq, shift=32.0, repeat=1):
    import concourse.bass as bass
    import concourse.mybir as mybir
    import concourse.tile as tile

    f32 = mybir.dt.float32
    bf16 = mybir.dt.bfloat16
    Act = mybir.ActivationFunctionType

    P = 128
    EC = E // P              # E chunks (contraction tiles for projections)
    DIM = HL * D             # this core's head dims (256)
    MT = DIM // P            # m-tiles of QT/KT (2)
    QG = 512                 # q-group width
    NQG = S // QG
    NQB = QG // P            # q-blocks per group
    NKT = S // P             # key tiles
    NST = S // P             # s tiles
    VW = HL * (D + 1)        # V width incl. ones columns (260)
    EGW = min(QG, E)         # output E slice width
    NEG = E // EGW           # output E slices (2)

    # live key tiles of group g with the first live q-block column offset;
    # the first entry is widened to cover every column any later kt writes so
    # its start=True matmul initializes the whole accumulation region.
    def kt_offs(g):
        out = []
        for kt in range(NKT):
            lives = [j for j in range(NQB) if block_live[kt, g * NQB + j]]
            if lives:
                out.append((kt, lives[0] * P))
        if out:
            m0 = min(o for _, o in out)
            out[0] = (out[0][0], m0)
        return out

    nc = bass.Bass()
    dp = nc.declare_dram_parameter
    d_xq = dp("xqT", [E, S], bf16, isOutput=False)
    d_xk = dp("xkT", [E, S], bf16, isOutput=False)
    d_xv = dp("xvT", [E, S], bf16, isOutput=False)
    d_wq = dp("wq", [E, DIM], bf16, isOutput=False)
    d_wk = dp("wk", [E, DIM], bf16, isOutput=False)
    d_wv = dp("wv", [E, VW], bf16, isOutput=False)
    d_wo = dp("wo", [DIM, E], bf16, isOutput=False)
    d_bias = dp("biasT", [P, max(nuniq, 1) * P], bf16, isOutput=False)
    d_out = dp("out_p", [S, E], bf16, isOutput=True)

    import contextlib
    with tile.TileContext(nc) as tc, contextlib.ExitStack() as _stk:
        consts = _stk.enter_context(tc.tile_pool(name="consts", bufs=1))

        w_sb = {}
        for nm, width in (("wq", DIM), ("wk", DIM), ("wv", VW)):
            w_sb[nm] = consts.tile([P, EC, width], bf16, name=f"sb_{nm}",
                                   tag=f"sb_{nm}")
        w_dram = {"wq": d_wq, "wk": d_wk, "wv": d_wv}
        wo_sb = [consts.tile([2 * D, E], bf16, name=f"sb_wo{p}",
                             tag=f"sb_wo{p}") for p in range(HL // 2)]
        bias_sb = consts.tile([P, max(nuniq, 1) * P], bf16, name="sb_bias")
        negshift = consts.tile([P, 1], f32, name="negshift")
        nc.vector.memset(negshift, -shift)
        # selector for the pair Zinv broadcast: the two heads' Zinv rows
        # live at partitions 0 and 64 (the only legal cross-partition write
        # offsets); out rows 0..63 copy row 0, rows 64..127 copy row 64
        sel2 = consts.tile([D + 1, P], bf16, name="sel2")
        nc.vector.memset(sel2, 0.0)
        nc.vector.memset(sel2[0:1, 0:D], 1.0)
        nc.vector.memset(sel2[D:D + 1, D:P], 1.0)

        def load_w(nm):
            nc.sync.dma_start(
                out=w_sb[nm],
                in_=w_dram[nm][:, :].rearrange("(e p) n -> p e n", p=P))

        def emit_once():
            # persistent projection outputs
            QT = [consts.tile([P, S], bf16, name=f"QT{m}", tag=f"QT{m}")
                  for m in range(MT)]
            KT = [consts.tile([P, S], bf16, name=f"KT{m}", tag=f"KT{m}")
                  for m in range(MT)]
            V = [consts.tile([P, VW], bf16, name=f"V{s}", tag=f"V{s}")
                 for s in range(NST)]
            attnP = [[consts.tile([2 * D, QG], bf16, name=f"attnP{p}g{g}",
                                  tag=f"attnP{p}g{g}") for g in range(NQG)]
                     for p in range(HL // 2)]
            # persistent, fully-zeroed pair-Zinv tiles (rows 1..63 must be
            # finite zeros: the K=65 selector matmul reads every partition)
            z2 = [consts.tile([D + 1, QG], bf16, name=f"zi2_{i}",
                              tag=f"zi2_{i}") for i in range(2)]
            for t2 in z2:
                nc.vector.memset(t2, 0.0)

            with tc.tile_pool(name="xt", bufs=2 * EC + 4) as xt_pool:
                psA_ctx = contextlib.ExitStack()
                psV = psA_ctx.enter_context(
                    tc.tile_pool(name="psV", bufs=1, space="PSUM"))
                psM = psA_ctx.enter_context(
                    tc.tile_pool(name="psM", bufs=1, space="PSUM"))

                def stream_chunks(dram):
                    chunks = []
                    for e in range(EC):
                        ch = xt_pool.tile([P, S], bf16, tag="xt", name=f"xch{e}")
                        nc.sync.dma_start(out=ch, in_=dram[e * P:(e + 1) * P, :])
                        chunks.append(ch)
                    return chunks

                # ---------------- phase A: projections ----------------
                load_w("wv")
                vchunks = stream_chunks(d_xv)
                load_w("wq")
                qchunks = stream_chunks(d_xq)
                load_w("wk")
                kchunks = stream_chunks(d_xk)
                for p in range(HL // 2):
                    nc.sync.dma_start(
                        out=wo_sb[p], in_=d_wo[p * 2 * D:(p + 1) * 2 * D, :])
                nc.sync.dma_start(out=bias_sb, in_=d_bias[:, :])

                def finish_v(st, ps):
                    nc.vector.tensor_copy(V[st], ps)
                    onescols = V[st].rearrange(
                        "p (h c) -> p h c", c=D + 1)[:, :, D]
                    nc.gpsimd.memset(onescols, 1.0)

                # V tiles e-inner, starting once the whole xv stream has
                # landed (~11.4us): the PE then runs CONTINUOUSLY (ramping to
                # the full p-state) through V and the m0 projections while
                # xq/xk stream in, instead of chunk-paced stuttering.
                for st in range(0, NST):
                    ps = psV.tile([P, VW], f32, tag=f"v{st % 4}",
                                  name=f"psv{st}")
                    for e in range(EC):
                        nc.tensor.matmul(
                            ps,
                            lhsT=vchunks[e][:, st * P:(st + 1) * P],
                            rhs=w_sb["wv"][:, e, :],
                            start=(e == 0), stop=(e == EC - 1))
                    finish_v(st, ps)

                # QK projections for m-tile m into QT[m]/KT[m]; e-outer over
                # the 4 q-groups of each (weight, group) so PE work is
                # chunk-paced.  `pool`/`tag` chooses the psum bank set.
                def qk_proj(m, pool, tagf):
                    for wname, dst, chunks in (("wq", QT, qchunks),
                                               ("wk", KT, kchunks)):
                        pss = [pool.tile([P, QG], f32, tag=tagf(g),
                                         name=f"psqk{m}{wname}{g}")
                               for g in range(NQG)]
                        for e in range(EC):
                            for g in range(NQG):
                                nc.tensor.matmul(
                                    pss[g],
                                    lhsT=w_sb[wname][:, e, m * P:(m + 1) * P],
                                    rhs=chunks[e][:, g * QG:(g + 1) * QG],
                                    start=(e == 0), stop=(e == EC - 1))
                        for g in range(NQG):
                            # the first two K copies gate the first attention
                            # scores (data + psS bank WAR); run them on the
                            # still-idle ACT engine, in parallel with DVE
                            if wname == "wk" and g < 2:
                                nc.scalar.copy(
                                    dst[m][:, g * QG:(g + 1) * QG], pss[g])
                            else:
                                nc.vector.tensor_copy(
                                    dst[m][:, g * QG:(g + 1) * QG], pss[g])

                qk_proj(0, psM, lambda g: f"m{g}")
                psA_ctx.close()  # free the 8 phase-A PSUM banks for phase B

                # ---------------- phase B: attention ----------------
                # PSUM bank alignment: psPV/psO open first so they land on
                # the psV banks (free early); psS lands on the psM banks,
                # which free exactly when the m0 copies complete.
                with tc.tile_pool(name="probs", bufs=6) as probs_pool, \
                     tc.tile_pool(name="zrow", bufs=2) as z_pool, \
                     tc.tile_pool(name="evs", bufs=2) as ev_pool, \
                     tc.tile_pool(name="outst", bufs=16) as out_pool, \
                     tc.tile_pool(name="psS", bufs=2, space="PSUM") as psS, \
                     tc.tile_pool(name="psPV", bufs=2, space="PSUM") as psPV, \
                     tc.tile_pool(name="psO", bufs=2, space="PSUM") as psO:

                    def attn(g, h, carry=(), defer=False, act_recip=False,
                             pair_ctx=None):
                        """Emit attention for (g, h).  `carry` holds the
                        previous head's deferred PV-flush + normalize chain;
                        it is emitted right after this head's first score
                        matmuls so the ACT engine sees the next exp without
                        waiting for the previous head's PV tail.  With
                        defer=True the tail thunks are returned instead of
                        emitted."""
                        m, po = h // 2, (h % 2) * D
                        kts = kt_offs(g)
                        total = len(kts)
                        pairs = [kts[i:i + 2] for i in range(0, total, 2)]
                        pv = psPV.tile([D + 1, QG], f32, tag="pv")
                        npv = 0
                        pend = []
                        carried = list(carry)

                        def emit_pv(entry):
                            nonlocal npv
                            pb, regions = entry
                            for (kt, off), c, w in regions:
                                nc.tensor.matmul(
                                    pv[0:D + 1, off:QG],
                                    lhsT=V[kt][:, h * (D + 1):
                                               (h + 1) * (D + 1)],
                                    rhs=pb[:, c:c + w],
                                    start=(npv == 0),
                                    stop=(npv == total - 1),
                                    skip_group_check=True)
                                npv += 1

                        for pi, pair in enumerate(pairs):
                            regions = []
                            col = 0
                            for (kt, off) in pair:
                                w = QG - off
                                regions.append(((kt, off), col, w))
                                col += w
                            sps = psS.tile([P, 2 * QG], f32, tag="s")
                            pb = probs_pool.tile([P, 2 * QG], bf16, tag="pb")
                            for (kt, off), c, w in regions:
                                nc.tensor.matmul(
                                    sps[:, c:c + w],
                                    lhsT=KT[m][po:po + D, kt * P:(kt + 1) * P],
                                    rhs=QT[m][po:po + D,
                                              g * QG + off:(g + 1) * QG],
                                    start=True, stop=True)
                            if pi == 0:
                                while carried:
                                    carried.pop(0)()
                            nc.scalar.activation(pb[:, 0:col], sps[:, 0:col],
                                                 Act.Exp,
                                                 bias=negshift[:, 0:1])
                            # masking after exp: multiplicative 0/1, exact
                            for (kt, off), c, w in regions:
                                for j in range(off // P, NQB):
                                    qb = g * NQB + j
                                    bidx = bias_idx[(kt, qb)]
                                    if bidx is None:
                                        continue
                                    cc = c + j * P - off
                                    blk = pb[:, cc:cc + P]
                                    if not block_live[kt, qb]:
                                        nc.gpsimd.memset(blk, 0.0)
                                    else:
                                        nc.vector.tensor_mul(
                                            blk, blk,
                                            bias_sb[:, bidx * P:
                                                    (bidx + 1) * P])
                            pend.append((pb, regions))
                            if len(pend) > 3:
                                emit_pv(pend.pop(0))

                        def t_pv():
                            while pend:
                                emit_pv(pend.pop(0))

                        # normalize: Zinv = 1/Z (bf16, matching the
                        # reference-passing baseline's precision).  The two
                        # heads of a pair stack their Zinv rows in one
                        # [2, QG] tile; the ODD head's chain broadcasts both
                        # with a single K=2 selector matmul into a borrowed
                        # psO bank, then multiplies both ev tiles
                        # (SBUF x PSUM, verifier-friendly).
                        def t_chain():
                            zi2 = z2[pair_ctx["idx"]]
                            zo = (h % 2) * D
                            zrow = zi2[zo:zo + 1, :]
                            if act_recip:
                                # tail path: ACT is idle there; Zinv =
                                # exp(-ln(Z)) is exact to bf16 rounding
                                zf = z_pool.tile([1, QG], f32,
                                                 tag=f"zf{h % 2}",
                                                 name=f"zf{h}")
                                nc.scalar.activation(zf, pv[D:D + 1, :],
                                                     Act.Ln)
                                nc.scalar.activation(zrow, zf, Act.Exp,
                                                     scale=-1.0)
                            else:
                                with nc.allow_low_precision(
                                        reason="bf16 Zinv, like the Wo "
                                               "operands downstream"):
                                    nc.vector.reciprocal(zrow, pv[D:D + 1, :])
                            # both heads' ev stack in one [128, QG] tile
                            # (bases 0/64) so the normalize is ONE multiply
                            if h % 2 == 0:
                                ev2 = ev_pool.tile([P, QG], f32, tag="ev2",
                                                   name=f"ev2g{g}")
                                pair_ctx["ev2"] = ev2
                            else:
                                ev2 = pair_ctx["ev2"]
                            evd = ev2[zo:zo + D, :]
                            if act_recip:
                                nc.scalar.copy(evd, pv[0:D, :])
                            else:
                                nc.vector.tensor_copy(evd, pv[0:D, :])
                            if h % 2 == 1:
                                bps2 = psO.tile([P, QG], f32, tag="o",
                                                name=f"bps{h}")
                                nc.tensor.matmul(bps2, lhsT=sel2, rhs=zi2,
                                                 start=True, stop=True)
                                nc.vector.tensor_mul(
                                    attnP[h // 2][g], ev2, bps2)

                        if defer:
                            return [t_pv, t_chain]
                        t_pv()
                        t_chain()
                        return []

                    def wo_proj(g, last=False):
                        # For the final group the psS pool is free (no more
                        # scores), so use its 2-bank tiles to double the
                        # psum buffering and shorten the drain.
                        for j in range(NQB):
                            st = g * NQB + j
                            ot = out_pool.tile([P, NEG * EGW], bf16, tag="ot")
                            if last:
                                ops2 = psS.tile([P, 2 * QG], f32, tag="s",
                                                name="opss")
                            for eg in range(NEG):
                                if last:
                                    ops = ops2[:, eg * EGW:(eg + 1) * EGW]
                                else:
                                    ops = psO.tile([P, EGW], f32, tag="o",
                                                   name="opso")
                                for p in range(HL // 2):
                                    nc.tensor.matmul(
                                        ops,
                                        lhsT=attnP[p][g][:, j * P:(j + 1) * P],
                                        rhs=wo_sb[p][:, eg * EGW:
                                                     (eg + 1) * EGW],
                                        start=(p == 0),
                                        stop=(p == HL // 2 - 1),
                                        skip_group_check=last)
                                # copy PSUM->SBUF (GPSIMD cannot touch PSUM);
                                # at the tail the idle ACT engine takes half
                                dst = ot[:, eg * EGW:(eg + 1) * EGW]
                                if eg == 0:
                                    nc.scalar.copy(dst, ops)
                                else:
                                    nc.vector.tensor_copy(dst, ops)
                                if last:
                                    nc.sync.dma_start(
                                        out=d_out[st * P:(st + 1) * P,
                                                  eg * EGW:(eg + 1) * EGW],
                                        in_=dst)
                            if not last:
                                nc.sync.dma_start(
                                    out=d_out[st * P:(st + 1) * P, :], in_=ot)

                    # m1 projection piece for one (weight, group): psum
                    # borrowed from the (pass-1-unused) psO pool; e-inner so
                    # the two psO buffers ping-pong.
                    def m1_piece(wname, g):
                        dst = QT if wname == "wq" else KT
                        chunks = qchunks if wname == "wq" else kchunks
                        ps = psO.tile([P, QG], f32, tag="o",
                                      name=f"psqk1{wname}{g}")
                        for e in range(EC):
                            nc.tensor.matmul(
                                ps,
                                lhsT=w_sb[wname][:, e, P:2 * P],
                                rhs=chunks[e][:, g * QG:(g + 1) * QG],
                                start=(e == 0), stop=(e == EC - 1))
                        nc.vector.tensor_copy(
                            dst[1][:, g * QG:(g + 1) * QG], ps)

                    # pass 1: heads 0/1 over all groups (needs only m0 + V),
                    # ascending (ends on the biggest exp backlog).  The 8 m1
                    # projection pieces are woven between the later head
                    # visits, where the exp backlog hides their ACT-less PE
                    # time; PV tails and normalize chains are carried into
                    # the next head so ACT never waits on them.
                    g_up = sorted(range(NQG), key=lambda g: len(kt_offs(g)))
                    m1_sched = {1: ["wq0"], 2: ["wq1"], 3: ["wq2"],
                                4: ["wq3"], 5: ["wk0", "wk1"],
                                6: ["wk2"], 7: ["wk3"]}
                    carry = []
                    for i, g in enumerate(g_up):
                        pctx = {"idx": i % 2}
                        for h in (0, 1):
                            carry = attn(g, h, carry=carry, defer=True,
                                         pair_ctx=pctx)
                            for pc in m1_sched.get(2 * i + h, []):
                                m1_piece("wq" if pc[:2] == "wq" else "wk",
                                         int(pc[2]))
                    for t in carry:
                        t()

                    # pass 2: heads 2/3 + output projection per group;
                    # descending so the big group lands right after m1 and
                    # the kernel tail ends on the smallest one.  Each group's
                    # h3 tail + output projection are carried into the next
                    # group's first score matmuls; the last group runs its
                    # reciprocals on the (by then idle) ACT engine and
                    # interleaves its two normalize chains to cut the drain.
                    carry = []
                    gs2 = list(reversed(g_up))
                    for i, g in enumerate(gs2):
                        last = (i == NQG - 1)
                        pctx = {"idx": i % 2}
                        c2 = attn(g, 2, carry=carry, defer=True,
                                  pair_ctx=pctx)
                        c3 = attn(g, 3, carry=c2 if last else c2[:1],
                                  defer=True, act_recip=last, pair_ctx=pctx)
                        if not last:
                            # [pv3, chain2, chain3, wo(g)] ride into the next
                            # group's emission
                            carry = [c3[0], c2[1], c3[1],
                                     lambda g=g: wo_proj(g)]
                        else:
                            c3[0]()
                            c3[1]()
                            wo_proj(g, last=True)

        for _rep in range(repeat):
            emit_once()

    _split_multi_waits(nc)
    return nc


# ---------------------------------------------------------------------------
# Host entry point
# ---------------------------------------------------------------------------
LAST_EXEC_NS = None
LAST_RESULT = None


def kernel(query, key, value, mask, Wq, Wk, Wv, Wo, bo):
    global LAST_EXEC_NS, LAST_RESULT
    _install_tile_drain_patch()
    from concourse.bass_utils import run_bass_kernel_spmd

    B, S, E = 2, 2048, 1024
    H, D = 16, 64
    N_CORES = 8
    BG = 2                    # batch groups
    HG = N_CORES // BG        # head groups per batch
    HL = H // HG              # heads per core
    DIM = HL * D

    query = np.asarray(query, dtype=np.float32)
    key = np.asarray(key, dtype=np.float32)
    value = np.asarray(value, dtype=np.float32)
    mask2d = np.asarray(mask).reshape(S, S).astype(bool)
    Wq = np.asarray(Wq, dtype=np.float32)
    Wk = np.asarray(Wk, dtype=np.float32)
    Wv = np.asarray(Wv, dtype=np.float32)
    Wo = np.asarray(Wo, dtype=np.float32)
    bo = np.asarray(bo, dtype=np.float32)

    bias_idx, biases, block_live = classify_mask(mask2d, S)
    nuniq = len(biases)
    bias_stack = (np.concatenate(biases, axis=1) if nuniq
                  else np.zeros((128, 128), np.float32))

    nc = build_nc(S, E, D, HL, bias_idx, block_live, nuniq)

    scale = np.float32(1.0 / np.sqrt(D))
    in_maps = []
    for c in range(N_CORES):
        b, hg = c // HG, c % HG
        cols = slice(hg * DIM, (hg + 1) * DIM)
        wv_l = Wv[:, cols].reshape(E, HL, D)
        wv_aug = np.zeros((E, HL, D + 1), np.float32)
        wv_aug[:, :, :D] = wv_l
        in_maps.append({
            "xqT": _bf16(query[b].T),
            "xkT": _bf16(key[b].T),
            "xvT": _bf16(value[b].T),
            "wq": _bf16(Wq[:, cols] * scale),
            "wk": _bf16(Wk[:, cols]),
            "wv": _bf16(wv_aug.reshape(E, HL * (D + 1))),
            "wo": _bf16(Wo[cols, :]),
            "biasT": _bf16(bias_stack),
        })

    res = run_bass_kernel_spmd(nc, in_maps, list(range(N_CORES)))
    LAST_RESULT = res
    LAST_EXEC_NS = res.exec_time_ns or res.mean_exec_time_ns

    out = np.empty((B, S, E), np.float32)
    for b in range(BG):
        acc = res.results[b * HG]["out_p"].astype(np.float32)
        for j in range(1, HG):
            acc = acc + res.results[b * HG + j]["out_p"]
        out[b] = acc + bo[None, :]
    return out


def _bf16(a):
    import ml_dtypes
    return np.ascontiguousarray(np.asarray(a, np.float32)).astype(
        ml_dtypes.bfloat16)
